# revision 1
# baseline (speedup 1.0000x reference)
"""Kernel builder for the dual-stream linear-attention transformer (per-core).

Layout convention:
  - "layout 1" activation: [E, N] feature-major; SBUF tiles [128, KE, C]
    (feature f = 128*k + p -> partition p, k-th slice; tokens on free dim).
  - "layout 2" activation: [N, E] token-major; SBUF tiles [128(tokens), E].
  - Residual streams live in internal DRAM as [E, N] (layout 1), streamed
    through SBUF in token chunks of C.

fp32r discipline (cfg.f32r): every matmul operand tile is declared
float32r. Producers are either DMA byte-casts (bitcast both sides) or DVE
ops (which round correctly on write). ACT must never WRITE an f32r tile
(hardware produces garbage); ACT/DVE readers view f32r tiles via
.bitcast(F32), which is exact.
"""

from dataclasses import dataclass
from contextlib import ExitStack

import numpy as np

import concourse.bass as bass
import concourse.mybir as mybir
import concourse.tile as tile

F32 = mybir.dt.float32
F32R = mybir.dt.float32r
AF = mybir.ActivationFunctionType
ALU = mybir.AluOpType

LN_EPS = 1e-5
BN_EPS = 1e-5


@dataclass
class Cfg:
    N: int = 2048
    E: int = 512
    R: int = 256
    X: int = 1024
    H: int = 8
    L: int = 3
    OUT: int = 15
    C: int = 512          # token chunk (free dim of layout-1 tiles)
    f32r: bool = True

    @property
    def KE(self):
        return self.E // 128

    @property
    def KR(self):
        return self.R // 128

    @property
    def KX(self):
        return self.X // 128

    @property
    def NC(self):
        return self.N // self.C

    @property
    def NTT(self):
        return self.C // 128  # token tiles per chunk


def host_constants(cfg):
    """Constant tensors passed as extra inputs (identical on every core)."""
    E, H = cfg.E, cfg.H
    dh = E // H
    ident = np.eye(128, dtype=np.float32)
    ones = np.ones((128, 128), dtype=np.float32)
    hmask = np.zeros((E, H), dtype=np.float32)
    for f in range(E):
        hmask[f, f // dh] = 1.0
    cmask = hmask.T.copy()
    return {"ident": ident, "ones128": ones, "hmask": hmask, "cmask": cmask}


PHASES = []


def build(nc, cfg):
    """Declare IO and build the whole program inside a TileContext."""
    c = cfg
    E, R, X, H, N, C, L = c.E, c.R, c.X, c.H, c.N, c.C, c.L
    KE, KR, KX, NC, NTT = c.KE, c.KR, c.KX, c.NC, c.NTT
    E4, E2, E8 = E // 4, E // 2, E // 8

    dt = F32
    MMDT = F32R if c.f32r else F32
    din = {}

    def inp(name, shape):
        din[name] = nc.dram_tensor(name, list(shape), dt, kind="ExternalInput")
        return din[name].ap()

    body_feats = inp("body_feats", (N, E))
    limb_feats = inp("limb_feats", (N, E))
    dw = inp("dw", (L, 4, 3, E, R))
    uw = inp("uw", (L, 4, 3, R, E))
    ub = inp("ub", (L, 4, 3, E))
    ow = inp("ow", (L, 4, E, E))
    ob = inp("ob", (L, 4, E))
    w1 = inp("w1", (L, 2, E, X))
    b1 = inp("b1", (L, 2, X))
    cw = inp("cw", (L, 2, X, 3))
    cb = inp("cb", (L, 2, X))
    bng = inp("bng", (L, 2, X))
    bnb = inp("bnb", (L, 2, X))
    w2 = inp("w2", (L, 2, X, E))
    b2 = inp("b2", (L, 2, E))
    lng = inp("lng", (L, 5, E))
    lnb = inp("lnb", (L, 5, E))
    gw1 = inp("gw1", (L, 2 * E, E4))
    gb1 = inp("gb1", (L, E4))
    gw2 = inp("gw2", (L, E4, 2))
    gb2 = inp("gb2", (L, 2))
    fw1 = inp("fw1", (2 * E, E2))
    fb1 = inp("fb1", (E2,))
    fw2 = inp("fw2", (E2, E))
    fb2 = inp("fb2", (E,))
    flng = inp("flng", (E,))
    flnb = inp("flnb", (E,))
    rw1 = inp("rw1", (E, E4))
    rb1 = inp("rb1", (E4,))
    rw2 = inp("rw2", (E4, E8))
    rb2 = inp("rb2", (E8,))
    rw3 = inp("rw3", (E8, c.OUT))
    rb3 = inp("rb3", (c.OUT,))
    ident_in = inp("ident", (128, 128))
    ones_in = inp("ones128", (128, 128))
    hmask_in = inp("hmask", (E, H))
    cmask_in = inp("cmask", (H, E))

    out_dram = nc.dram_tensor("out", [N, c.OUT], dt, kind="ExternalOutput")

    def idram(name):
        return nc.dram_tensor(name, [E, N], dt).ap().rearrange(
            "(k p) n -> p k n", p=128)

    rs = {}
    for s in ("b", "l"):
        rs[s, 0] = idram(f"r{s}0")
        for l in range(L):
            for st in (1, 2, 3):
                rs[s, (l, st)] = idram(f"r{s}_{l}_{st}")

    lowp = nc.allow_low_precision("f32r matmul operand rounding")

    with tile.TileContext(nc) as tc, ExitStack() as ctx, lowp:
        p_ = ctx.enter_context
        cst = p_(tc.tile_pool(name="cst", bufs=1))
        wbig = p_(tc.tile_pool(name="wbig", bufs=3))
        wsm = p_(tc.tile_pool(name="wsm", bufs=2))
        wcol = p_(tc.tile_pool(name="wcol", bufs=10))
        wrow = p_(tc.tile_pool(name="wrow", bufs=3))
        pa = p_(tc.tile_pool(name="pa", bufs=7))      # 8KB tiles
        pb = p_(tc.tile_pool(name="pb", bufs=3))      # 16KB tiles
        pc = p_(tc.tile_pool(name="pc", bufs=5))      # 2KB tiles
        pat = p_(tc.tile_pool(name="pat", bufs=2))    # per-attn persistents
        phl = p_(tc.tile_pool(name="phl", bufs=3))    # conv halos
        ps = p_(tc.tile_pool(name="ps", bufs=5, space="PSUM"))
        ps1 = p_(tc.tile_pool(name="ps1", bufs=2, space="PSUM"))
        psb = p_(tc.tile_pool(name="psb", bufs=1, space="PSUM"))

        v, sc, gp = nc.vector, nc.scalar, nc.gpsimd

        def mm(out, lhsT, rhs, start, stop):
            nc.tensor.matmul(out, lhsT, rhs, start=start, stop=stop)

        def F(ap):
            """fp32 view of an f32r tile (exact)."""
            return ap.bitcast(F32) if c.f32r else ap

        def M(ap):
            """f32r byte-view of an fp32 DRAM AP (for DMA byte-casts)."""
            return ap.bitcast(MMDT) if c.f32r else ap

        # ---- constants ----
        ident_t = cst.tile([128, 128], dt, tag="ident")
        nc.sync.dma_start(out=ident_t, in_=ident_in)
        ones_t = cst.tile([128, 128], MMDT, tag="ones")
        nc.sync.dma_start(out=ones_t, in_=M(ones_in))
        hmask_t = cst.tile([128, KE, H], dt, tag="hmask")
        nc.sync.dma_start(out=hmask_t,
                          in_=hmask_in.rearrange("(k p) h -> p k h", p=128))
        cmask_t = cst.tile([H, KE, 128], MMDT, tag="cmask")
        nc.sync.dma_start(out=cmask_t,
                          in_=M(cmask_in.rearrange("h (k p) -> h k p", p=128)))
        ONES_COL = ones_t[:, 0:1]
        ONES_ROW = ones_t[0:1, :]
        eps_den = cst.tile([8, 1], dt, tag="epsd")
        v.memset(eps_den, 1e-6)
        eps_ln = cst.tile([1, 1], dt, tag="epsl")
        v.memset(eps_ln, LN_EPS)

        def col_tile(src_ap, m, tag="col"):
            t = wcol.tile([128, m], dt, tag=tag)
            nc.sync.dma_start(out=t, in_=src_ap.rearrange("(m p) -> p m", p=128))
            return t

        def ln_stats_apply(xs, g_col, b_col, out_tiles, relu=False,
                           out_f32r=False):
            """LayerNorm over features (layout 1). xs: KE f32r APs [128, C]."""
            sq = pa.tile([128, KE, C], MMDT, tag="a8")
            for m in range(KE):
                v.tensor_tensor(out=sq[:, m, :], in0=F(xs[m]), in1=F(xs[m]),
                                op=ALU.mult)
            ps_s = ps1.tile([1, C], F32, tag="st")
            ps_ss = ps1.tile([1, C], F32, tag="st")
            for m in range(KE):
                mm(ps_s, ONES_COL, xs[m], start=(m == 0), stop=(m == KE - 1))
                mm(ps_ss, ONES_COL, sq[:, m, :], start=(m == 0),
                   stop=(m == KE - 1))
            arow = pc.tile([1, C], dt, tag="a2")   # mean
            brow = pc.tile([1, C], dt, tag="a2")   # msq -> var -> sd
            trow2 = pc.tile([1, C], dt, tag="a2")  # mean^2
            sc.activation(arow, ps_s, AF.Copy, scale=1.0 / E)
            sc.activation(brow, ps_ss, AF.Copy, scale=1.0 / E)
            sc.activation(trow2, arow, AF.Square)
            v.tensor_tensor(out=brow, in0=brow, in1=trow2, op=ALU.subtract)
            sc.activation(brow, brow, AF.Sqrt, bias=eps_ln[0:1, 0:1])
            srow = pc.tile([1, C], MMDT, tag="a2")
            v.reciprocal(out=srow, in_=brow)
            trow = pc.tile([1, C], MMDT, tag="a2")
            v.tensor_tensor(out=trow, in0=arow, in1=F(srow), op=ALU.mult)
            bc_s = psb.tile([128, C], F32, tag="bc")
            mm(bc_s, ONES_ROW, srow, start=True, stop=True)
            sb_s = pc.tile([128, C], dt, tag="a2")
            sc.activation(sb_s, bc_s, AF.Copy)
            bc_t = psb.tile([128, C], F32, tag="bc")
            mm(bc_t, ONES_ROW, trow, start=True, stop=True)
            sb_t = pc.tile([128, C], dt, tag="a2")
            sc.activation(sb_t, bc_t, AF.Copy)
            tmp = pa.tile([128, KE, C], dt, tag="a8")
            for m in range(KE):
                gp.tensor_tensor(out=tmp[:, m, :], in0=F(xs[m]), in1=sb_s,
                                 op=ALU.mult)
                gp.tensor_tensor(out=tmp[:, m, :], in0=tmp[:, m, :], in1=sb_t,
                                 op=ALU.subtract)
                if not out_f32r:
                    fn = AF.Relu if relu else AF.Identity
                    sc.activation(out_tiles[m], tmp[:, m, :], fn,
                                  bias=b_col[:, m:m + 1],
                                  scale=g_col[:, m:m + 1])
                elif relu:
                    tmpf = pc.tile([128, C], dt, tag="a2")
                    v.tensor_scalar(out=tmpf, in0=tmp[:, m, :],
                                    scalar1=g_col[:, m:m + 1],
                                    scalar2=b_col[:, m:m + 1],
                                    op0=ALU.mult, op1=ALU.add)
                    v.tensor_scalar_max(out_tiles[m], tmpf, 0.0)
                else:
                    v.tensor_scalar(out=out_tiles[m], in0=tmp[:, m, :],
                                    scalar1=g_col[:, m:m + 1],
                                    scalar2=b_col[:, m:m + 1],
                                    op0=ALU.mult, op1=ALU.add)

        def load_x_chunk(dram_l1, ci, tag="a8"):
            xt = pa.tile([128, KE, C], MMDT, tag=tag)
            nc.sync.dma_start(out=xt,
                              in_=M(dram_l1[:, :, ci * C:(ci + 1) * C]))
            return xt

        def store_chunk(dram_l1, ci, t):
            # stores ride the GPSIMD SWDGE queue so they never delay the
            # SP-queue loads that sit on the next phase's critical path
            gp.dma_start(out=dram_l1[:, :, ci * C:(ci + 1) * C], in_=t)

        # ---- entry transpose ----
        def entry(x_ap, dst):
            for ttk in range(N // 128):
                x2 = pa.tile([128, E], dt, tag="a8")
                nc.sync.dma_start(out=x2, in_=x_ap[ttk * 128:(ttk + 1) * 128, :])
                xt = pa.tile([128, KE, 128], dt, tag="a8")
                for f in range(KE):
                    pt = ps.tile([128, 128], F32, tag="mm")
                    nc.tensor.transpose(pt, x2[:, f * 128:(f + 1) * 128],
                                        ident_t)
                    sc.activation(xt[:, f, :], pt, AF.Copy)
                nc.sync.dma_start(out=dst[:, :, ttk * 128:(ttk + 1) * 128],
                                  in_=xt)

        PHASES.append(("entry", len(nc.inst_map)))
        entry(body_feats, rs["b", 0])
        entry(limb_feats, rs["l", 0])

        # ---- linear attention ----
        def attn(l, a, xq_dram, xkv_dram, tail):
            """tail(ci, proj_tiles(f32r, ob added), xq(f32r))."""
            dwt = wbig.tile([128, KE, 3, R], MMDT, tag="w")
            for t3 in range(3):
                nc.sync.dma_start(
                    out=dwt[:, :, t3, :],
                    in_=M(dw[l, a, t3].rearrange("(k p) r -> p k r", p=128)))
            uwt = wbig.tile([128, KR, 3, E], MMDT, tag="w")
            for t3 in range(3):
                nc.sync.dma_start(
                    out=uwt[:, :, t3, :],
                    in_=M(uw[l, a, t3].rearrange("(k p) e -> p k e", p=128)))
            owt = wbig.tile([128, KE, E], MMDT, tag="w")
            nc.sync.dma_start(
                out=owt, in_=M(ow[l, a].rearrange("(k p) e -> p k e", p=128)))
            ubq_col = col_tile(ub[l, a, 0], KE)
            ubk_row = wrow.tile([1, E], MMDT, tag="row")
            nc.sync.dma_start(out=ubk_row, in_=M(ub[l, a, 1][None, :]))
            ubv_row = wrow.tile([1, E], MMDT, tag="row")
            nc.sync.dma_start(out=ubv_row, in_=M(ub[l, a, 2][None, :]))
            ob_col = col_tile(ob[l, a], KE)

            PHASES.append((f"attn{l}.{a}.alpha", len(nc.inst_map)))
            kv_acc = pat.tile([128, 4, 258], dt, tag="kva")

            # alpha: k/v -> kv, ksum (ones column appended to v)
            for ci in range(NC):
                xt = load_x_chunk(xkv_dram, ci)
                lowk = pa.tile([128, KR, C], MMDT, tag="a8")
                lowv = pa.tile([128, KR, C], MMDT, tag="a8")
                for t, low in ((1, lowk), (2, lowv)):
                    pls = [ps.tile([128, C], F32, tag="mm", name=f"pl{_i}")
                           for _i in range(KR)]
                    for k in range(KE):
                        for m in range(KR):
                            mm(pls[m], dwt[:, k, t, m * 128:(m + 1) * 128],
                               xt[:, k, :], start=(k == 0), stop=(k == KE - 1))
                    for m in range(KR):
                        v.tensor_copy(low[:, m, :], pls[m])
                k2f = pa.tile([128, NTT, E], MMDT, tag="a8")
                v2x = pa.tile([128, NTT, 2, 258], MMDT, tag="a8")
                v.memset(F(v2x[:, :, :, 256:258]), 1.0)
                for tt in range(NTT):
                    pk = ps.tile([128, E], F32, tag="mm")
                    pv = ps.tile([128, E], F32, tag="mm")
                    for k in range(KR):
                        mm(pk, lowk[:, k, tt * 128:(tt + 1) * 128],
                           uwt[:, k, 1, :], start=(k == 0), stop=False)
                        mm(pv, lowv[:, k, tt * 128:(tt + 1) * 128],
                           uwt[:, k, 2, :], start=(k == 0), stop=False)
                    mm(pk, ONES_ROW, ubk_row, start=False, stop=True)
                    mm(pv, ONES_ROW, ubv_row, start=False, stop=True)
                    ee = pc.tile([128, E], dt, tag="a2")
                    rr = pc.tile([128, E], dt, tag="a2")
                    sc.activation(ee, pk, AF.Exp)
                    sc.activation(rr, pk, AF.Relu)
                    gp.tensor_scalar_min(ee, ee, 1.0)
                    v.tensor_tensor(out=k2f[:, tt, :], in0=ee, in1=rr,
                                    op=ALU.add)
                    v.tensor_copy(v2x[:, tt, 0, 0:256], pv[:, 0:256])
                    v.tensor_copy(v2x[:, tt, 1, 0:256], pv[:, 256:512])
                pkvs = [ps.tile([128, 258], F32, tag="mm", name=f"pkv{_i}")
                        for _i in range(4)]
                for tt in range(NTT):
                    for p in range(4):
                        mm(pkvs[p], k2f[:, tt, p * 128:(p + 1) * 128],
                           v2x[:, tt, p // 2, :],
                           start=(tt == 0), stop=(tt == NTT - 1))
                for p in range(4):
                    if ci == 0:
                        sc.activation(kv_acc[:, p, :], pkvs[p], AF.Copy)
                    else:
                        v.tensor_tensor(out=kv_acc[:, p, :],
                                        in0=kv_acc[:, p, :], in1=pkvs[p],
                                        op=ALU.add)

            bd = pat.tile([128, KE, 128], MMDT, tag="bd")
            v.memset(F(bd), 0.0)
            for p in range(4):
                h0c = (2 * p % 4) * 64
                h1c = ((2 * p + 1) % 4) * 64
                v.tensor_copy(bd[0:64, p, 0:64], kv_acc[0:64, p, h0c:h0c + 64])
                v.tensor_copy(bd[64:128, p, 64:128],
                              kv_acc[64:128, p, h1c:h1c + 64])
            kmm = pat.tile([128, KE, H], MMDT, tag="km")
            for k in range(KE):
                v.tensor_scalar_mul(kmm[:, k, :], hmask_t[:, k, :],
                                    kv_acc[:, k, 256:257])

            # beta: q -> attention out-proj
            PHASES.append((f"attn{l}.{a}.beta", len(nc.inst_map)))
            for ci in range(NC):
                xq = load_x_chunk(xq_dram, ci)
                lowq = pa.tile([128, KR, C], MMDT, tag="a8")
                pls = [ps.tile([128, C], F32, tag="mm", name=f"plq{_i}") for _i in range(KR)]
                for k in range(KE):
                    for m in range(KR):
                        mm(pls[m], dwt[:, k, 0, m * 128:(m + 1) * 128],
                           xq[:, k, :], start=(k == 0), stop=(k == KE - 1))
                for m in range(KR):
                    v.tensor_copy(lowq[:, m, :], pls[m])
                qf = pa.tile([128, KE, C], MMDT, tag="a8")
                pqs = [ps.tile([128, C], F32, tag="mm", name=f"pq{_i}") for _i in range(KE)]
                for k in range(KR):
                    for m in range(KE):
                        mm(pqs[m], uwt[:, k, 0, m * 128:(m + 1) * 128],
                           lowq[:, k, :], start=(k == 0), stop=(k == KR - 1))
                for m in range(KE):
                    ee = pc.tile([128, C], dt, tag="a2")
                    rr = pc.tile([128, C], dt, tag="a2")
                    sc.activation(ee, pqs[m], AF.Exp, bias=ubq_col[:, m:m + 1])
                    sc.activation(rr, pqs[m], AF.Relu, bias=ubq_col[:, m:m + 1])
                    gp.tensor_scalar_min(ee, ee, 1.0)
                    v.tensor_tensor(out=qf[:, m, :], in0=ee, in1=rr, op=ALU.add)
                pd = ps.tile([8, C], F32, tag="mm")
                for k in range(KE):
                    mm(pd, kmm[:, k, :], qf[:, k, :], start=(k == 0),
                       stop=(k == KE - 1))
                den = pc.tile([8, C], dt, tag="a2")
                sc.activation(den, pd, AF.Identity, bias=eps_den)
                rec = pc.tile([8, C], MMDT, tag="a2")
                v.reciprocal(out=rec, in_=den)
                att = pa.tile([128, KE, C], MMDT, tag="a8")
                for m in range(KE):
                    pn = ps.tile([128, C], F32, tag="mm")
                    mm(pn, bd[:, m, :], qf[:, m, :], start=True, stop=True)
                    pr = ps.tile([128, C], F32, tag="mm")
                    mm(pr, cmask_t[:, m, :], rec, start=True, stop=True)
                    rb = pc.tile([128, C], dt, tag="a2")
                    sc.activation(rb, pr, AF.Copy)
                    v.tensor_tensor(out=att[:, m, :], in0=pn, in1=rb,
                                    op=ALU.mult)
                proj = pa.tile([128, KE, C], MMDT, tag="a8")
                pos = [ps.tile([128, C], F32, tag="mm", name=f"po{_i}") for _i in range(KE)]
                for k in range(KE):
                    for m in range(KE):
                        mm(pos[m], owt[:, k, m * 128:(m + 1) * 128],
                           att[:, k, :], start=(k == 0), stop=(k == KE - 1))
                for m in range(KE):
                    v.tensor_scalar_add(proj[:, m, :], pos[m],
                                        ob_col[:, m:m + 1])
                tail(ci, proj, xq)

        # ---- tails ----
        def make_self_tail(l, s, dst):
            g_col = col_tile(lng[l, 0 if s == "b" else 1], KE, tag="lncol")
            b_col = col_tile(lnb[l, 0 if s == "b" else 1], KE, tag="lncol")

            def tail(ci, proj, xq):
                for m in range(KE):
                    v.tensor_tensor(out=proj[:, m, :], in0=F(proj[:, m, :]),
                                    in1=F(xq[:, m, :]), op=ALU.add)
                outt = pa.tile([128, KE, C], dt, tag="a8")
                ln_stats_apply([proj[:, m, :] for m in range(KE)], g_col, b_col,
                               [outt[:, m, :] for m in range(KE)])
                store_chunk(dst, ci, outt)

            return tail

        def make_cross_tail(l, s, dst):
            gw1t = wsm.tile([128, 2 * KE, E4], MMDT, tag="ws")
            nc.sync.dma_start(out=gw1t,
                              in_=M(gw1[l].rearrange("(k p) g -> p k g", p=128)))
            gw2t = wsm.tile([128, 2], dt, tag="ws")
            nc.sync.dma_start(out=gw2t, in_=gw2[l])
            gwd = pat.tile([128, 1], MMDT, tag="gwd")
            v.tensor_tensor(out=gwd, in0=gw2t[:, 0:1], in1=gw2t[:, 1:2],
                            op=ALU.subtract)
            gb1_col = col_tile(gb1[l], 1, tag="lncol")
            gb2a = pat.tile([1, 1], dt, tag="gb2")
            nc.sync.dma_start(out=gb2a, in_=gb2[l, 0:1][None, :])
            gb2b = pat.tile([1, 1], dt, tag="gb2b")
            nc.sync.dma_start(out=gb2b, in_=gb2[l, 1:2][None, :])
            gb2d = pat.tile([1, 1], dt, tag="gb2d")
            v.tensor_tensor(out=gb2d, in0=gb2a, in1=gb2b, op=ALU.subtract)
            g_col = col_tile(lng[l, 2], KE, tag="lncol")
            b_col = col_tile(lnb[l, 2], KE, tag="lncol")

            def tail(ci, proj, xq):
                pg = ps.tile([128, C], F32, tag="mm")
                for k in range(2 * KE):
                    rhs = xq[:, k, :] if k < KE else proj[:, k - KE, :]
                    mm(pg, gw1t[:, k, :], rhs, start=(k == 0),
                       stop=(k == 2 * KE - 1))
                g1f = pc.tile([128, C], dt, tag="a2")
                v.tensor_scalar(out=g1f, in0=pg, scalar1=gb1_col[:, 0:1],
                                scalar2=0.0, op0=ALU.add, op1=ALU.max)
                g1t = pc.tile([128, C], MMDT, tag="a2")
                v.tensor_scalar_min(g1t, g1f, 6.0)
                pg2 = ps.tile([1, C], F32, tag="mm")
                mm(pg2, gwd, g1t, start=True, stop=True)
                bgf = pc.tile([1, C], dt, tag="a2")
                sc.activation(bgf, pg2, AF.Sigmoid, bias=gb2d[0:1, 0:1])
                bg = pc.tile([1, C], MMDT, tag="a2")
                v.tensor_copy(bg, bgf)
                pbg = psb.tile([128, C], F32, tag="bc")
                mm(pbg, ONES_ROW, bg, start=True, stop=True)
                mt = pa.tile([128, KE, C], MMDT, tag="a8")
                for m in range(KE):
                    dtmp = pc.tile([128, C], dt, tag="a2")
                    gp.tensor_tensor(out=dtmp, in0=F(xq[:, m, :]),
                                     in1=F(proj[:, m, :]), op=ALU.subtract)
                    v.tensor_tensor(out=dtmp, in0=dtmp, in1=pbg, op=ALU.mult)
                    v.tensor_tensor(out=mt[:, m, :], in0=dtmp,
                                    in1=F(proj[:, m, :]), op=ALU.add)
                outt = pa.tile([128, KE, C], dt, tag="a8")
                ln_stats_apply([mt[:, m, :] for m in range(KE)], g_col, b_col,
                               [outt[:, m, :] for m in range(KE)])
                store_chunk(dst, ci, outt)

            return tail

        # ---- FFN ----
        def ffn(l, s, src, dst):
            PHASES.append((f"ffn{l}.{s}", len(nc.inst_map)))
            si = 0 if s == "b" else 1
            w1t = wbig.tile([128, KE, X], MMDT, tag="w")
            nc.sync.dma_start(
                out=w1t, in_=M(w1[l, si].rearrange("(k p) x -> p k x", p=128)))
            w2t = wbig.tile([128, KX, E], MMDT, tag="w")
            nc.sync.dma_start(
                out=w2t, in_=M(w2[l, si].rearrange("(k p) e -> p k e", p=128)))
            b1_col = col_tile(b1[l, si], KX, tag="ffcol")
            b2_col = col_tile(b2[l, si], KE, tag="ffcol")
            w0_col = col_tile(cw[l, si, :, 0], KX, tag="ffcol")
            w1c_col = col_tile(cw[l, si, :, 1], KX, tag="ffcol")
            w2_col = col_tile(cw[l, si, :, 2], KX, tag="ffcol")
            cb_col = col_tile(cb[l, si], KX, tag="ffcol")
            bng_col = col_tile(bng[l, si], KX, tag="ffcol")
            bnb_col = col_tile(bnb[l, si], KX, tag="ffcol")
            rsq = float(1.0 / np.sqrt(1.0 + BN_EPS))
            A_col = wcol.tile([128, KX], dt, tag="ffcol")
            sc.activation(A_col, bng_col, AF.Copy, scale=rsq)
            B_col = wcol.tile([128, KX], dt, tag="ffcol")
            v.tensor_tensor(out=B_col, in0=cb_col, in1=A_col, op=ALU.mult)
            v.tensor_tensor(out=B_col, in0=B_col, in1=bnb_col, op=ALU.add)
            g_col = col_tile(lng[l, 3 if s == "b" else 4], KE, tag="lncol")
            bb_col = col_tile(lnb[l, 3 if s == "b" else 4], KE, tag="lncol")

            hts = [None] * NC
            xts = [None] * NC
            hl0 = [None] * NC   # last col scaled by w0
            hf2 = [None] * NC   # first col scaled by w2

            def compute_h(ci):
                xt = load_x_chunk(src, ci)
                xts[ci] = xt
                ht = pb.tile([128, KX, C], dt, tag="a16")
                for g in range(2):
                    phs = [ps.tile([128, C], F32, tag="mm", name=f"ph{_i}") for _i in range(4)]
                    for k in range(KE):
                        for j in range(4):
                            m = g * 4 + j
                            mm(phs[j], w1t[:, k, m * 128:(m + 1) * 128],
                               xt[:, k, :], start=(k == 0),
                               stop=(k == KE - 1))
                    for j in range(4):
                        m = g * 4 + j
                        sc.activation(ht[:, m, :], phs[j], AF.Relu,
                                      bias=b1_col[:, m:m + 1])
                        gp.tensor_scalar_min(ht[:, m, :], ht[:, m, :], 6.0)
                hts[ci] = ht
                l0 = phl.tile([128, KX, 1], dt, tag="hl")
                f2 = phl.tile([128, KX, 1], dt, tag="hf")
                for m in range(KX):
                    sc.activation(l0[:, m, :], ht[:, m, C - 1:C], AF.Copy,
                                  scale=w0_col[:, m:m + 1])
                    sc.activation(f2[:, m, :], ht[:, m, 0:1], AF.Copy,
                                  scale=w2_col[:, m:m + 1])
                hl0[ci], hf2[ci] = l0, f2

            def conv_tail(ci):
                ht = hts[ci]
                h2 = pb.tile([128, KX, C], MMDT, tag="a16")
                for m in range(KX):
                    acc = pc.tile([128, C], dt, tag="a2")
                    tmp = pc.tile([128, C], dt, tag="a2")
                    sc.activation(acc, ht[:, m, :], AF.Copy,
                                  scale=w1c_col[:, m:m + 1])
                    sc.activation(tmp, ht[:, m, :], AF.Copy,
                                  scale=w0_col[:, m:m + 1])
                    gp.tensor_tensor(out=acc[:, 1:C], in0=acc[:, 1:C],
                                     in1=tmp[:, 0:C - 1], op=ALU.add)
                    if ci > 0:
                        gp.tensor_tensor(out=acc[:, 0:1], in0=acc[:, 0:1],
                                         in1=hl0[ci - 1][:, m, :], op=ALU.add)
                    sc.activation(tmp, ht[:, m, :], AF.Copy,
                                  scale=w2_col[:, m:m + 1])
                    gp.tensor_tensor(out=acc[:, 0:C - 1], in0=acc[:, 0:C - 1],
                                     in1=tmp[:, 1:C], op=ALU.add)
                    if ci < NC - 1:
                        gp.tensor_tensor(out=acc[:, C - 1:C],
                                         in0=acc[:, C - 1:C],
                                         in1=hf2[ci + 1][:, m, :], op=ALU.add)
                    acc2 = pc.tile([128, C], dt, tag="a2")
                    sc.activation(acc2, acc, AF.Relu,
                                  scale=A_col[:, m:m + 1],
                                  bias=B_col[:, m:m + 1])
                    v.tensor_scalar_min(h2[:, m, :], acc2, 6.0)
                rt = pa.tile([128, KE, C], MMDT, tag="a8")
                pws = [ps.tile([128, C], F32, tag="mm", name=f"pw{_i}") for _i in range(KE)]
                for k in range(KX):
                    for m in range(KE):
                        mm(pws[m], w2t[:, k, m * 128:(m + 1) * 128],
                           h2[:, k, :], start=(k == 0), stop=(k == KX - 1))
                for m in range(KE):
                    rtf = pc.tile([128, C], dt, tag="a2")
                    sc.activation(rtf, pws[m], AF.Identity,
                                  bias=b2_col[:, m:m + 1])
                    v.tensor_tensor(out=rt[:, m, :], in0=rtf,
                                    in1=F(xts[ci][:, m, :]), op=ALU.add)
                outt = pa.tile([128, KE, C], dt, tag="a8")
                ln_stats_apply([rt[:, m, :] for m in range(KE)], g_col, bb_col,
                               [outt[:, m, :] for m in range(KE)])
                store_chunk(dst, ci, outt)
                hts[ci] = xts[ci] = None

            compute_h(0)
            for ci in range(1, NC):
                compute_h(ci)
                conv_tail(ci - 1)
            conv_tail(NC - 1)

        # ---- layers ----
        for l in range(L):
            bsrc = rs["b", 0] if l == 0 else rs["b", (l - 1, 3)]
            lsrc = rs["l", 0] if l == 0 else rs["l", (l - 1, 3)]
            attn(l, 0, bsrc, bsrc, make_self_tail(l, "b", rs["b", (l, 1)]))
            attn(l, 1, lsrc, lsrc, make_self_tail(l, "l", rs["l", (l, 1)]))
            attn(l, 2, rs["b", (l, 1)], rs["l", (l, 1)],
                 make_cross_tail(l, "b", rs["b", (l, 2)]))
            attn(l, 3, rs["l", (l, 1)], rs["b", (l, 1)],
                 make_cross_tail(l, "l", rs["l", (l, 2)]))
            ffn(l, "b", rs["b", (l, 2)], rs["b", (l, 3)])
            ffn(l, "l", rs["l", (l, 2)], rs["l", (l, 3)])

        PHASES.append(("final", len(nc.inst_map)))
        # ---- final head ----
        fw1t = wbig.tile([128, 2 * KE, E2], MMDT, tag="w")
        nc.sync.dma_start(out=fw1t,
                          in_=M(fw1.rearrange("(k p) g -> p k g", p=128)))
        fw2t = wsm.tile([128, 2, E], MMDT, tag="ws")
        nc.sync.dma_start(out=fw2t,
                          in_=M(fw2.rearrange("(k p) e -> p k e", p=128)))
        rw1t = wsm.tile([128, KE, E4], MMDT, tag="ws")
        nc.sync.dma_start(out=rw1t,
                          in_=M(rw1.rearrange("(k p) g -> p k g", p=128)))
        rw2t = wrow.tile([128, E8], MMDT, tag="row")
        nc.sync.dma_start(out=rw2t, in_=M(rw2))
        rw3t = wrow.tile([E8, 16], MMDT, tag="row")
        v.memset(F(rw3t), 0.0)
        nc.sync.dma_start(out=rw3t[:, 0:c.OUT], in_=M(rw3))
        rb3_row = wrow.tile([1, 16], MMDT, tag="row")
        v.memset(F(rb3_row), 0.0)
        nc.sync.dma_start(out=rb3_row[:, 0:c.OUT], in_=M(rb3[None, :]))
        fb1_col = col_tile(fb1, 2, tag="fcol")
        fb2_col = col_tile(fb2, KE, tag="fcol")
        flng_col = col_tile(flng, KE, tag="fcol")
        flnb_col = col_tile(flnb, KE, tag="fcol")
        rb1_col = col_tile(rb1, 1, tag="fcol")
        rb2_col = wcol.tile([E8, 1], dt, tag="fcol")
        nc.sync.dma_start(out=rb2_col, in_=rb2[:, None])
        out_ap = out_dram.ap()

        bsrc, lsrc = rs["b", (L - 1, 3)], rs["l", (L - 1, 3)]
        for ci in range(NC):
            xb = load_x_chunk(bsrc, ci)
            xl = load_x_chunk(lsrc, ci)
            f1t = [pc.tile([128, C], MMDT, tag="a2", name=f"f1t{_i}")
                   for _i in range(2)]
            pfs = [ps.tile([128, C], F32, tag="mm", name=f"pf{_i}") for _i in range(2)]
            for k in range(2 * KE):
                rhs = xb[:, k, :] if k < KE else xl[:, k - KE, :]
                for m in range(2):
                    mm(pfs[m], fw1t[:, k, m * 128:(m + 1) * 128], rhs,
                       start=(k == 0), stop=(k == 2 * KE - 1))
            for m in range(2):
                f1f = pc.tile([128, C], dt, tag="a2")
                sc.activation(f1f, pfs[m], AF.Relu, bias=fb1_col[:, m:m + 1])
                v.tensor_scalar_min(f1t[m], f1f, 6.0)
            ft = pa.tile([128, KE, C], MMDT, tag="a8")
            pf2s = [ps.tile([128, C], F32, tag="mm", name=f"pf2{_i}") for _i in range(KE)]
            for k in range(2):
                for m in range(KE):
                    mm(pf2s[m], fw2t[:, k, m * 128:(m + 1) * 128],
                       f1t[k], start=(k == 0), stop=(k == 1))
            for m in range(KE):
                v.tensor_scalar_add(ft[:, m, :], pf2s[m], fb2_col[:, m:m + 1])
            frt = pa.tile([128, KE, C], MMDT, tag="a8")
            ln_stats_apply([ft[:, m, :] for m in range(KE)], flng_col,
                           flnb_col, [frt[:, m, :] for m in range(KE)],
                           relu=True, out_f32r=True)
            p1 = ps.tile([128, C], F32, tag="mm")
            for k in range(KE):
                mm(p1, rw1t[:, k, :], frt[:, k, :], start=(k == 0),
                   stop=(k == KE - 1))
            h1f = pc.tile([128, C], dt, tag="a2")
            sc.activation(h1f, p1, AF.Relu, bias=rb1_col[:, 0:1])
            h1t = pc.tile([128, C], MMDT, tag="a2")
            v.tensor_scalar_min(h1t, h1f, 6.0)
            p2 = ps.tile([E8, C], F32, tag="mm")
            mm(p2, rw2t, h1t, start=True, stop=True)
            h2f = pc.tile([E8, C], dt, tag="a2")
            sc.activation(h2f, p2, AF.Relu, bias=rb2_col[:, 0:1])
            h2t = pc.tile([E8, C], MMDT, tag="a2")
            v.tensor_scalar_min(h2t, h2f, 6.0)
            ot = pc.tile([128, NTT, c.OUT], dt, tag="a2")
            for tt in range(NTT):
                p3 = ps.tile([128, 16], F32, tag="mm")
                mm(p3, h2t[:, tt * 128:(tt + 1) * 128], rw3t,
                   start=True, stop=False)
                mm(p3, ONES_ROW[:, 0:128], rb3_row, start=False, stop=True)
                sc.activation(ot[:, tt, :], p3[:, 0:c.OUT], AF.Copy)
            nc.sync.dma_start(
                out=out_ap[ci * C:(ci + 1) * C, :].rearrange(
                    "(tt p) o -> p tt o", p=128),
                in_=ot)

    return din, out_dram


# ======================================================================
# kernel() entry point: full inputs in, full outputs out (8-core SPMD).
# ======================================================================
import concourse.bacc as _bacc
from concourse.bass_utils import run_bass_kernel_spmd as _run_spmd

_N_CORES = 8
_CACHE = {}


def _get_nc():
    if "nc" not in _CACHE:
        nc = _bacc.Bacc("TRN2", target_bir_lowering=False, debug=False)
        build(nc, Cfg())
        nc.finalize()
        _CACHE["nc"] = nc
    return _CACHE["nc"]


def kernel(**inputs):
    nc = _get_nc()
    cfg = Cfg()
    consts = host_constants(cfg)
    arr = {k: np.ascontiguousarray(np.asarray(v, dtype=np.float32))
           for k, v in inputs.items()}
    shared = {k: a for k, a in arr.items()
              if k not in ("body_feats", "limb_feats")}
    shared.update(consts)
    in_maps = []
    for i in range(_N_CORES):
        m = dict(shared)
        m["body_feats"] = np.ascontiguousarray(arr["body_feats"][i])
        m["limb_feats"] = np.ascontiguousarray(arr["limb_feats"][i])
        in_maps.append(m)
    res = run_kernel_spmd_cached(nc, in_maps)
    out = np.stack([res[i]["out"] for i in range(_N_CORES)], axis=0)
    return out.astype(np.float32)


def run_kernel_spmd_cached(nc, in_maps, **kw):
    r = _run_spmd(nc, in_maps, list(range(_N_CORES)), **kw)
    _CACHE["last_result"] = r
    return r.results



# revision 2
# speedup vs baseline: 1.0343x; 1.0343x over previous
"""Dual-stream linear-attention transformer — bf16 redesign (per-core).

Layout convention (same as baseline):
  - "layout 1" activation: [E, N] feature-major; SBUF tiles [128, KE, C]
    (feature f = 128*k + p -> partition p, k-th slice; tokens on free dim).
  - alpha k/v are produced token-major per 128-token tile [128, E].
  - Residual streams live in internal DRAM as [E, N] bf16.

Key changes vs baseline:
  - All matmul operands + SBUF activations bf16 (same PE rate as f32r>=256,
    but DVE tensor_tensor 2x / tensor_scalar 4x, half DMA bytes).
  - QKV down+up projections composed into single E x E / E x 2E weights on
    the host (removes the low-rank intermediate copies).
  - Biases folded into matmuls as rank-1 accumulates (ones_row x bias_row).
  - elu+1 via 3 ops: ACT Exp, ACT Relu, DVE scalar_tensor_tensor(min,add).
  - LayerNorm apply via 3 bf16 DVE ops per slice (TT,TT,TS) instead of
    gpsimd tensor_tensor pairs.
  - FFN dwconv via TS/STT chain on DVE; BN folded on host into A,B.
  - relu6 of FFN h on gpsimd (idle engine) straight from PSUM.
  - m-outer matmul groups -> 1 PSUM bank live per group, fewer stalls.
"""

from dataclasses import dataclass
from contextlib import ExitStack

import numpy as np

import concourse.bass as bass
import concourse.mybir as mybir
import concourse.tile as tile

F32 = mybir.dt.float32
BF16 = mybir.dt.bfloat16
AF = mybir.ActivationFunctionType
ALU = mybir.AluOpType

LN_EPS = 1e-5
BN_EPS = 1e-5


@dataclass
class Cfg:
    N: int = 2048
    E: int = 512
    R: int = 256
    X: int = 1024
    H: int = 8
    L: int = 3
    OUT: int = 15
    C: int = 512

    @property
    def KE(self):
        return self.E // 128

    @property
    def KX(self):
        return self.X // 128

    @property
    def NC(self):
        return self.N // self.C

    @property
    def NTT(self):
        return self.C // 128


PHASES = []


def build(nc, cfg):
    c = cfg
    E, X, H, N, C, L = c.E, c.X, c.H, c.N, c.C, c.L
    KE, KX, NC, NTT = c.KE, c.KX, c.NC, c.NTT
    E4, E2, E8 = E // 4, E // 2, E // 8

    din = {}

    def inp(name, shape, dt=BF16):
        din[name] = nc.dram_tensor(name, list(shape), dt, kind="ExternalInput")
        return din[name].ap()

    # activations (host converts to bf16)
    body_feats = inp("body_feats", (N, E))
    limb_feats = inp("limb_feats", (N, E))
    # attention weights (host-composed)
    wq = inp("wq", (L, 4, E, E))
    wkv = inp("wkv", (L, 4, E, 2 * E))
    ubq = inp("ubq", (L, 4, E), F32)
    ubkv = inp("ubkv", (L, 4, 2 * E))
    ow = inp("ow", (L, 4, E, E))
    ob = inp("ob", (L, 4, E))
    # FFN
    w1 = inp("w1", (L, 2, E, X))
    b1 = inp("b1", (L, 2, X))
    cwf = inp("cwf", (L, 2, 3, X), F32)     # conv taps, tap-major
    bnA = inp("bnA", (L, 2, X), F32)        # bng*rsqrt(1+eps)
    bnB = inp("bnB", (L, 2, X), F32)        # cb*A + bnb
    w2 = inp("w2", (L, 2, X, E))
    b2 = inp("b2", (L, 2, E))
    lng = inp("lng", (L, 5, E), F32)
    lnb = inp("lnb", (L, 5, E), F32)
    # gating
    gw1 = inp("gw1", (L, 2 * E, E4))
    gb1 = inp("gb1", (L, E4), F32)
    gwd = inp("gwd", (L, E4))               # gw2[:,0]-gw2[:,1]
    gb2d = inp("gb2d", (L, 1), F32)         # gb2[0]-gb2[1]
    # final head
    fw1 = inp("fw1", (2 * E, E2))
    fb1 = inp("fb1", (E2,), F32)
    fw2 = inp("fw2", (E2, E))
    fb2 = inp("fb2", (E,))
    flng = inp("flng", (E,), F32)
    flnb = inp("flnb", (E,), F32)
    rw1 = inp("rw1", (E, E4))
    rb1 = inp("rb1", (E4,), F32)
    rw2 = inp("rw2", (E4, E8))
    rb2 = inp("rb2", (E8,), F32)
    rw3p = inp("rw3p", (E8, 16))            # zero-padded to 16
    rb3p = inp("rb3p", (16,))               # zero-padded
    ident_in = inp("ident", (128, 128))
    ones_in = inp("ones128", (128, 128))
    hmask_in = inp("hmask", (E, H))
    cmask_in = inp("cmask", (H, E))

    out_dram = nc.dram_tensor("out", [N, c.OUT], F32, kind="ExternalOutput")

    def idram(name):
        return nc.dram_tensor(name, [E, N], BF16).ap().rearrange(
            "(k p) n -> p k n", p=128)

    rs = {}
    for s in ("b", "l"):
        rs[s, 0] = idram(f"r{s}0")
        for l in range(L):
            for st in (1, 2, 3):
                rs[s, (l, st)] = idram(f"r{s}_{l}_{st}")

    lowp = nc.allow_low_precision("bf16 activations within rel-err budget")

    with tile.TileContext(nc) as tc, ExitStack() as ctx, lowp:
        p_ = ctx.enter_context
        cst = p_(tc.tile_pool(name="cst", bufs=1))
        wbig = p_(tc.tile_pool(name="wbig", bufs=3))
        wsm = p_(tc.tile_pool(name="wsm", bufs=2))
        wcol = p_(tc.tile_pool(name="wcol", bufs=12))
        wrow = p_(tc.tile_pool(name="wrow", bufs=4))
        pa = p_(tc.tile_pool(name="pa", bufs=7))      # 4KB bf16 chunk tiles
        pb = p_(tc.tile_pool(name="pb", bufs=3))      # 8KB ht tiles
        pc = p_(tc.tile_pool(name="pc", bufs=8))      # 1KB bf16 / rows
        pat = p_(tc.tile_pool(name="pat", bufs=2))    # per-attn persistents
        phl = p_(tc.tile_pool(name="phl", bufs=3))    # conv halos
        ps = p_(tc.tile_pool(name="ps", bufs=5, space="PSUM"))
        psr = p_(tc.tile_pool(name="psr", bufs=3, space="PSUM"))

        v, sc, gp = nc.vector, nc.scalar, nc.gpsimd

        def mm(out, lhsT, rhs, start, stop):
            nc.tensor.matmul(out, lhsT, rhs, start=start, stop=stop)

        # ---- constants ----
        ident_t = cst.tile([128, 128], BF16, tag="ident")
        nc.sync.dma_start(out=ident_t, in_=ident_in)
        ones_t = cst.tile([128, 128], BF16, tag="ones")
        nc.sync.dma_start(out=ones_t, in_=ones_in)
        hmask_t = cst.tile([128, KE, H], BF16, tag="hmask")
        nc.sync.dma_start(out=hmask_t,
                          in_=hmask_in.rearrange("(k p) h -> p k h", p=128))
        cmask_t = cst.tile([H, KE, 128], BF16, tag="cmask")
        nc.sync.dma_start(out=cmask_t,
                          in_=cmask_in.rearrange("h (k p) -> h k p", p=128))
        ONES_COL = ones_t[:, 0:1]
        ONES_ROW = ones_t[0:1, :]
        onesc_t = cst.tile([1, C], BF16, tag="onesc")
        v.memset(onesc_t, 1.0)
        ONES_C = onesc_t[0:1, :]
        eps_ln = cst.tile([1, 1], F32, tag="epsl")
        v.memset(eps_ln, LN_EPS)

        def col_tile(src_ap, m, tag="col"):
            t = wcol.tile([128, m], F32, tag=tag)
            nc.sync.dma_start(out=t, in_=src_ap.rearrange("(m p) -> p m", p=128))
            return t

        def row_tile(src_ap, n, tag="row", pool=None):
            t = (pool or wrow).tile([1, n], BF16, tag=tag)
            nc.sync.dma_start(out=t, in_=src_ap[None, :])
            return t

        def ln_apply(xs, g_col, b_col, outt, extra_tt=None):
            """LayerNorm over features (layout 1). xs: [128, KE, C] bf16 tile.
            outt: [128, KE, C] bf16 out. extra_tt(m): None."""
            sq = pa.tile([128, KE, C], BF16, tag="a4")
            xf = xs.rearrange("p k c -> p (k c)")
            v.tensor_tensor(out=sq.rearrange("p k c -> p (k c)"),
                            in0=xf, in1=xf, op=ALU.mult)
            ps_s = psr.tile([1, C], F32, tag="row")
            ps_ss = psr.tile([1, C], F32, tag="row")
            for m in range(KE):
                mm(ps_s, ONES_COL, xs[:, m, :], start=(m == 0),
                   stop=(m == KE - 1))
                mm(ps_ss, ONES_COL, sq[:, m, :], start=(m == 0),
                   stop=(m == KE - 1))
            msq = pc.tile([1, C], F32, tag="row")
            sc.activation(msq, ps_ss, AF.Copy, scale=1.0 / E)
            m2 = pc.tile([1, C], F32, tag="row")
            sc.activation(m2, ps_s, AF.Square, scale=1.0 / E)
            var = pc.tile([1, C], F32, tag="row")
            v.tensor_tensor(out=var, in0=msq, in1=m2, op=ALU.subtract)
            sd = pc.tile([1, C], F32, tag="row")
            sc.activation(sd, var, AF.Sqrt, bias=eps_ln[0:1, 0:1])
            srow = pc.tile([1, C], BF16, tag="rowh")
            v.reciprocal(out=srow, in_=sd)
            trow = pc.tile([1, C], BF16, tag="rowh")
            v.scalar_tensor_tensor(out=trow, in0=ps_s, scalar=1.0 / E,
                                   in1=srow, op0=ALU.mult, op1=ALU.mult)
            bc_s = ps.tile([128, C], F32, tag="mm")
            mm(bc_s, ONES_ROW, srow, start=True, stop=True)
            sb_s = pc.tile([128, C], BF16, tag="a1")
            sc.activation(sb_s, bc_s, AF.Copy)
            bc_t = ps.tile([128, C], F32, tag="mm")
            mm(bc_t, ONES_ROW, trow, start=True, stop=True)
            sb_t = pc.tile([128, C], BF16, tag="a1")
            sc.activation(sb_t, bc_t, AF.Copy)
            for m in range(KE):
                u = pc.tile([128, C], BF16, tag="a1")
                v.tensor_tensor(out=u, in0=xs[:, m, :], in1=sb_s, op=ALU.mult)
                v.tensor_tensor(out=u, in0=u, in1=sb_t, op=ALU.subtract)
                v.tensor_scalar(out=outt[:, m, :], in0=u,
                                scalar1=g_col[:, m:m + 1],
                                scalar2=b_col[:, m:m + 1],
                                op0=ALU.mult, op1=ALU.add)

        def load_x_chunk(dram_l1, ci, tag="a4"):
            xt = pa.tile([128, KE, C], BF16, tag=tag)
            nc.sync.dma_start(out=xt, in_=dram_l1[:, :, ci * C:(ci + 1) * C])
            return xt

        def store_chunk(dram_l1, ci, t):
            gp.dma_start(out=dram_l1[:, :, ci * C:(ci + 1) * C], in_=t)

        # ---- entry transpose ----
        def entry(x_ap, dst):
            for ttk in range(N // 128):
                x2 = pa.tile([128, E], BF16, tag="a4")
                nc.sync.dma_start(out=x2, in_=x_ap[ttk * 128:(ttk + 1) * 128, :])
                xt = pa.tile([128, KE, 128], BF16, tag="a4")
                for f in range(KE):
                    pt = ps.tile([128, 128], BF16, tag="mm")
                    nc.tensor.transpose(pt, x2[:, f * 128:(f + 1) * 128],
                                        ident_t)
                    sc.activation(xt[:, f, :], pt, AF.Copy)
                nc.sync.dma_start(out=dst[:, :, ttk * 128:(ttk + 1) * 128],
                                  in_=xt)

        PHASES.append(("entry", len(nc.inst_map)))
        entry(body_feats, rs["b", 0])
        entry(limb_feats, rs["l", 0])

        # ---- linear attention ----
        def attn(l, a, xq_dram, xkv_dram, tail_m, tail_post):
            """tail_m(ci, m, pos_psum, xq) consumes out-proj psum slice m;
            tail_post(ci, xq) finishes the chunk (LN + store)."""
            wqt = wbig.tile([128, KE, E], BF16, tag="w")
            nc.sync.dma_start(
                out=wqt, in_=wq[l, a].rearrange("(k p) e -> p k e", p=128))
            wkvt = wbig.tile([128, KE, 2 * E], BF16, tag="w2x")
            nc.sync.dma_start(
                out=wkvt, in_=wkv[l, a].rearrange("(k p) e -> p k e", p=128))
            owt = wbig.tile([128, KE, E], BF16, tag="w")
            nc.sync.dma_start(
                out=owt, in_=ow[l, a].rearrange("(k p) e -> p k e", p=128))
            ubq_col = col_tile(ubq[l, a], KE)
            ubkv_row = row_tile(ubkv[l, a], 2 * E)
            ob_row = row_tile(ob[l, a], E)

            PHASES.append((f"attn{l}.{a}.alpha", len(nc.inst_map)))
            kv_acc = pat.tile([128, KE, 258], F32, tag="kva")

            for ci in range(NC):
                xt = load_x_chunk(xkv_dram, ci)
                k2f = pa.tile([128, NTT, E], BF16, tag="a4")
                v2x = pa.tile([128, NTT, 2, 258], BF16, tag="a4")
                v.memset(v2x[:, :, :, 256:258], 1.0)
                for tt in range(NTT):
                    xs = xt[:, :, tt * 128:(tt + 1) * 128]
                    pk = ps.tile([128, E], F32, tag="mm")
                    pv = ps.tile([128, E], F32, tag="mm")
                    for k in range(KE):
                        mm(pk, xs[:, k, :], wkvt[:, k, 0:E],
                           start=(k == 0), stop=False)
                        mm(pv, xs[:, k, :], wkvt[:, k, E:2 * E],
                           start=(k == 0), stop=False)
                    mm(pk, ONES_ROW, ubkv_row[:, 0:E], start=False, stop=True)
                    mm(pv, ONES_ROW, ubkv_row[:, E:2 * E], start=False,
                       stop=True)
                    ee = pc.tile([128, E], BF16, tag="a1")
                    rr = pc.tile([128, E], BF16, tag="a1")
                    sc.activation(ee, pk, AF.Exp)
                    sc.activation(rr, pk, AF.Relu)
                    v.scalar_tensor_tensor(out=k2f[:, tt, :], in0=ee,
                                           scalar=1.0, in1=rr,
                                           op0=ALU.min, op1=ALU.add)
                    v.tensor_copy(v2x[:, tt, 0, 0:256], pv[:, 0:256])
                    v.tensor_copy(v2x[:, tt, 1, 0:256], pv[:, 256:512])
                for p in range(4):
                    pkv = ps.tile([128, 258], F32, tag="mm")
                    for tt in range(NTT):
                        mm(pkv, k2f[:, tt, p * 128:(p + 1) * 128],
                           v2x[:, tt, p // 2, :],
                           start=(tt == 0), stop=(tt == NTT - 1))
                    if ci == 0:
                        sc.activation(kv_acc[:, p, :], pkv, AF.Copy)
                    else:
                        v.tensor_tensor(out=kv_acc[:, p, :],
                                        in0=kv_acc[:, p, :], in1=pkv,
                                        op=ALU.add)

            bd = pat.tile([128, KE, 128], BF16, tag="bd")
            v.memset(bd, 0.0)
            for p in range(4):
                h0c = (2 * p % 4) * 64
                h1c = ((2 * p + 1) % 4) * 64
                v.tensor_copy(bd[0:64, p, 0:64], kv_acc[0:64, p, h0c:h0c + 64])
                v.tensor_copy(bd[64:128, p, 64:128],
                              kv_acc[64:128, p, h1c:h1c + 64])
            kmm = pat.tile([128, KE, H], BF16, tag="km")
            for k in range(KE):
                v.tensor_scalar_mul(kmm[:, k, :], hmask_t[:, k, :],
                                    kv_acc[:, k, 256:257])

            PHASES.append((f"attn{l}.{a}.beta", len(nc.inst_map)))
            for ci in range(NC):
                xq = load_x_chunk(xq_dram, ci)
                qf = pa.tile([128, KE, C], BF16, tag="a4")
                for m in range(KE):
                    pq = ps.tile([128, C], F32, tag="mm")
                    for k in range(KE):
                        mm(pq, wqt[:, k, m * 128:(m + 1) * 128], xq[:, k, :],
                           start=(k == 0), stop=(k == KE - 1))
                    ee = pc.tile([128, C], BF16, tag="a1")
                    rr = pc.tile([128, C], BF16, tag="a1")
                    sc.activation(ee, pq, AF.Exp, bias=ubq_col[:, m:m + 1])
                    sc.activation(rr, pq, AF.Relu, bias=ubq_col[:, m:m + 1])
                    v.scalar_tensor_tensor(out=qf[:, m, :], in0=ee, scalar=1.0,
                                           in1=rr, op0=ALU.min, op1=ALU.add)
                pd = psr.tile([8, C], F32, tag="row")
                for k in range(KE):
                    mm(pd, kmm[:, k, :], qf[:, k, :], start=(k == 0),
                       stop=(k == KE - 1))
                rec = pc.tile([8, C], BF16, tag="a1")
                v.reciprocal(out=rec, in_=pd)
                att = pa.tile([128, KE, C], BF16, tag="a4")
                for m in range(KE):
                    pn = ps.tile([128, C], F32, tag="mm")
                    mm(pn, bd[:, m, :], qf[:, m, :], start=True, stop=True)
                    pr = ps.tile([128, C], F32, tag="mm")
                    mm(pr, cmask_t[:, m, :], rec, start=True, stop=True)
                    rb = pc.tile([128, C], BF16, tag="a1")
                    sc.activation(rb, pr, AF.Copy)
                    v.tensor_tensor(out=att[:, m, :], in0=pn, in1=rb,
                                    op=ALU.mult)
                for m in range(KE):
                    pos = ps.tile([128, C], F32, tag="mm")
                    for k in range(KE):
                        mm(pos, owt[:, k, m * 128:(m + 1) * 128],
                           att[:, k, :], start=(k == 0), stop=False)
                    mm(pos, ob_row[:, m * 128:(m + 1) * 128], ONES_C,
                       start=False, stop=True)
                    tail_m(ci, m, pos, xq)
                tail_post(ci, xq)

        # ---- tails ----
        def make_self_tail(l, s, dst):
            g_col = col_tile(lng[l, 0 if s == "b" else 1], KE, tag="lncol")
            b_col = col_tile(lnb[l, 0 if s == "b" else 1], KE, tag="lncol")
            rt_box = [None]

            def tail_m(ci, m, pos, xq):
                if m == 0:
                    rt_box[0] = pa.tile([128, KE, C], BF16, tag="a4",
                                        name="rt")
                v.tensor_tensor(out=rt_box[0][:, m, :], in0=pos,
                                in1=xq[:, m, :], op=ALU.add)

            def tail_post(ci, xq):
                rt = rt_box[0]
                outt = pa.tile([128, KE, C], BF16, tag="a4")
                ln_apply(rt, g_col, b_col, outt)
                store_chunk(dst, ci, outt)

            return tail_m, tail_post

        def make_cross_tail(l, s, dst):
            gw1t = wsm.tile([128, 2 * KE, E4], BF16, tag="ws")
            nc.sync.dma_start(out=gw1t,
                              in_=gw1[l].rearrange("(k p) g -> p k g", p=128))
            gwd_col = wcol.tile([128, 1], BF16, tag="gwd")
            nc.sync.dma_start(out=gwd_col, in_=gwd[l][:, None])
            gb1_col = col_tile(gb1[l], 1, tag="lncol")
            gb2d_t = pat.tile([1, 1], F32, tag="gb2d")
            nc.sync.dma_start(out=gb2d_t, in_=gb2d[l][None, :])
            g_col = col_tile(lng[l, 2], KE, tag="lncol")
            b_col = col_tile(lnb[l, 2], KE, tag="lncol")
            proj_box = [None]

            def tail_m(ci, m, pos, xq):
                if m == 0:
                    proj_box[0] = pa.tile([128, KE, C], BF16, tag="a4",
                                          name="proj")
                sc.activation(proj_box[0][:, m, :], pos, AF.Copy)

            def tail_post(ci, xq):
                proj = proj_box[0]
                pg = ps.tile([128, C], F32, tag="mm")
                for k in range(2 * KE):
                    rhs = xq[:, k, :] if k < KE else proj[:, k - KE, :]
                    mm(pg, gw1t[:, k, :], rhs, start=(k == 0),
                       stop=(k == 2 * KE - 1))
                g1 = pc.tile([128, C], BF16, tag="a1")
                sc.activation(g1, pg, AF.Relu, bias=gb1_col[:, 0:1])
                g1t = pc.tile([128, C], BF16, tag="a1")
                v.tensor_scalar_min(g1t, g1, 6.0)
                pg2 = psr.tile([1, C], F32, tag="row")
                mm(pg2, gwd_col, g1t, start=True, stop=True)
                bg = pc.tile([1, C], BF16, tag="rowh")
                sc.activation(bg, pg2, AF.Sigmoid, bias=gb2d_t[0:1, 0:1])
                pbg = ps.tile([128, C], F32, tag="mm")
                mm(pbg, ONES_ROW, bg, start=True, stop=True)
                bgb = pc.tile([128, C], BF16, tag="a1")
                sc.activation(bgb, pbg, AF.Copy)
                mt = pa.tile([128, KE, C], BF16, tag="a4")
                for m in range(KE):
                    dtmp = pc.tile([128, C], BF16, tag="a1")
                    v.tensor_tensor(out=dtmp, in0=xq[:, m, :],
                                    in1=proj[:, m, :], op=ALU.subtract)
                    v.tensor_tensor(out=dtmp, in0=dtmp, in1=bgb, op=ALU.mult)
                    v.tensor_tensor(out=mt[:, m, :], in0=dtmp,
                                    in1=proj[:, m, :], op=ALU.add)
                outt = pa.tile([128, KE, C], BF16, tag="a4")
                ln_apply(mt, g_col, b_col, outt)
                store_chunk(dst, ci, outt)

            return tail_m, tail_post

        # ---- FFN ----
        def ffn(l, s, src, dst):
            PHASES.append((f"ffn{l}.{s}", len(nc.inst_map)))
            si = 0 if s == "b" else 1
            w1t = wbig.tile([128, KE, X], BF16, tag="w2x")
            nc.sync.dma_start(
                out=w1t, in_=w1[l, si].rearrange("(k p) x -> p k x", p=128))
            w2t = wbig.tile([128, KX, E], BF16, tag="w2x")
            nc.sync.dma_start(
                out=w2t, in_=w2[l, si].rearrange("(k p) e -> p k e", p=128))
            b1_row = row_tile(b1[l, si], X)
            b2_row = row_tile(b2[l, si], E)
            w0_col = col_tile(cwf[l, si, 0], KX, tag="ffcol")
            w1c_col = col_tile(cwf[l, si, 1], KX, tag="ffcol")
            w2_col = col_tile(cwf[l, si, 2], KX, tag="ffcol")
            A_col = col_tile(bnA[l, si], KX, tag="ffcol")
            B_col = col_tile(bnB[l, si], KX, tag="ffcol")
            g_col = col_tile(lng[l, 3 if s == "b" else 4], KE, tag="lncol")
            bb_col = col_tile(lnb[l, 3 if s == "b" else 4], KE, tag="lncol")

            hts = [None] * NC
            xts = [None] * NC
            hl0 = [None] * NC
            hf2 = [None] * NC

            def compute_h(ci):
                xt = load_x_chunk(src, ci)
                xts[ci] = xt
                ht = pb.tile([128, KX, C], BF16, tag="a8")
                for m in range(KX):
                    ph = ps.tile([128, C], F32, tag="mm")
                    for k in range(KE):
                        mm(ph, w1t[:, k, m * 128:(m + 1) * 128], xt[:, k, :],
                           start=(k == 0), stop=False)
                    mm(ph, b1_row[:, m * 128:(m + 1) * 128], ONES_C,
                       start=False, stop=True)
                    v.tensor_scalar(out=ht[:, m, :], in0=ph, scalar1=0.0,
                                    scalar2=6.0, op0=ALU.max, op1=ALU.min)
                hts[ci] = ht
                l0 = phl.tile([128, KX, 1], BF16, tag="hl")
                f2 = phl.tile([128, KX, 1], BF16, tag="hf")
                for m in range(KX):
                    v.tensor_scalar_mul(l0[:, m, :], ht[:, m, C - 1:C],
                                        w0_col[:, m:m + 1])
                    v.tensor_scalar_mul(f2[:, m, :], ht[:, m, 0:1],
                                        w2_col[:, m:m + 1])
                hl0[ci], hf2[ci] = l0, f2

            def conv_tail(ci):
                ht = hts[ci]
                h2 = pb.tile([128, KX, C], BF16, tag="a8")
                for m in range(KX):
                    acc = pc.tile([128, C], BF16, tag="a1")
                    v.tensor_scalar_mul(acc, ht[:, m, :], w1c_col[:, m:m + 1])
                    v.scalar_tensor_tensor(out=acc[:, 1:C],
                                           in0=ht[:, m, 0:C - 1],
                                           scalar=w0_col[:, m:m + 1],
                                           in1=acc[:, 1:C],
                                           op0=ALU.mult, op1=ALU.add)
                    if ci > 0:
                        v.tensor_tensor(out=acc[:, 0:1], in0=acc[:, 0:1],
                                        in1=hl0[ci - 1][:, m, :], op=ALU.add)
                    v.scalar_tensor_tensor(out=acc[:, 0:C - 1],
                                           in0=ht[:, m, 1:C],
                                           scalar=w2_col[:, m:m + 1],
                                           in1=acc[:, 0:C - 1],
                                           op0=ALU.mult, op1=ALU.add)
                    if ci < NC - 1:
                        v.tensor_tensor(out=acc[:, C - 1:C],
                                        in0=acc[:, C - 1:C],
                                        in1=hf2[ci + 1][:, m, :], op=ALU.add)
                    a2 = pc.tile([128, C], BF16, tag="a1")
                    sc.activation(a2, acc, AF.Relu, scale=A_col[:, m:m + 1],
                                  bias=B_col[:, m:m + 1])
                    v.tensor_scalar_min(h2[:, m, :], a2, 6.0)
                rt = pa.tile([128, KE, C], BF16, tag="a4")
                for m in range(KE):
                    pw = ps.tile([128, C], F32, tag="mm")
                    for k in range(KX):
                        mm(pw, w2t[:, k, m * 128:(m + 1) * 128], h2[:, k, :],
                           start=(k == 0), stop=False)
                    mm(pw, b2_row[:, m * 128:(m + 1) * 128], ONES_C,
                       start=False, stop=True)
                    v.tensor_tensor(out=rt[:, m, :], in0=pw,
                                    in1=xts[ci][:, m, :], op=ALU.add)
                outt = pa.tile([128, KE, C], BF16, tag="a4")
                ln_apply(rt, g_col, bb_col, outt)
                store_chunk(dst, ci, outt)
                hts[ci] = xts[ci] = None

            compute_h(0)
            for ci in range(1, NC):
                compute_h(ci)
                conv_tail(ci - 1)
            conv_tail(NC - 1)

        # ---- layers ----
        for l in range(L):
            bsrc = rs["b", 0] if l == 0 else rs["b", (l - 1, 3)]
            lsrc = rs["l", 0] if l == 0 else rs["l", (l - 1, 3)]
            attn(l, 0, bsrc, bsrc, *make_self_tail(l, "b", rs["b", (l, 1)]))
            attn(l, 1, lsrc, lsrc, *make_self_tail(l, "l", rs["l", (l, 1)]))
            attn(l, 2, rs["b", (l, 1)], rs["l", (l, 1)],
                 *make_cross_tail(l, "b", rs["b", (l, 2)]))
            attn(l, 3, rs["l", (l, 1)], rs["b", (l, 1)],
                 *make_cross_tail(l, "l", rs["l", (l, 2)]))
            ffn(l, "b", rs["b", (l, 2)], rs["b", (l, 3)])
            ffn(l, "l", rs["l", (l, 2)], rs["l", (l, 3)])

        PHASES.append(("final", len(nc.inst_map)))
        # ---- final head ----
        fw1t = wbig.tile([128, 2 * KE, E2], BF16, tag="w")
        nc.sync.dma_start(out=fw1t,
                          in_=fw1.rearrange("(k p) g -> p k g", p=128))
        fw2t = wsm.tile([128, 2, E], BF16, tag="wfin", bufs=4)
        nc.sync.dma_start(out=fw2t,
                          in_=fw2.rearrange("(k p) e -> p k e", p=128))
        rw1t = wsm.tile([128, KE, E4], BF16, tag="wfin", bufs=4)
        nc.sync.dma_start(out=rw1t,
                          in_=rw1.rearrange("(k p) g -> p k g", p=128))
        rw2t = wsm.tile([128, E8], BF16, tag="wfin", bufs=4)
        nc.sync.dma_start(out=rw2t, in_=rw2)
        rw3t = wsm.tile([E8, 16], BF16, tag="wfin", bufs=4)
        nc.sync.dma_start(out=rw3t, in_=rw3p)
        rb3_row = row_tile(rb3p, 16)
        fb2_row = row_tile(fb2, E)
        fb1_col = col_tile(fb1, 2, tag="fcol")
        flng_col = col_tile(flng, KE, tag="fcol")
        flnb_col = col_tile(flnb, KE, tag="fcol")
        rb1_col = col_tile(rb1, 1, tag="fcol")
        rb2_col = wcol.tile([E8, 1], F32, tag="fcol")
        nc.sync.dma_start(out=rb2_col, in_=rb2[:, None])
        out_ap = out_dram.ap()

        bsrc, lsrc = rs["b", (L - 1, 3)], rs["l", (L - 1, 3)]
        for ci in range(NC):
            xb = load_x_chunk(bsrc, ci)
            xl = load_x_chunk(lsrc, ci)
            f1t = pa.tile([128, 2, C], BF16, tag="a4")
            for m in range(2):
                pf = ps.tile([128, C], F32, tag="mm")
                for k in range(2 * KE):
                    rhs = xb[:, k, :] if k < KE else xl[:, k - KE, :]
                    mm(pf, fw1t[:, k, m * 128:(m + 1) * 128], rhs,
                       start=(k == 0), stop=(k == 2 * KE - 1))
                f1 = pc.tile([128, C], BF16, tag="a1")
                sc.activation(f1, pf, AF.Relu, bias=fb1_col[:, m:m + 1])
                v.tensor_scalar_min(f1t[:, m, :], f1, 6.0)
            ft = pa.tile([128, KE, C], BF16, tag="a4")
            for m in range(KE):
                pf2 = ps.tile([128, C], F32, tag="mm")
                for k in range(2):
                    mm(pf2, fw2t[:, k, m * 128:(m + 1) * 128], f1t[:, k, :],
                       start=(k == 0), stop=False)
                mm(pf2, fb2_row[:, m * 128:(m + 1) * 128], ONES_C,
                   start=False, stop=True)
                sc.activation(ft[:, m, :], pf2, AF.Copy)
            frt = pa.tile([128, KE, C], BF16, tag="a4")
            ln_apply(ft, flng_col, flnb_col, frt)
            # relu after LN
            frf = frt.rearrange("p k c -> p (k c)")
            v.tensor_scalar_max(frf, frf, 0.0)
            p1 = ps.tile([128, C], F32, tag="mm")
            for k in range(KE):
                mm(p1, rw1t[:, k, :], frt[:, k, :], start=(k == 0),
                   stop=(k == KE - 1))
            h1f = pc.tile([128, C], BF16, tag="a1")
            sc.activation(h1f, p1, AF.Relu, bias=rb1_col[:, 0:1])
            h1t = pc.tile([128, C], BF16, tag="a1")
            v.tensor_scalar_min(h1t, h1f, 6.0)
            p2 = ps.tile([E8, C], F32, tag="mm")
            mm(p2, rw2t, h1t, start=True, stop=True)
            h2f = pc.tile([E8, C], BF16, tag="a1")
            sc.activation(h2f, p2, AF.Relu, bias=rb2_col[:, 0:1])
            h2t = pc.tile([E8, C], BF16, tag="a1")
            v.tensor_scalar_min(h2t, h2f, 6.0)
            ot = pc.tile([128, NTT, c.OUT], F32, tag="a1")
            for tt in range(NTT):
                p3 = ps.tile([128, 16], F32, tag="mm")
                mm(p3, h2t[:, tt * 128:(tt + 1) * 128], rw3t,
                   start=True, stop=False)
                mm(p3, ONES_ROW, rb3_row, start=False, stop=True)
                sc.activation(ot[:, tt, :], p3[:, 0:c.OUT], AF.Copy)
            nc.sync.dma_start(
                out=out_ap[ci * C:(ci + 1) * C, :].rearrange(
                    "(tt p) o -> p tt o", p=128),
                in_=ot)

    return din, out_dram


# ======================================================================
# kernel() entry point: full inputs in, full outputs out (8-core SPMD).
# ======================================================================
import concourse.bacc as _bacc
from concourse.bass_utils import run_bass_kernel_spmd as _run_spmd

_N_CORES = 8
_CACHE = {}


def _get_nc():
    if "nc" not in _CACHE:
        nc = _bacc.Bacc("TRN2", target_bir_lowering=False, debug=False)
        build(nc, Cfg())
        nc.finalize()
        _CACHE["nc"] = nc
    return _CACHE["nc"]


def _bf16(x):
    import ml_dtypes
    return np.asarray(x, dtype=np.float32).astype(ml_dtypes.bfloat16)


def host_prep(inputs):
    """Host-side weight preprocessing: compose QKV, fold BN, convert bf16."""
    c = Cfg()
    E, X, H, L = c.E, c.X, c.H, c.L
    E4, E2, E8 = E // 4, E // 2, E // 8
    f = {k: np.asarray(v, dtype=np.float32) for k, v in inputs.items()}
    dw, uw, ub = f["dw"], f["uw"], f["ub"]
    # composed q and k|v projection weights
    wq = np.matmul(dw[:, :, 0], uw[:, :, 0])          # (L,4,E,E)
    wk = np.matmul(dw[:, :, 1], uw[:, :, 1])
    wv = np.matmul(dw[:, :, 2], uw[:, :, 2])
    wkv = np.concatenate([wk, wv], axis=-1)           # (L,4,E,2E)
    ubq = ub[:, :, 0]                                 # (L,4,E)
    ubkv = np.concatenate([ub[:, :, 1], ub[:, :, 2]], axis=-1)
    rsq = np.float32(1.0 / np.sqrt(1.0 + BN_EPS))
    A = f["bng"] * rsq                                # (L,2,X)
    B = f["cb"] * A + f["bnb"]
    cwf = f["cw"].transpose(0, 1, 3, 2).copy()        # (L,2,3,X)
    gwd = f["gw2"][:, :, 0] - f["gw2"][:, :, 1]       # (L,E4)
    gb2d = (f["gb2"][:, 0] - f["gb2"][:, 1])[:, None]  # (L,1)
    rw3p = np.zeros((E8, 16), np.float32)
    rw3p[:, :c.OUT] = f["rw3"]
    rb3p = np.zeros((16,), np.float32)
    rb3p[:c.OUT] = f["rb3"]
    dh = E // H
    ident = np.eye(128, dtype=np.float32)
    ones = np.ones((128, 128), dtype=np.float32)
    hmask = np.zeros((E, H), dtype=np.float32)
    for ff in range(E):
        hmask[ff, ff // dh] = 1.0
    cmask = hmask.T.copy()

    b16 = dict(wq=wq, wkv=wkv, ubkv=ubkv, ow=f["ow"], ob=f["ob"],
               w1=f["w1"], b1=f["b1"], w2=f["w2"], b2=f["b2"],
               gw1=f["gw1"], gwd=gwd, fw1=f["fw1"], fw2=f["fw2"],
               fb2=f["fb2"], rw1=f["rw1"], rw2=f["rw2"], rw3p=rw3p,
               rb3p=rb3p, ident=ident, ones128=ones, hmask=hmask,
               cmask=cmask)
    f32 = dict(ubq=ubq, cwf=cwf, bnA=A, bnB=B, lng=f["lng"], lnb=f["lnb"],
               gb1=f["gb1"], gb2d=gb2d, fb1=f["fb1"], flng=f["flng"],
               flnb=f["flnb"], rb1=f["rb1"], rb2=f["rb2"])
    shared = {k: _bf16(v) for k, v in b16.items()}
    shared.update({k: np.ascontiguousarray(v, dtype=np.float32)
                   for k, v in f32.items()})
    return shared, f["body_feats"], f["limb_feats"]


def kernel(**inputs):
    nc = _get_nc()
    shared, body, limb = host_prep(inputs)
    in_maps = []
    for i in range(_N_CORES):
        m = dict(shared)
        m["body_feats"] = _bf16(body[i])
        m["limb_feats"] = _bf16(limb[i])
        in_maps.append(m)
    res = run_kernel_spmd_cached(nc, in_maps)
    out = np.stack([res[i]["out"] for i in range(_N_CORES)], axis=0)
    return out.astype(np.float32)


def run_kernel_spmd_cached(nc, in_maps, **kw):
    r = _run_spmd(nc, in_maps, list(range(_N_CORES)), **kw)
    _CACHE["last_result"] = r
    return r.results


# revision 3
# speedup vs baseline: 1.1252x; 1.0879x over previous
"""Dual-stream linear-attention transformer — bf16 redesign (per-core).

Layout convention (same as baseline):
  - "layout 1" activation: [E, N] feature-major; SBUF tiles [128, KE, C]
    (feature f = 128*k + p -> partition p, k-th slice; tokens on free dim).
  - alpha k/v are produced token-major per 128-token tile [128, E].
  - Residual streams live in internal DRAM as [E, N] bf16.

Key changes vs baseline:
  - All matmul operands + SBUF activations bf16 (same PE rate as f32r>=256,
    but DVE tensor_tensor 2x / tensor_scalar 4x, half DMA bytes).
  - QKV down+up projections composed into single E x E / E x 2E weights on
    the host (removes the low-rank intermediate copies).
  - Biases folded into matmuls as rank-1 accumulates (ones_row x bias_row).
  - elu+1 via 3 ops: ACT Exp, ACT Relu, DVE scalar_tensor_tensor(min,add).
  - LayerNorm apply via 3 bf16 DVE ops per slice (TT,TT,TS) instead of
    gpsimd tensor_tensor pairs.
  - FFN dwconv via TS/STT chain on DVE; BN folded on host into A,B.
  - relu6 of FFN h on gpsimd (idle engine) straight from PSUM.
  - m-outer matmul groups -> 1 PSUM bank live per group, fewer stalls.
"""

from dataclasses import dataclass
from contextlib import ExitStack

import numpy as np

import concourse.bass as bass
import concourse.mybir as mybir
import concourse.tile as tile

F32 = mybir.dt.float32
BF16 = mybir.dt.bfloat16
AF = mybir.ActivationFunctionType
ALU = mybir.AluOpType

LN_EPS = 1e-5
BN_EPS = 1e-5


@dataclass
class Cfg:
    N: int = 2048
    E: int = 512
    R: int = 256
    X: int = 1024
    H: int = 8
    L: int = 3
    OUT: int = 15
    C: int = 512

    @property
    def KE(self):
        return self.E // 128

    @property
    def KX(self):
        return self.X // 128

    @property
    def NC(self):
        return self.N // self.C

    @property
    def NTT(self):
        return self.C // 128


PHASES = []


def build(nc, cfg):
    c = cfg
    E, X, H, N, C, L = c.E, c.X, c.H, c.N, c.C, c.L
    KE, KX, NC, NTT = c.KE, c.KX, c.NC, c.NTT
    E4, E2, E8 = E // 4, E // 2, E // 8

    din = {}

    def inp(name, shape, dt=BF16):
        din[name] = nc.dram_tensor(name, list(shape), dt, kind="ExternalInput")
        return din[name].ap()

    # activations (host converts to bf16)
    body_feats = inp("body_feats", (N, E))
    limb_feats = inp("limb_feats", (N, E))
    # attention weights (host-composed)
    wq = inp("wq", (L, 4, E, E))
    wkv = inp("wkv", (L, 4, E, 2 * E))
    ubq = inp("ubq", (L, 4, E), F32)
    ubkv = inp("ubkv", (L, 4, 2 * E))
    ow = inp("ow", (L, 4, E, E))
    obf = inp("ob", (L, 4, E), F32)
    # FFN
    w1 = inp("w1", (L, 2, E, X))
    b1f = inp("b1", (L, 2, X), F32)
    cwf = inp("cwf", (L, 2, 3, X), F32)     # conv taps, tap-major
    bnA = inp("bnA", (L, 2, X), F32)        # bng*rsqrt(1+eps)
    bnB = inp("bnB", (L, 2, X), F32)        # cb*A + bnb
    w2 = inp("w2", (L, 2, X, E))
    b2f = inp("b2", (L, 2, E), F32)
    lng = inp("lng", (L, 5, E), F32)
    lnb = inp("lnb", (L, 5, E), F32)
    # gating
    gw1 = inp("gw1", (L, 2 * E, E4))
    gb1 = inp("gb1", (L, E4), F32)
    gwd = inp("gwd", (L, E4))               # gw2[:,0]-gw2[:,1]
    gb2d = inp("gb2d", (L, 1), F32)         # gb2[0]-gb2[1]
    # final head
    fw1 = inp("fw1", (2 * E, E2))
    fb1 = inp("fb1", (E2,), F32)
    fw2 = inp("fw2", (E2, E))
    fb2f = inp("fb2", (E,), F32)
    flng = inp("flng", (E,), F32)
    flnb = inp("flnb", (E,), F32)
    rw1 = inp("rw1", (E, E4))
    rb1 = inp("rb1", (E4,), F32)
    rw2 = inp("rw2", (E4, E8))
    rb2 = inp("rb2", (E8,), F32)
    rw3p = inp("rw3p", (E8, 16))            # zero-padded to 16
    rb3p = inp("rb3p", (16,))               # zero-padded
    ident_in = inp("ident", (128, 128))
    ones_in = inp("ones128", (128, 128))
    hmask_in = inp("hmask", (E, H))
    cmask_in = inp("cmask", (H, E))

    out_dram = nc.dram_tensor("out", [N, c.OUT], F32, kind="ExternalOutput")

    def idram(name):
        return nc.dram_tensor(name, [E, N], BF16).ap().rearrange(
            "(k p) n -> p k n", p=128)

    rs = {}
    for s in ("b", "l"):
        rs[s, 0] = idram(f"r{s}0")
        for l in range(L):
            for st in (1, 2, 3):
                rs[s, (l, st)] = idram(f"r{s}_{l}_{st}")

    lowp = nc.allow_low_precision("bf16 activations within rel-err budget")

    with tile.TileContext(nc) as tc, ExitStack() as ctx, lowp:
        p_ = ctx.enter_context
        cst = p_(tc.tile_pool(name="cst", bufs=1))
        wbig = p_(tc.tile_pool(name="wbig", bufs=3))
        wsm = p_(tc.tile_pool(name="wsm", bufs=2))
        wcol = p_(tc.tile_pool(name="wcol", bufs=10))
        wrow = p_(tc.tile_pool(name="wrow", bufs=6))
        pa = p_(tc.tile_pool(name="pa", bufs=9))      # 4KB bf16 chunk tiles
        pb = p_(tc.tile_pool(name="pb", bufs=5))      # 8KB ht tiles
        pc = p_(tc.tile_pool(name="pc", bufs=12))      # 1KB bf16 / rows
        pat = p_(tc.tile_pool(name="pat", bufs=3))    # per-attn persistents
        phl = p_(tc.tile_pool(name="phl", bufs=6))    # conv halos
        ps = p_(tc.tile_pool(name="ps", bufs=5, space="PSUM"))
        psr = p_(tc.tile_pool(name="psr", bufs=3, space="PSUM"))

        v, sc, gp = nc.vector, nc.scalar, nc.gpsimd

        def mm(out, lhsT, rhs, start, stop):
            nc.tensor.matmul(out, lhsT, rhs, start=start, stop=stop)

        # ---- constants ----
        ident_t = cst.tile([128, 128], BF16, tag="ident")
        nc.sync.dma_start(out=ident_t, in_=ident_in)
        ones_t = cst.tile([128, 128], BF16, tag="ones")
        nc.sync.dma_start(out=ones_t, in_=ones_in)
        hmask_t = cst.tile([128, KE, H], BF16, tag="hmask")
        nc.sync.dma_start(out=hmask_t,
                          in_=hmask_in.rearrange("(k p) h -> p k h", p=128))
        cmask_t = cst.tile([H, KE, 128], BF16, tag="cmask")
        nc.sync.dma_start(out=cmask_t,
                          in_=cmask_in.rearrange("h (k p) -> h k p", p=128))
        ONES_COL = ones_t[:, 0:1]
        ONES_ROW = ones_t[0:1, :]
        onesc_t = cst.tile([1, C], BF16, tag="onesc")
        v.memset(onesc_t, 1.0)
        ONES_C = onesc_t[0:1, :]
        eps_ln = cst.tile([1, 1], F32, tag="epsl")
        v.memset(eps_ln, LN_EPS)

        def col_tile(src_ap, m, tag="col", bufs=None):
            t = wcol.tile([128, m], F32, tag=tag,
                          bufs=(12 if tag == "col" else bufs))
            nc.sync.dma_start(out=t, in_=src_ap.rearrange("(m p) -> p m", p=128))
            return t

        def row_tile(src_ap, n, tag="row", pool=None):
            t = (pool or wrow).tile([1, n], BF16, tag=tag)
            nc.sync.dma_start(out=t, in_=src_ap[None, :])
            return t

        def ln_apply(xs, g_col, b_col, outt, extra_tt=None):
            """LayerNorm over features (layout 1). xs: [128, KE, C] bf16 tile.
            outt: [128, KE, C] bf16 out. extra_tt(m): None."""
            sq = pa.tile([128, KE, C], BF16, tag="a4")
            xf = xs.rearrange("p k c -> p (k c)")
            v.tensor_tensor(out=sq.rearrange("p k c -> p (k c)"),
                            in0=xf, in1=xf, op=ALU.mult)
            ps_s = psr.tile([1, C], F32, tag="row")
            ps_ss = psr.tile([1, C], F32, tag="row")
            for m in range(KE):
                mm(ps_s, ONES_COL, xs[:, m, :], start=(m == 0),
                   stop=(m == KE - 1))
                mm(ps_ss, ONES_COL, sq[:, m, :], start=(m == 0),
                   stop=(m == KE - 1))
            msq = pc.tile([1, C], F32, tag="row", bufs=8)
            sc.activation(msq, ps_ss, AF.Copy, scale=1.0 / E)
            m2 = pc.tile([1, C], F32, tag="row", bufs=8)
            sc.activation(m2, ps_s, AF.Square, scale=1.0 / E)
            var = pc.tile([1, C], F32, tag="row", bufs=8)
            v.tensor_tensor(out=var, in0=msq, in1=m2, op=ALU.subtract)
            sd = pc.tile([1, C], F32, tag="row", bufs=8)
            sc.activation(sd, var, AF.Sqrt, bias=eps_ln[0:1, 0:1])
            srow = pc.tile([1, C], BF16, tag="rowh", bufs=8)
            v.reciprocal(out=srow, in_=sd)
            trow = pc.tile([1, C], BF16, tag="rowh", bufs=8)
            v.scalar_tensor_tensor(out=trow, in0=ps_s, scalar=1.0 / E,
                                   in1=srow, op0=ALU.mult, op1=ALU.mult)
            bc_s = ps.tile([128, C], F32, tag="mm")
            mm(bc_s, ONES_ROW, srow, start=True, stop=True)
            sb_s = pc.tile([128, C], BF16, tag="a1")
            sc.activation(sb_s, bc_s, AF.Copy)
            bc_t = ps.tile([128, C], F32, tag="mm")
            mm(bc_t, ONES_ROW, trow, start=True, stop=True)
            sb_t = pc.tile([128, C], BF16, tag="a1")
            sc.activation(sb_t, bc_t, AF.Copy)
            for m in range(KE):
                u = pc.tile([128, C], BF16, tag="a1")
                v.tensor_tensor(out=u, in0=xs[:, m, :], in1=sb_s, op=ALU.mult)
                v.tensor_tensor(out=u, in0=u, in1=sb_t, op=ALU.subtract)
                v.tensor_scalar(out=outt[:, m, :], in0=u,
                                scalar1=g_col[:, m:m + 1],
                                scalar2=b_col[:, m:m + 1],
                                op0=ALU.mult, op1=ALU.add)

        def load_x_chunk(dram_l1, ci, tag="a4"):
            xt = pa.tile([128, KE, C], BF16, tag=tag)
            nc.sync.dma_start(out=xt, in_=dram_l1[:, :, ci * C:(ci + 1) * C])
            return xt

        def store_chunk(dram_l1, ci, t):
            gp.dma_start(out=dram_l1[:, :, ci * C:(ci + 1) * C], in_=t)

        # ---- entry transpose ----
        def entry(x_ap, dst):
            for ttk in range(N // 128):
                x2 = pa.tile([128, E], BF16, tag="a4")
                nc.sync.dma_start(out=x2, in_=x_ap[ttk * 128:(ttk + 1) * 128, :])
                xt = pa.tile([128, KE, 128], BF16, tag="a4")
                for f in range(KE):
                    pt = ps.tile([128, 128], BF16, tag="mm")
                    nc.tensor.transpose(pt, x2[:, f * 128:(f + 1) * 128],
                                        ident_t)
                    sc.activation(xt[:, f, :], pt, AF.Copy)
                nc.sync.dma_start(out=dst[:, :, ttk * 128:(ttk + 1) * 128],
                                  in_=xt)

        PHASES.append(("entry", len(nc.inst_map)))
        entry(body_feats, rs["b", 0])
        entry(limb_feats, rs["l", 0])

        # ---- linear attention ----
        def attn_gen(l, a, xq_dram, xkv_dram, tail_m, tail_post):
            """Generator: yields after weight loads, after each alpha chunk
            (bd/kmm ride with the last), and after each beta chunk."""
            wqt = wbig.tile([128, KE, E], BF16, tag="w", bufs=6)
            nc.sync.dma_start(
                out=wqt, in_=wq[l, a].rearrange("(k p) e -> p k e", p=128))
            wkvt = wbig.tile([128, KE, 2 * E], BF16, tag="w2x", bufs=4)
            nc.sync.dma_start(
                out=wkvt, in_=wkv[l, a].rearrange("(k p) e -> p k e", p=128))
            owt = wbig.tile([128, KE, E], BF16, tag="w", bufs=6)
            nc.sync.dma_start(
                out=owt, in_=ow[l, a].rearrange("(k p) e -> p k e", p=128))
            ubq_col = col_tile(ubq[l, a], KE)
            ubkv_row = row_tile(ubkv[l, a], 2 * E)
            ob_col = col_tile(obf[l, a], KE)
            yield

            PHASES.append((f"attn{l}.{a}.alpha", len(nc.inst_map)))
            kv_acc = pat.tile([128, KE, 258], F32, tag="kva", bufs=2)

            for ci in range(NC):
                xt = load_x_chunk(xkv_dram, ci)
                k2f = pa.tile([128, NTT, E], BF16, tag="a4")
                v2x = pa.tile([128, NTT, 2, 258], BF16, tag="a4")
                v.memset(v2x[:, :, :, 256:258], 1.0)
                for tt in range(NTT):
                    xs = xt[:, :, tt * 128:(tt + 1) * 128]
                    pk = ps.tile([128, E], F32, tag="mm")
                    pv = ps.tile([128, E], F32, tag="mm")
                    for k in range(KE):
                        mm(pk, xs[:, k, :], wkvt[:, k, 0:E],
                           start=(k == 0), stop=False)
                        mm(pv, xs[:, k, :], wkvt[:, k, E:2 * E],
                           start=(k == 0), stop=False)
                    mm(pk, ONES_ROW, ubkv_row[:, 0:E], start=False, stop=True)
                    mm(pv, ONES_ROW, ubkv_row[:, E:2 * E], start=False,
                       stop=True)
                    ee = pc.tile([128, E], BF16, tag="a1")
                    rr = pc.tile([128, E], BF16, tag="a1")
                    sc.activation(ee, pk, AF.Exp)
                    sc.activation(rr, pk, AF.Relu)
                    v.scalar_tensor_tensor(out=k2f[:, tt, :], in0=ee,
                                           scalar=1.0, in1=rr,
                                           op0=ALU.min, op1=ALU.add)
                    v.tensor_copy(v2x[:, tt, 0, 0:256], pv[:, 0:256])
                    v.tensor_copy(v2x[:, tt, 1, 0:256], pv[:, 256:512])
                yield "a1"
                for p in range(4):
                    pkv = ps.tile([128, 258], F32, tag="mm")
                    for tt in range(NTT):
                        mm(pkv, k2f[:, tt, p * 128:(p + 1) * 128],
                           v2x[:, tt, p // 2, :],
                           start=(tt == 0), stop=(tt == NTT - 1))
                    if ci == 0:
                        sc.activation(kv_acc[:, p, :], pkv, AF.Copy)
                    else:
                        v.tensor_tensor(out=kv_acc[:, p, :],
                                        in0=kv_acc[:, p, :], in1=pkv,
                                        op=ALU.add)
                yield "a2"

            bd = pat.tile([128, KE, 128], BF16, tag="bd", bufs=2)
            v.memset(bd, 0.0)
            for p in range(4):
                h0c = (2 * p % 4) * 64
                h1c = ((2 * p + 1) % 4) * 64
                v.tensor_copy(bd[0:64, p, 0:64], kv_acc[0:64, p, h0c:h0c + 64])
                v.tensor_copy(bd[64:128, p, 64:128],
                              kv_acc[64:128, p, h1c:h1c + 64])
            kmm = pat.tile([128, KE, H], BF16, tag="km")
            for k in range(KE):
                v.tensor_scalar_mul(kmm[:, k, :], hmask_t[:, k, :],
                                    kv_acc[:, k, 256:257])
            yield "bd"

            PHASES.append((f"attn{l}.{a}.beta", len(nc.inst_map)))
            for ci in range(NC):
                xq = load_x_chunk(xq_dram, ci)
                qf = pa.tile([128, KE, C], BF16, tag="a4")
                for m in range(KE):
                    pq = ps.tile([128, C], F32, tag="mm")
                    for k in range(KE):
                        mm(pq, wqt[:, k, m * 128:(m + 1) * 128], xq[:, k, :],
                           start=(k == 0), stop=(k == KE - 1))
                    ee = pc.tile([128, C], BF16, tag="a1")
                    rr = pc.tile([128, C], BF16, tag="a1")
                    sc.activation(ee, pq, AF.Exp, bias=ubq_col[:, m:m + 1])
                    sc.activation(rr, pq, AF.Relu, bias=ubq_col[:, m:m + 1])
                    v.scalar_tensor_tensor(out=qf[:, m, :], in0=ee, scalar=1.0,
                                           in1=rr, op0=ALU.min, op1=ALU.add)
                yield "b1"
                pd = psr.tile([8, C], F32, tag="row")
                for k in range(KE):
                    mm(pd, kmm[:, k, :], qf[:, k, :], start=(k == 0),
                       stop=(k == KE - 1))
                rec = pc.tile([8, C], BF16, tag="a1")
                v.reciprocal(out=rec, in_=pd)
                att = pa.tile([128, KE, C], BF16, tag="a4")
                for m in range(KE):
                    pn = ps.tile([128, C], F32, tag="mm")
                    mm(pn, bd[:, m, :], qf[:, m, :], start=True, stop=True)
                    pr = ps.tile([128, C], F32, tag="mm")
                    mm(pr, cmask_t[:, m, :], rec, start=True, stop=True)
                    rb = pc.tile([128, C], BF16, tag="a1")
                    sc.activation(rb, pr, AF.Copy)
                    v.tensor_tensor(out=att[:, m, :], in0=pn, in1=rb,
                                    op=ALU.mult)
                yield "b2"
                for m in range(KE):
                    pos = ps.tile([128, C], F32, tag="mm")
                    for k in range(KE):
                        mm(pos, owt[:, k, m * 128:(m + 1) * 128],
                           att[:, k, :], start=(k == 0), stop=(k == KE - 1))
                    tail_m(ci, m, pos, xq, ob_col)
                tail_post(ci, xq)
                yield "b3"

        # ---- tails ----
        def make_self_tail(l, s, dst):
            g_col = col_tile(lng[l, 0 if s == "b" else 1], KE, tag="lncol", bufs=16)
            b_col = col_tile(lnb[l, 0 if s == "b" else 1], KE, tag="lncol", bufs=16)
            rt_box = [None]

            def tail_m(ci, m, pos, xq, ob_col):
                if m == 0:
                    rt_box[0] = pa.tile([128, KE, C], BF16, tag="a4",
                                        name="rt")
                v.scalar_tensor_tensor(out=rt_box[0][:, m, :], in0=pos,
                                       scalar=ob_col[:, m:m + 1],
                                       in1=xq[:, m, :],
                                       op0=ALU.add, op1=ALU.add)

            def tail_post(ci, xq):
                rt = rt_box[0]
                outt = pa.tile([128, KE, C], BF16, tag="a4")
                ln_apply(rt, g_col, b_col, outt)
                store_chunk(dst, ci, outt)

            return tail_m, tail_post

        def make_cross_tail(l, s, dst):
            gw1t = wsm.tile([128, 2 * KE, E4], BF16, tag="ws")
            nc.sync.dma_start(out=gw1t,
                              in_=gw1[l].rearrange("(k p) g -> p k g", p=128))
            gwd_col = wcol.tile([128, 1], BF16, tag="gwd")
            nc.sync.dma_start(out=gwd_col, in_=gwd[l][:, None])
            gb1_col = col_tile(gb1[l], 1, tag="lncol", bufs=16)
            gb2d_t = pat.tile([1, 1], F32, tag="gb2d")
            nc.sync.dma_start(out=gb2d_t, in_=gb2d[l][None, :])
            g_col = col_tile(lng[l, 2], KE, tag="lncol", bufs=16)
            b_col = col_tile(lnb[l, 2], KE, tag="lncol", bufs=16)
            proj_box = [None]

            def tail_m(ci, m, pos, xq, ob_col):
                if m == 0:
                    proj_box[0] = pa.tile([128, KE, C], BF16, tag="a4",
                                          name="proj")
                sc.activation(proj_box[0][:, m, :], pos, AF.Identity,
                              bias=ob_col[:, m:m + 1])

            def tail_post(ci, xq):
                proj = proj_box[0]
                pg = ps.tile([128, C], F32, tag="mm")
                for k in range(2 * KE):
                    rhs = xq[:, k, :] if k < KE else proj[:, k - KE, :]
                    mm(pg, gw1t[:, k, :], rhs, start=(k == 0),
                       stop=(k == 2 * KE - 1))
                g1 = pc.tile([128, C], BF16, tag="a1")
                sc.activation(g1, pg, AF.Relu, bias=gb1_col[:, 0:1])
                g1t = pc.tile([128, C], BF16, tag="a1")
                v.tensor_scalar_min(g1t, g1, 6.0)
                pg2 = psr.tile([1, C], F32, tag="row")
                mm(pg2, gwd_col, g1t, start=True, stop=True)
                bg = pc.tile([1, C], BF16, tag="rowh", bufs=8)
                sc.activation(bg, pg2, AF.Sigmoid, bias=gb2d_t[0:1, 0:1])
                pbg = ps.tile([128, C], F32, tag="mm")
                mm(pbg, ONES_ROW, bg, start=True, stop=True)
                bgb = pc.tile([128, C], BF16, tag="a1")
                sc.activation(bgb, pbg, AF.Copy)
                mt = pa.tile([128, KE, C], BF16, tag="a4")
                for m in range(KE):
                    dtmp = pc.tile([128, C], BF16, tag="a1")
                    v.tensor_tensor(out=dtmp, in0=xq[:, m, :],
                                    in1=proj[:, m, :], op=ALU.subtract)
                    v.tensor_tensor(out=dtmp, in0=dtmp, in1=bgb, op=ALU.mult)
                    v.tensor_tensor(out=mt[:, m, :], in0=dtmp,
                                    in1=proj[:, m, :], op=ALU.add)
                outt = pa.tile([128, KE, C], BF16, tag="a4")
                ln_apply(mt, g_col, b_col, outt)
                store_chunk(dst, ci, outt)

            return tail_m, tail_post

        # ---- FFN ----
        def ffn_gen(l, s, src, dst):
            si = 0 if s == "b" else 1
            w1t = wbig.tile([128, KE, X], BF16, tag="w2x", bufs=4)
            nc.sync.dma_start(
                out=w1t, in_=w1[l, si].rearrange("(k p) x -> p k x", p=128))
            w2t = wbig.tile([128, KX, E], BF16, tag="w2x", bufs=4)
            nc.sync.dma_start(
                out=w2t, in_=w2[l, si].rearrange("(k p) e -> p k e", p=128))
            b1_col = col_tile(b1f[l, si], KX, tag="ffcol", bufs=16)
            b2_col = col_tile(b2f[l, si], KE, tag="ffcol", bufs=16)
            w0_col = col_tile(cwf[l, si, 0], KX, tag="ffcol", bufs=16)
            w1c_col = col_tile(cwf[l, si, 1], KX, tag="ffcol", bufs=16)
            w2_col = col_tile(cwf[l, si, 2], KX, tag="ffcol", bufs=16)
            A_col = col_tile(bnA[l, si], KX, tag="ffcol", bufs=16)
            B_col = col_tile(bnB[l, si], KX, tag="ffcol", bufs=16)
            g_col = col_tile(lng[l, 3 if s == "b" else 4], KE, tag="lncol", bufs=16)
            bb_col = col_tile(lnb[l, 3 if s == "b" else 4], KE, tag="lncol", bufs=16)
            yield
            PHASES.append((f"ffn{l}.{s}", len(nc.inst_map)))

            hts = [None] * NC
            xts = [None] * NC
            hl0 = [None] * NC
            hf2 = [None] * NC

            def compute_h(ci):
                xt = load_x_chunk(src, ci)
                xts[ci] = xt
                ht = pb.tile([128, KX, C], BF16, tag="a8")
                for m in range(KX):
                    ph = ps.tile([128, C], F32, tag="mm")
                    for k in range(KE):
                        mm(ph, w1t[:, k, m * 128:(m + 1) * 128], xt[:, k, :],
                           start=(k == 0), stop=(k == KE - 1))
                    hf = pc.tile([128, C], BF16, tag="a1")
                    v.tensor_scalar(out=hf, in0=ph,
                                    scalar1=b1_col[:, m:m + 1],
                                    scalar2=0.0, op0=ALU.add, op1=ALU.max)
                    v.tensor_scalar_min(ht[:, m, :], hf, 6.0)
                hts[ci] = ht
                l0 = phl.tile([128, KX, 1], BF16, tag="hl")
                f2 = phl.tile([128, KX, 1], BF16, tag="hf")
                for m in range(KX):
                    v.tensor_scalar_mul(l0[:, m, :], ht[:, m, C - 1:C],
                                        w0_col[:, m:m + 1])
                    v.tensor_scalar_mul(f2[:, m, :], ht[:, m, 0:1],
                                        w2_col[:, m:m + 1])
                hl0[ci], hf2[ci] = l0, f2

            def conv_elem(ci):
                ht = hts[ci]
                h2 = pb.tile([128, KX, C], BF16, tag="a8")
                for m in range(KX):
                    acc = pc.tile([128, C], BF16, tag="a1")
                    v.tensor_scalar_mul(acc, ht[:, m, :], w1c_col[:, m:m + 1])
                    v.scalar_tensor_tensor(out=acc[:, 1:C],
                                           in0=ht[:, m, 0:C - 1],
                                           scalar=w0_col[:, m:m + 1],
                                           in1=acc[:, 1:C],
                                           op0=ALU.mult, op1=ALU.add)
                    if ci > 0:
                        v.tensor_tensor(out=acc[:, 0:1], in0=acc[:, 0:1],
                                        in1=hl0[ci - 1][:, m, :], op=ALU.add)
                    v.scalar_tensor_tensor(out=acc[:, 0:C - 1],
                                           in0=ht[:, m, 1:C],
                                           scalar=w2_col[:, m:m + 1],
                                           in1=acc[:, 0:C - 1],
                                           op0=ALU.mult, op1=ALU.add)
                    if ci < NC - 1:
                        v.tensor_tensor(out=acc[:, C - 1:C],
                                        in0=acc[:, C - 1:C],
                                        in1=hf2[ci + 1][:, m, :], op=ALU.add)
                    a2 = pc.tile([128, C], BF16, tag="a1")
                    sc.activation(a2, acc, AF.Relu, scale=A_col[:, m:m + 1],
                                  bias=B_col[:, m:m + 1])
                    v.tensor_scalar_min(h2[:, m, :], a2, 6.0)
                return h2

            def conv_pw(ci, h2):
                rt = pa.tile([128, KE, C], BF16, tag="a4")
                for m in range(KE):
                    pw = ps.tile([128, C], F32, tag="mm")
                    for k in range(KX):
                        mm(pw, w2t[:, k, m * 128:(m + 1) * 128], h2[:, k, :],
                           start=(k == 0), stop=(k == KX - 1))
                    v.scalar_tensor_tensor(out=rt[:, m, :], in0=pw,
                                           scalar=b2_col[:, m:m + 1],
                                           in1=xts[ci][:, m, :],
                                           op0=ALU.add, op1=ALU.add)
                outt = pa.tile([128, KE, C], BF16, tag="a4")
                ln_apply(rt, g_col, bb_col, outt)
                store_chunk(dst, ci, outt)
                hts[ci] = xts[ci] = None

            compute_h(0)
            yield "h"
            compute_h(1)
            yield "h"
            for ci in range(NC - 1):
                h2 = conv_elem(ci)
                yield "cv"
                conv_pw(ci, h2)
                yield "pw"
                if ci + 2 < NC:
                    compute_h(ci + 2)
                    yield "h"
            h2 = conv_elem(NC - 1)
            yield "cv"
            conv_pw(NC - 1, h2)
            yield "pw"

        # ---- layers (chunk-interleaved across independent streams) ----
        def adv(g, n=1):
            for _ in range(n):
                next(g, None)

        for l in range(L):
            bsrc = rs["b", 0] if l == 0 else rs["b", (l - 1, 3)]
            lsrc = rs["l", 0] if l == 0 else rs["l", (l - 1, 3)]
            g0 = attn_gen(l, 0, bsrc, bsrc,
                          *make_self_tail(l, "b", rs["b", (l, 1)]))
            g1 = attn_gen(l, 1, lsrc, lsrc,
                          *make_self_tail(l, "l", rs["l", (l, 1)]))
            g2 = attn_gen(l, 2, rs["b", (l, 1)], rs["l", (l, 1)],
                          *make_cross_tail(l, "b", rs["b", (l, 2)]))
            g3 = attn_gen(l, 3, rs["l", (l, 1)], rs["b", (l, 1)],
                          *make_cross_tail(l, "l", rs["l", (l, 2)]))
            gb = ffn_gen(l, "b", rs["b", (l, 2)], rs["b", (l, 3)])
            gl = ffn_gen(l, "l", rs["l", (l, 2)], rs["l", (l, 3)])
            adv(g0)                     # weights
            adv(g1)
            for _ in range(2 * NC):     # self alphas: a1/a2 staged
                adv(g0)
                adv(g1)
            adv(g0)                     # bd/kmm
            adv(g1)
            adv(g2)                     # prefetch cross weights
            adv(g3)
            for _ in range(3 * NC):     # self betas: b1/b2/b3 staged
                adv(g0)
                adv(g1)
            for _ in range(2 * NC):     # cross alphas staged
                adv(g2)
                adv(g3)
            adv(g2)                     # bd/kmm
            adv(g3)
            adv(gb)                     # prefetch ffn weights
            adv(gl)
            for _ in range(3 * NC):     # cross betas staged
                adv(g2)
                adv(g3)
            for _ in range(12):         # ffn h/cv/pw staged
                adv(gb)
                adv(gl)

        PHASES.append(("final", len(nc.inst_map)))
        # ---- final head ----
        fw1t = wbig.tile([128, 2 * KE, E2], BF16, tag="w", bufs=6)
        nc.sync.dma_start(out=fw1t,
                          in_=fw1.rearrange("(k p) g -> p k g", p=128))
        fw2t = wsm.tile([128, 2, E], BF16, tag="wfin", bufs=4)
        nc.sync.dma_start(out=fw2t,
                          in_=fw2.rearrange("(k p) e -> p k e", p=128))
        rw1t = wsm.tile([128, KE, E4], BF16, tag="wfin", bufs=4)
        nc.sync.dma_start(out=rw1t,
                          in_=rw1.rearrange("(k p) g -> p k g", p=128))
        rw2t = wsm.tile([128, E8], BF16, tag="wfin", bufs=4)
        nc.sync.dma_start(out=rw2t, in_=rw2)
        rw3t = wsm.tile([E8, 16], BF16, tag="wfin", bufs=4)
        nc.sync.dma_start(out=rw3t, in_=rw3p)
        rb3_row = row_tile(rb3p, 16)
        fb2_col = col_tile(fb2f, KE, tag="fcol")
        fb1_col = col_tile(fb1, 2, tag="fcol")
        flng_col = col_tile(flng, KE, tag="fcol")
        flnb_col = col_tile(flnb, KE, tag="fcol")
        rb1_col = col_tile(rb1, 1, tag="fcol")
        rb2_col = wcol.tile([E8, 1], F32, tag="fcol")
        nc.sync.dma_start(out=rb2_col, in_=rb2[:, None])
        out_ap = out_dram.ap()

        bsrc, lsrc = rs["b", (L - 1, 3)], rs["l", (L - 1, 3)]
        for ci in range(NC):
            xb = load_x_chunk(bsrc, ci)
            xl = load_x_chunk(lsrc, ci)
            f1t = pa.tile([128, 2, C], BF16, tag="a4")
            for m in range(2):
                pf = ps.tile([128, C], F32, tag="mm")
                for k in range(2 * KE):
                    rhs = xb[:, k, :] if k < KE else xl[:, k - KE, :]
                    mm(pf, fw1t[:, k, m * 128:(m + 1) * 128], rhs,
                       start=(k == 0), stop=(k == 2 * KE - 1))
                f1 = pc.tile([128, C], BF16, tag="a1")
                sc.activation(f1, pf, AF.Relu, bias=fb1_col[:, m:m + 1])
                v.tensor_scalar_min(f1t[:, m, :], f1, 6.0)
            ft = pa.tile([128, KE, C], BF16, tag="a4")
            for m in range(KE):
                pf2 = ps.tile([128, C], F32, tag="mm")
                for k in range(2):
                    mm(pf2, fw2t[:, k, m * 128:(m + 1) * 128], f1t[:, k, :],
                       start=(k == 0), stop=(k == 1))
                sc.activation(ft[:, m, :], pf2, AF.Identity,
                              bias=fb2_col[:, m:m + 1])
            frt = pa.tile([128, KE, C], BF16, tag="a4")
            ln_apply(ft, flng_col, flnb_col, frt)
            # relu after LN
            frf = frt.rearrange("p k c -> p (k c)")
            v.tensor_scalar_max(frf, frf, 0.0)
            p1 = ps.tile([128, C], F32, tag="mm")
            for k in range(KE):
                mm(p1, rw1t[:, k, :], frt[:, k, :], start=(k == 0),
                   stop=(k == KE - 1))
            h1f = pc.tile([128, C], BF16, tag="a1")
            sc.activation(h1f, p1, AF.Relu, bias=rb1_col[:, 0:1])
            h1t = pc.tile([128, C], BF16, tag="a1")
            v.tensor_scalar_min(h1t, h1f, 6.0)
            p2 = ps.tile([E8, C], F32, tag="mm")
            mm(p2, rw2t, h1t, start=True, stop=True)
            h2f = pc.tile([E8, C], BF16, tag="a1")
            sc.activation(h2f, p2, AF.Relu, bias=rb2_col[:, 0:1])
            h2t = pc.tile([E8, C], BF16, tag="a1")
            v.tensor_scalar_min(h2t, h2f, 6.0)
            ot = pc.tile([128, NTT, c.OUT], F32, tag="a1")
            for tt in range(NTT):
                p3 = ps.tile([128, 16], F32, tag="mm")
                mm(p3, h2t[:, tt * 128:(tt + 1) * 128], rw3t,
                   start=True, stop=False)
                mm(p3, ONES_ROW, rb3_row, start=False, stop=True)
                sc.activation(ot[:, tt, :], p3[:, 0:c.OUT], AF.Copy)
            nc.sync.dma_start(
                out=out_ap[ci * C:(ci + 1) * C, :].rearrange(
                    "(tt p) o -> p tt o", p=128),
                in_=ot)

    return din, out_dram


# ======================================================================
# kernel() entry point: full inputs in, full outputs out (8-core SPMD).
# ======================================================================
import concourse.bacc as _bacc
from concourse.bass_utils import run_bass_kernel_spmd as _run_spmd

_N_CORES = 8
_CACHE = {}


def _get_nc():
    if "nc" not in _CACHE:
        nc = _bacc.Bacc("TRN2", target_bir_lowering=False, debug=False)
        build(nc, Cfg())
        nc.finalize()
        _CACHE["nc"] = nc
    return _CACHE["nc"]


def _bf16(x):
    import ml_dtypes
    return np.asarray(x, dtype=np.float32).astype(ml_dtypes.bfloat16)


def host_prep(inputs):
    """Host-side weight preprocessing: compose QKV, fold BN, convert bf16."""
    c = Cfg()
    E, X, H, L = c.E, c.X, c.H, c.L
    E4, E2, E8 = E // 4, E // 2, E // 8
    f = {k: np.asarray(v, dtype=np.float32) for k, v in inputs.items()}
    dw, uw, ub = f["dw"], f["uw"], f["ub"]
    # composed q and k|v projection weights
    wq = np.matmul(dw[:, :, 0], uw[:, :, 0])          # (L,4,E,E)
    wk = np.matmul(dw[:, :, 1], uw[:, :, 1])
    wv = np.matmul(dw[:, :, 2], uw[:, :, 2])
    wkv = np.concatenate([wk, wv], axis=-1)           # (L,4,E,2E)
    ubq = ub[:, :, 0]                                 # (L,4,E)
    ubkv = np.concatenate([ub[:, :, 1], ub[:, :, 2]], axis=-1)
    rsq = np.float32(1.0 / np.sqrt(1.0 + BN_EPS))
    A = f["bng"] * rsq                                # (L,2,X)
    B = f["cb"] * A + f["bnb"]
    cwf = f["cw"].transpose(0, 1, 3, 2).copy()        # (L,2,3,X)
    gwd = f["gw2"][:, :, 0] - f["gw2"][:, :, 1]       # (L,E4)
    gb2d = (f["gb2"][:, 0] - f["gb2"][:, 1])[:, None]  # (L,1)
    rw3p = np.zeros((E8, 16), np.float32)
    rw3p[:, :c.OUT] = f["rw3"]
    rb3p = np.zeros((16,), np.float32)
    rb3p[:c.OUT] = f["rb3"]
    dh = E // H
    ident = np.eye(128, dtype=np.float32)
    ones = np.ones((128, 128), dtype=np.float32)
    hmask = np.zeros((E, H), dtype=np.float32)
    for ff in range(E):
        hmask[ff, ff // dh] = 1.0
    cmask = hmask.T.copy()

    b16 = dict(wq=wq, wkv=wkv, ubkv=ubkv, ow=f["ow"],
               w1=f["w1"], w2=f["w2"],
               gw1=f["gw1"], gwd=gwd, fw1=f["fw1"], fw2=f["fw2"],
               rw1=f["rw1"], rw2=f["rw2"], rw3p=rw3p,
               rb3p=rb3p, ident=ident, ones128=ones, hmask=hmask,
               cmask=cmask)
    f32 = dict(ubq=ubq, ob=f["ob"], b1=f["b1"], b2=f["b2"], fb2=f["fb2"],
               cwf=cwf, bnA=A, bnB=B,
               lng=f["lng"], lnb=f["lnb"],
               gb1=f["gb1"], gb2d=gb2d, fb1=f["fb1"], flng=f["flng"],
               flnb=f["flnb"], rb1=f["rb1"], rb2=f["rb2"])
    shared = {k: _bf16(v) for k, v in b16.items()}
    shared.update({k: np.ascontiguousarray(v, dtype=np.float32)
                   for k, v in f32.items()})
    return shared, f["body_feats"], f["limb_feats"]


def kernel(**inputs):
    nc = _get_nc()
    shared, body, limb = host_prep(inputs)
    in_maps = []
    for i in range(_N_CORES):
        m = dict(shared)
        m["body_feats"] = _bf16(body[i])
        m["limb_feats"] = _bf16(limb[i])
        in_maps.append(m)
    res = run_kernel_spmd_cached(nc, in_maps)
    out = np.stack([res[i]["out"] for i in range(_N_CORES)], axis=0)
    return out.astype(np.float32)


def run_kernel_spmd_cached(nc, in_maps, **kw):
    r = _run_spmd(nc, in_maps, list(range(_N_CORES)), **kw)
    _CACHE["last_result"] = r
    return r.results


# revision 4
# speedup vs baseline: 1.1470x; 1.0193x over previous
"""Dual-stream linear-attention transformer — bf16 redesign (per-core).

Layout convention (same as baseline):
  - "layout 1" activation: [E, N] feature-major; SBUF tiles [128, KE, C]
    (feature f = 128*k + p -> partition p, k-th slice; tokens on free dim).
  - alpha k/v are produced token-major per 128-token tile [128, E].
  - Residual streams live in internal DRAM as [E, N] bf16.

Key changes vs baseline:
  - All matmul operands + SBUF activations bf16 (same PE rate as f32r>=256,
    but DVE tensor_tensor 2x / tensor_scalar 4x, half DMA bytes).
  - QKV down+up projections composed into single E x E / E x 2E weights on
    the host (removes the low-rank intermediate copies).
  - Biases folded into matmuls as rank-1 accumulates (ones_row x bias_row).
  - elu+1 via 3 ops: ACT Exp, ACT Relu, DVE scalar_tensor_tensor(min,add).
  - LayerNorm apply via 3 bf16 DVE ops per slice (TT,TT,TS) instead of
    gpsimd tensor_tensor pairs.
  - FFN dwconv via TS/STT chain on DVE; BN folded on host into A,B.
  - relu6 of FFN h on gpsimd (idle engine) straight from PSUM.
  - m-outer matmul groups -> 1 PSUM bank live per group, fewer stalls.
"""

from dataclasses import dataclass
from contextlib import ExitStack

import numpy as np

import concourse.bass as bass
import concourse.mybir as mybir
import concourse.tile as tile

F32 = mybir.dt.float32
BF16 = mybir.dt.bfloat16
AF = mybir.ActivationFunctionType
ALU = mybir.AluOpType

LN_EPS = 1e-5
BN_EPS = 1e-5


@dataclass
class Cfg:
    N: int = 2048
    E: int = 512
    R: int = 256
    X: int = 1024
    H: int = 8
    L: int = 3
    OUT: int = 15
    C: int = 512

    @property
    def KE(self):
        return self.E // 128

    @property
    def KX(self):
        return self.X // 128

    @property
    def NC(self):
        return self.N // self.C

    @property
    def NTT(self):
        return self.C // 128


PHASES = []


def build(nc, cfg):
    c = cfg
    E, X, H, N, C, L = c.E, c.X, c.H, c.N, c.C, c.L
    KE, KX, NC, NTT = c.KE, c.KX, c.NC, c.NTT
    E4, E2, E8 = E // 4, E // 2, E // 8

    din = {}

    def inp(name, shape, dt=BF16):
        din[name] = nc.dram_tensor(name, list(shape), dt, kind="ExternalInput")
        return din[name].ap()

    # activations (host converts to bf16)
    body_feats = inp("body_feats", (N, E))
    limb_feats = inp("limb_feats", (N, E))
    # attention weights (host-composed)
    wq = inp("wq", (L, 4, E, E))
    wkv = inp("wkv", (L, 4, E, 2 * E))
    ubq = inp("ubq", (L, 4, E), F32)
    ubkv = inp("ubkv", (L, 4, 2 * E))
    ow = inp("ow", (L, 4, E, E))
    obf = inp("ob", (L, 4, E), F32)
    # FFN
    w1 = inp("w1", (L, 2, E, X))
    b1f = inp("b1", (L, 2, X), F32)
    cwf = inp("cwf", (L, 2, 3, X), F32)     # conv taps, tap-major
    bnA = inp("bnA", (L, 2, X), F32)        # bng*rsqrt(1+eps)
    bnB = inp("bnB", (L, 2, X), F32)        # cb*A + bnb
    w2 = inp("w2", (L, 2, X, E))
    b2f = inp("b2", (L, 2, E), F32)
    lng = inp("lng", (L, 5, E), F32)
    lnb = inp("lnb", (L, 5, E), F32)
    # gating
    gw1 = inp("gw1", (L, 2 * E, E4))
    gb1 = inp("gb1", (L, E4), F32)
    gwd = inp("gwd", (L, E4))               # gw2[:,0]-gw2[:,1]
    gb2d = inp("gb2d", (L, 1), F32)         # gb2[0]-gb2[1]
    # final head
    fw1 = inp("fw1", (2 * E, E2))
    fb1 = inp("fb1", (E2,), F32)
    fw2 = inp("fw2", (E2, E))
    fb2f = inp("fb2", (E,), F32)
    flng = inp("flng", (E,), F32)
    flnb = inp("flnb", (E,), F32)
    rw1 = inp("rw1", (E, E4))
    rb1 = inp("rb1", (E4,), F32)
    rw2 = inp("rw2", (E4, E8))
    rb2 = inp("rb2", (E8,), F32)
    rw3p = inp("rw3p", (E8, 16))            # zero-padded to 16
    rb3p = inp("rb3p", (16,))               # zero-padded
    ident_in = inp("ident", (128, 128))
    ones_in = inp("ones128", (128, 128))
    hmask_in = inp("hmask", (E, H))
    cmask_in = inp("cmask", (H, E))

    out_dram = nc.dram_tensor("out", [N, c.OUT], F32, kind="ExternalOutput")

    def idram(name):
        return nc.dram_tensor(name, [E, N], BF16).ap().rearrange(
            "(k p) n -> p k n", p=128)

    rs = {}
    for s in ("b", "l"):
        rs[s, 0] = idram(f"r{s}0")
        for l in range(L):
            for st in (1, 2, 3):
                rs[s, (l, st)] = idram(f"r{s}_{l}_{st}")

    lowp = nc.allow_low_precision("bf16 activations within rel-err budget")

    with tile.TileContext(nc) as tc, ExitStack() as ctx, lowp:
        p_ = ctx.enter_context
        cst = p_(tc.tile_pool(name="cst", bufs=1))
        wbig = p_(tc.tile_pool(name="wbig", bufs=3))
        wsm = p_(tc.tile_pool(name="wsm", bufs=2))
        wcol = p_(tc.tile_pool(name="wcol", bufs=10))
        wrow = p_(tc.tile_pool(name="wrow", bufs=6))
        pa = p_(tc.tile_pool(name="pa", bufs=9))      # 4KB bf16 chunk tiles
        pb = p_(tc.tile_pool(name="pb", bufs=5))      # 8KB ht tiles
        pc = p_(tc.tile_pool(name="pc", bufs=12))      # 1KB bf16 / rows
        pat = p_(tc.tile_pool(name="pat", bufs=3))    # per-attn persistents
        phl = p_(tc.tile_pool(name="phl", bufs=6))    # conv halos
        ps = p_(tc.tile_pool(name="ps", bufs=5, space="PSUM"))
        psr = p_(tc.tile_pool(name="psr", bufs=3, space="PSUM"))

        v, sc, gp = nc.vector, nc.scalar, nc.gpsimd

        def mm(out, lhsT, rhs, start, stop):
            nc.tensor.matmul(out, lhsT, rhs, start=start, stop=stop)

        # ---- constants ----
        ident_t = cst.tile([128, 128], BF16, tag="ident")
        nc.sync.dma_start(out=ident_t, in_=ident_in)
        ones_t = cst.tile([128, 128], BF16, tag="ones")
        nc.sync.dma_start(out=ones_t, in_=ones_in)
        hmask_t = cst.tile([128, KE, H], BF16, tag="hmask")
        nc.sync.dma_start(out=hmask_t,
                          in_=hmask_in.rearrange("(k p) h -> p k h", p=128))
        cmask_t = cst.tile([H, KE, 128], BF16, tag="cmask")
        nc.sync.dma_start(out=cmask_t,
                          in_=cmask_in.rearrange("h (k p) -> h k p", p=128))
        ONES_COL = ones_t[:, 0:1]
        ONES_ROW = ones_t[0:1, :]
        onesc_t = cst.tile([1, C], BF16, tag="onesc")
        v.memset(onesc_t, 1.0)
        ONES_C = onesc_t[0:1, :]
        eps_ln = cst.tile([1, 1], F32, tag="epsl")
        v.memset(eps_ln, LN_EPS)

        def col_tile(src_ap, m, tag="col", bufs=None):
            t = wcol.tile([128, m], F32, tag=tag,
                          bufs=(12 if tag == "col" else bufs))
            nc.sync.dma_start(out=t, in_=src_ap.rearrange("(m p) -> p m", p=128))
            return t

        def row_tile(src_ap, n, tag="row", pool=None):
            t = (pool or wrow).tile([1, n], BF16, tag=tag)
            nc.sync.dma_start(out=t, in_=src_ap[None, :])
            return t

        def ln_apply(xs, g_col, b_col, outt, extra_tt=None):
            """LayerNorm over features (layout 1). xs: [128, KE, C] bf16 tile.
            outt: [128, KE, C] bf16 out. extra_tt(m): None."""
            sq = pa.tile([128, KE, C], BF16, tag="a4")
            xf = xs.rearrange("p k c -> p (k c)")
            v.tensor_tensor(out=sq.rearrange("p k c -> p (k c)"),
                            in0=xf, in1=xf, op=ALU.mult)
            ps_s = psr.tile([1, C], F32, tag="row")
            ps_ss = psr.tile([1, C], F32, tag="row")
            for m in range(KE):
                mm(ps_s, ONES_COL, xs[:, m, :], start=(m == 0),
                   stop=(m == KE - 1))
                mm(ps_ss, ONES_COL, sq[:, m, :], start=(m == 0),
                   stop=(m == KE - 1))
            msq = pc.tile([1, C], F32, tag="row", bufs=8)
            sc.activation(msq, ps_ss, AF.Copy, scale=1.0 / E)
            m2 = pc.tile([1, C], F32, tag="row", bufs=8)
            sc.activation(m2, ps_s, AF.Square, scale=1.0 / E)
            var = pc.tile([1, C], F32, tag="row", bufs=8)
            v.tensor_tensor(out=var, in0=msq, in1=m2, op=ALU.subtract)
            srow = pc.tile([1, C], BF16, tag="rowh", bufs=8)
            sc.activation(srow, var, AF.Abs_reciprocal_sqrt,
                          bias=eps_ln[0:1, 0:1])
            trow = pc.tile([1, C], BF16, tag="rowh", bufs=8)
            v.scalar_tensor_tensor(out=trow, in0=ps_s, scalar=1.0 / E,
                                   in1=srow, op0=ALU.mult, op1=ALU.mult)
            bc_s = ps.tile([128, C], F32, tag="mm")
            mm(bc_s, ONES_ROW, srow, start=True, stop=True)
            sb_s = pc.tile([128, C], BF16, tag="a1")
            sc.activation(sb_s, bc_s, AF.Copy)
            bc_t = ps.tile([128, C], F32, tag="mm")
            mm(bc_t, ONES_ROW, trow, start=True, stop=True)
            sb_t = pc.tile([128, C], BF16, tag="a1")
            sc.activation(sb_t, bc_t, AF.Copy)
            for m in range(KE):
                u = pc.tile([128, C], BF16, tag="a1")
                v.tensor_tensor(out=u, in0=xs[:, m, :], in1=sb_s, op=ALU.mult)
                v.tensor_tensor(out=u, in0=u, in1=sb_t, op=ALU.subtract)
                v.tensor_scalar(out=outt[:, m, :], in0=u,
                                scalar1=g_col[:, m:m + 1],
                                scalar2=b_col[:, m:m + 1],
                                op0=ALU.mult, op1=ALU.add)

        def load_x_chunk(dram_l1, ci, tag="a4"):
            xt = pa.tile([128, KE, C], BF16, tag=tag)
            nc.sync.dma_start(out=xt, in_=dram_l1[:, :, ci * C:(ci + 1) * C])
            return xt

        def store_chunk(dram_l1, ci, t):
            gp.dma_start(out=dram_l1[:, :, ci * C:(ci + 1) * C], in_=t)

        # ---- entry transpose ----
        def entry(x_ap, dst):
            for ttk in range(N // 128):
                x2 = pa.tile([128, E], BF16, tag="a4")
                nc.sync.dma_start(out=x2, in_=x_ap[ttk * 128:(ttk + 1) * 128, :])
                xt = pa.tile([128, KE, 128], BF16, tag="a4")
                for f in range(KE):
                    pt = ps.tile([128, 128], BF16, tag="mm")
                    nc.tensor.transpose(pt, x2[:, f * 128:(f + 1) * 128],
                                        ident_t)
                    sc.activation(xt[:, f, :], pt, AF.Copy)
                nc.sync.dma_start(out=dst[:, :, ttk * 128:(ttk + 1) * 128],
                                  in_=xt)

        PHASES.append(("entry", len(nc.inst_map)))
        entry(body_feats, rs["b", 0])
        entry(limb_feats, rs["l", 0])

        # ---- linear attention ----
        def attn_gen(l, a, xq_dram, xkv_dram, tail_m, tail_post):
            """Generator: yields after weight loads, after each alpha chunk
            (bd/kmm ride with the last), and after each beta chunk."""
            wqt = wbig.tile([128, KE, E], BF16, tag="w", bufs=6)
            nc.sync.dma_start(
                out=wqt, in_=wq[l, a].rearrange("(k p) e -> p k e", p=128))
            wkvt = wbig.tile([128, KE, 2 * E], BF16, tag="w2x", bufs=4)
            nc.sync.dma_start(
                out=wkvt, in_=wkv[l, a].rearrange("(k p) e -> p k e", p=128))
            owt = wbig.tile([128, KE, E], BF16, tag="w", bufs=6)
            nc.sync.dma_start(
                out=owt, in_=ow[l, a].rearrange("(k p) e -> p k e", p=128))
            ubq_col = col_tile(ubq[l, a], KE)
            ubkv_row = row_tile(ubkv[l, a], 2 * E)
            ob_col = col_tile(obf[l, a], KE)
            yield

            PHASES.append((f"attn{l}.{a}.alpha", len(nc.inst_map)))
            kv_acc = pat.tile([128, KE, 258], F32, tag="kva", bufs=2)

            for ci in range(NC):
                xt = load_x_chunk(xkv_dram, ci)
                k2f = pa.tile([128, NTT, E], BF16, tag="a4")
                v2x = pa.tile([128, NTT, 2, 258], BF16, tag="a4")
                v.memset(v2x[:, :, :, 256:258], 1.0)
                for tt in range(NTT):
                    xs = xt[:, :, tt * 128:(tt + 1) * 128]
                    pk = ps.tile([128, E], F32, tag="mm")
                    pv = ps.tile([128, E], F32, tag="mm")
                    for k in range(KE):
                        mm(pk, xs[:, k, :], wkvt[:, k, 0:E],
                           start=(k == 0), stop=False)
                        mm(pv, xs[:, k, :], wkvt[:, k, E:2 * E],
                           start=(k == 0), stop=False)
                    mm(pk, ONES_ROW, ubkv_row[:, 0:E], start=False, stop=True)
                    mm(pv, ONES_ROW, ubkv_row[:, E:2 * E], start=False,
                       stop=True)
                    ee = pc.tile([128, E], BF16, tag="a1")
                    rr = pc.tile([128, E], BF16, tag="a1")
                    sc.activation(ee, pk, AF.Exp)
                    sc.activation(rr, pk, AF.Relu)
                    v.scalar_tensor_tensor(out=k2f[:, tt, :], in0=ee,
                                           scalar=1.0, in1=rr,
                                           op0=ALU.min, op1=ALU.add)
                    v.tensor_copy(v2x[:, tt, 0, 0:256], pv[:, 0:256])
                    v.tensor_copy(v2x[:, tt, 1, 0:256], pv[:, 256:512])
                yield "a1"
                for p in range(4):
                    pkv = ps.tile([128, 258], F32, tag="mm")
                    for tt in range(NTT):
                        mm(pkv, k2f[:, tt, p * 128:(p + 1) * 128],
                           v2x[:, tt, p // 2, :],
                           start=(tt == 0), stop=(tt == NTT - 1))
                    if ci == 0:
                        sc.activation(kv_acc[:, p, :], pkv, AF.Copy)
                    else:
                        v.tensor_tensor(out=kv_acc[:, p, :],
                                        in0=kv_acc[:, p, :], in1=pkv,
                                        op=ALU.add)
                yield "a2"

            bd = pat.tile([128, KE, 128], BF16, tag="bd", bufs=2)
            v.memset(bd, 0.0)
            for p in range(4):
                h0c = (2 * p % 4) * 64
                h1c = ((2 * p + 1) % 4) * 64
                v.tensor_copy(bd[0:64, p, 0:64], kv_acc[0:64, p, h0c:h0c + 64])
                v.tensor_copy(bd[64:128, p, 64:128],
                              kv_acc[64:128, p, h1c:h1c + 64])
            kmm = pat.tile([128, KE, H], BF16, tag="km")
            for k in range(KE):
                v.tensor_scalar_mul(kmm[:, k, :], hmask_t[:, k, :],
                                    kv_acc[:, k, 256:257])
            yield "bd"

            PHASES.append((f"attn{l}.{a}.beta", len(nc.inst_map)))
            for ci in range(NC):
                xq = load_x_chunk(xq_dram, ci)
                qf = pa.tile([128, KE, C], BF16, tag="a4")
                for m in range(KE):
                    pq = ps.tile([128, C], F32, tag="mm")
                    for k in range(KE):
                        mm(pq, wqt[:, k, m * 128:(m + 1) * 128], xq[:, k, :],
                           start=(k == 0), stop=(k == KE - 1))
                    ee = pc.tile([128, C], BF16, tag="a1")
                    rr = pc.tile([128, C], BF16, tag="a1")
                    sc.activation(ee, pq, AF.Exp, bias=ubq_col[:, m:m + 1])
                    sc.activation(rr, pq, AF.Relu, bias=ubq_col[:, m:m + 1])
                    v.scalar_tensor_tensor(out=qf[:, m, :], in0=ee, scalar=1.0,
                                           in1=rr, op0=ALU.min, op1=ALU.add)
                yield "b1"
                pd = psr.tile([8, C], F32, tag="row")
                for k in range(KE):
                    mm(pd, kmm[:, k, :], qf[:, k, :], start=(k == 0),
                       stop=(k == KE - 1))
                rec = pc.tile([8, C], BF16, tag="a1")
                v.reciprocal(out=rec, in_=pd)
                att = pa.tile([128, KE, C], BF16, tag="a4")
                for m in range(KE):
                    pn = ps.tile([128, C], F32, tag="mm")
                    mm(pn, bd[:, m, :], qf[:, m, :], start=True, stop=True)
                    pr = ps.tile([128, C], F32, tag="mm")
                    mm(pr, cmask_t[:, m, :], rec, start=True, stop=True)
                    rb = pc.tile([128, C], BF16, tag="a1")
                    sc.activation(rb, pr, AF.Copy)
                    v.tensor_tensor(out=att[:, m, :], in0=pn, in1=rb,
                                    op=ALU.mult)
                yield "b2"
                for m in range(KE):
                    pos = ps.tile([128, C], F32, tag="mm")
                    for k in range(KE):
                        mm(pos, owt[:, k, m * 128:(m + 1) * 128],
                           att[:, k, :], start=(k == 0), stop=(k == KE - 1))
                    tail_m(ci, m, pos, xq, ob_col)
                tail_post(ci, xq)
                yield "b3"

        # ---- tails ----
        def make_self_tail(l, s, dst):
            g_col = col_tile(lng[l, 0 if s == "b" else 1], KE, tag="lncol", bufs=16)
            b_col = col_tile(lnb[l, 0 if s == "b" else 1], KE, tag="lncol", bufs=16)
            rt_box = [None]

            def tail_m(ci, m, pos, xq, ob_col):
                if m == 0:
                    rt_box[0] = pa.tile([128, KE, C], BF16, tag="a4",
                                        name="rt")
                v.scalar_tensor_tensor(out=rt_box[0][:, m, :], in0=pos,
                                       scalar=ob_col[:, m:m + 1],
                                       in1=xq[:, m, :],
                                       op0=ALU.add, op1=ALU.add)

            def tail_post(ci, xq):
                rt = rt_box[0]
                outt = pa.tile([128, KE, C], BF16, tag="a4")
                ln_apply(rt, g_col, b_col, outt)
                store_chunk(dst, ci, outt)

            return tail_m, tail_post

        def make_cross_tail(l, s, dst):
            gw1t = wsm.tile([128, 2 * KE, E4], BF16, tag="ws")
            nc.sync.dma_start(out=gw1t,
                              in_=gw1[l].rearrange("(k p) g -> p k g", p=128))
            gwd_col = wcol.tile([128, 1], BF16, tag="gwd")
            nc.sync.dma_start(out=gwd_col, in_=gwd[l][:, None])
            gb1_col = col_tile(gb1[l], 1, tag="lncol", bufs=16)
            gb2d_t = pat.tile([1, 1], F32, tag="gb2d")
            nc.sync.dma_start(out=gb2d_t, in_=gb2d[l][None, :])
            g_col = col_tile(lng[l, 2], KE, tag="lncol", bufs=16)
            b_col = col_tile(lnb[l, 2], KE, tag="lncol", bufs=16)
            proj_box = [None]

            def tail_m(ci, m, pos, xq, ob_col):
                if m == 0:
                    proj_box[0] = pa.tile([128, KE, C], BF16, tag="a4",
                                          name="proj")
                sc.activation(proj_box[0][:, m, :], pos, AF.Identity,
                              bias=ob_col[:, m:m + 1])

            def tail_post(ci, xq):
                proj = proj_box[0]
                pg = ps.tile([128, C], F32, tag="mm")
                for k in range(2 * KE):
                    rhs = xq[:, k, :] if k < KE else proj[:, k - KE, :]
                    mm(pg, gw1t[:, k, :], rhs, start=(k == 0),
                       stop=(k == 2 * KE - 1))
                g1 = pc.tile([128, C], BF16, tag="a1")
                sc.activation(g1, pg, AF.Relu, bias=gb1_col[:, 0:1])
                g1t = pc.tile([128, C], BF16, tag="a1")
                v.tensor_scalar_min(g1t, g1, 6.0)
                pg2 = psr.tile([1, C], F32, tag="row")
                mm(pg2, gwd_col, g1t, start=True, stop=True)
                bg = pc.tile([1, C], BF16, tag="rowh", bufs=8)
                sc.activation(bg, pg2, AF.Sigmoid, bias=gb2d_t[0:1, 0:1])
                pbg = ps.tile([128, C], F32, tag="mm")
                mm(pbg, ONES_ROW, bg, start=True, stop=True)
                bgb = pc.tile([128, C], BF16, tag="a1")
                sc.activation(bgb, pbg, AF.Copy)
                mt = pa.tile([128, KE, C], BF16, tag="a4")
                for m in range(KE):
                    dtmp = pc.tile([128, C], BF16, tag="a1")
                    v.tensor_tensor(out=dtmp, in0=xq[:, m, :],
                                    in1=proj[:, m, :], op=ALU.subtract)
                    v.tensor_tensor(out=dtmp, in0=dtmp, in1=bgb, op=ALU.mult)
                    v.tensor_tensor(out=mt[:, m, :], in0=dtmp,
                                    in1=proj[:, m, :], op=ALU.add)
                outt = pa.tile([128, KE, C], BF16, tag="a4")
                ln_apply(mt, g_col, b_col, outt)
                store_chunk(dst, ci, outt)

            return tail_m, tail_post

        # ---- FFN ----
        def ffn_gen(l, s, src, dst):
            si = 0 if s == "b" else 1
            w1t = wbig.tile([128, KE, X], BF16, tag="w2x", bufs=4)
            nc.sync.dma_start(
                out=w1t, in_=w1[l, si].rearrange("(k p) x -> p k x", p=128))
            w2t = wbig.tile([128, KX, E], BF16, tag="w2x", bufs=4)
            nc.sync.dma_start(
                out=w2t, in_=w2[l, si].rearrange("(k p) e -> p k e", p=128))
            b1_col = col_tile(b1f[l, si], KX, tag="ffcol", bufs=16)
            b2_col = col_tile(b2f[l, si], KE, tag="ffcol", bufs=16)
            w0_col = col_tile(cwf[l, si, 0], KX, tag="ffcol", bufs=16)
            w1c_col = col_tile(cwf[l, si, 1], KX, tag="ffcol", bufs=16)
            w2_col = col_tile(cwf[l, si, 2], KX, tag="ffcol", bufs=16)
            A_col = col_tile(bnA[l, si], KX, tag="ffcol", bufs=16)
            B_col = col_tile(bnB[l, si], KX, tag="ffcol", bufs=16)
            g_col = col_tile(lng[l, 3 if s == "b" else 4], KE, tag="lncol", bufs=16)
            bb_col = col_tile(lnb[l, 3 if s == "b" else 4], KE, tag="lncol", bufs=16)
            yield
            PHASES.append((f"ffn{l}.{s}", len(nc.inst_map)))

            hts = [None] * NC
            xts = [None] * NC
            hl0 = [None] * NC
            hf2 = [None] * NC

            def compute_h(ci):
                xt = load_x_chunk(src, ci)
                xts[ci] = xt
                ht = pb.tile([128, KX, C], BF16, tag="a8")
                for m in range(KX):
                    ph = ps.tile([128, C], F32, tag="mm")
                    for k in range(KE):
                        mm(ph, w1t[:, k, m * 128:(m + 1) * 128], xt[:, k, :],
                           start=(k == 0), stop=(k == KE - 1))
                    hf = pc.tile([128, C], BF16, tag="a1")
                    sc.activation(hf, ph, AF.Relu, bias=b1_col[:, m:m + 1])
                    v.tensor_scalar_min(ht[:, m, :], hf, 6.0)
                hts[ci] = ht
                l0 = phl.tile([128, KX, 1], BF16, tag="hl")
                f2 = phl.tile([128, KX, 1], BF16, tag="hf")
                for m in range(KX):
                    v.tensor_scalar_mul(l0[:, m, :], ht[:, m, C - 1:C],
                                        w0_col[:, m:m + 1])
                    v.tensor_scalar_mul(f2[:, m, :], ht[:, m, 0:1],
                                        w2_col[:, m:m + 1])
                hl0[ci], hf2[ci] = l0, f2

            def conv_elem(ci):
                ht = hts[ci]
                h2 = pb.tile([128, KX, C], BF16, tag="a8")
                for m in range(KX):
                    acc = pc.tile([128, C], BF16, tag="a1")
                    v.tensor_scalar_mul(acc, ht[:, m, :], w1c_col[:, m:m + 1])
                    v.scalar_tensor_tensor(out=acc[:, 1:C],
                                           in0=ht[:, m, 0:C - 1],
                                           scalar=w0_col[:, m:m + 1],
                                           in1=acc[:, 1:C],
                                           op0=ALU.mult, op1=ALU.add)
                    if ci > 0:
                        v.tensor_tensor(out=acc[:, 0:1], in0=acc[:, 0:1],
                                        in1=hl0[ci - 1][:, m, :], op=ALU.add)
                    v.scalar_tensor_tensor(out=acc[:, 0:C - 1],
                                           in0=ht[:, m, 1:C],
                                           scalar=w2_col[:, m:m + 1],
                                           in1=acc[:, 0:C - 1],
                                           op0=ALU.mult, op1=ALU.add)
                    if ci < NC - 1:
                        v.tensor_tensor(out=acc[:, C - 1:C],
                                        in0=acc[:, C - 1:C],
                                        in1=hf2[ci + 1][:, m, :], op=ALU.add)
                    a2 = pc.tile([128, C], BF16, tag="a1")
                    sc.activation(a2, acc, AF.Relu, scale=A_col[:, m:m + 1],
                                  bias=B_col[:, m:m + 1])
                    v.tensor_scalar_min(h2[:, m, :], a2, 6.0)
                return h2

            def conv_pw(ci, h2):
                rt = pa.tile([128, KE, C], BF16, tag="a4")
                for m in range(KE):
                    pw = ps.tile([128, C], F32, tag="mm")
                    for k in range(KX):
                        mm(pw, w2t[:, k, m * 128:(m + 1) * 128], h2[:, k, :],
                           start=(k == 0), stop=(k == KX - 1))
                    v.scalar_tensor_tensor(out=rt[:, m, :], in0=pw,
                                           scalar=b2_col[:, m:m + 1],
                                           in1=xts[ci][:, m, :],
                                           op0=ALU.add, op1=ALU.add)
                outt = pa.tile([128, KE, C], BF16, tag="a4")
                ln_apply(rt, g_col, bb_col, outt)
                store_chunk(dst, ci, outt)
                hts[ci] = xts[ci] = None

            compute_h(0)
            yield "h"
            compute_h(1)
            yield "h"
            for ci in range(NC - 1):
                h2 = conv_elem(ci)
                yield "cv"
                conv_pw(ci, h2)
                yield "pw"
                if ci + 2 < NC:
                    compute_h(ci + 2)
                    yield "h"
            h2 = conv_elem(NC - 1)
            yield "cv"
            conv_pw(NC - 1, h2)
            yield "pw"

        # ---- layers (chunk-interleaved across independent streams) ----
        def adv(g, n=1):
            for _ in range(n):
                next(g, None)

        for l in range(L):
            bsrc = rs["b", 0] if l == 0 else rs["b", (l - 1, 3)]
            lsrc = rs["l", 0] if l == 0 else rs["l", (l - 1, 3)]
            g0 = attn_gen(l, 0, bsrc, bsrc,
                          *make_self_tail(l, "b", rs["b", (l, 1)]))
            g1 = attn_gen(l, 1, lsrc, lsrc,
                          *make_self_tail(l, "l", rs["l", (l, 1)]))
            g2 = attn_gen(l, 2, rs["b", (l, 1)], rs["l", (l, 1)],
                          *make_cross_tail(l, "b", rs["b", (l, 2)]))
            g3 = attn_gen(l, 3, rs["l", (l, 1)], rs["b", (l, 1)],
                          *make_cross_tail(l, "l", rs["l", (l, 2)]))
            gb = ffn_gen(l, "b", rs["b", (l, 2)], rs["b", (l, 3)])
            gl = ffn_gen(l, "l", rs["l", (l, 2)], rs["l", (l, 3)])
            adv(g0)                     # weights
            adv(g1)
            for _ in range(2 * NC):     # self alphas: a1/a2 staged
                adv(g0)
                adv(g1)
            adv(g0)                     # bd/kmm
            adv(g1)
            adv(g2)                     # prefetch cross weights
            adv(g3)
            for _ in range(3 * NC):     # self betas: b1/b2/b3 staged
                adv(g0)
                adv(g1)
            for _ in range(2 * NC):     # cross alphas staged
                adv(g2)
                adv(g3)
            adv(g2)                     # bd/kmm
            adv(g3)
            adv(gb)                     # prefetch ffn weights
            adv(gl)
            for _ in range(3 * NC):     # cross betas staged
                adv(g2)
                adv(g3)
            for _ in range(12):         # ffn h/cv/pw staged
                adv(gb)
                adv(gl)

        PHASES.append(("final", len(nc.inst_map)))
        # ---- final head ----
        fw1t = wbig.tile([128, 2 * KE, E2], BF16, tag="w", bufs=6)
        nc.sync.dma_start(out=fw1t,
                          in_=fw1.rearrange("(k p) g -> p k g", p=128))
        fw2t = wsm.tile([128, 2, E], BF16, tag="wfin", bufs=4)
        nc.sync.dma_start(out=fw2t,
                          in_=fw2.rearrange("(k p) e -> p k e", p=128))
        rw1t = wsm.tile([128, KE, E4], BF16, tag="wfin", bufs=4)
        nc.sync.dma_start(out=rw1t,
                          in_=rw1.rearrange("(k p) g -> p k g", p=128))
        rw2t = wsm.tile([128, E8], BF16, tag="wfin", bufs=4)
        nc.sync.dma_start(out=rw2t, in_=rw2)
        rw3t = wsm.tile([E8, 16], BF16, tag="wfin", bufs=4)
        nc.sync.dma_start(out=rw3t, in_=rw3p)
        rb3_row = row_tile(rb3p, 16)
        fb2_col = col_tile(fb2f, KE, tag="fcol")
        fb1_col = col_tile(fb1, 2, tag="fcol")
        flng_col = col_tile(flng, KE, tag="fcol")
        flnb_col = col_tile(flnb, KE, tag="fcol")
        rb1_col = col_tile(rb1, 1, tag="fcol")
        rb2_col = wcol.tile([E8, 1], F32, tag="fcol")
        nc.sync.dma_start(out=rb2_col, in_=rb2[:, None])
        out_ap = out_dram.ap()

        bsrc, lsrc = rs["b", (L - 1, 3)], rs["l", (L - 1, 3)]
        for ci in range(NC):
            xb = load_x_chunk(bsrc, ci)
            xl = load_x_chunk(lsrc, ci)
            f1t = pa.tile([128, 2, C], BF16, tag="a4")
            for m in range(2):
                pf = ps.tile([128, C], F32, tag="mm")
                for k in range(2 * KE):
                    rhs = xb[:, k, :] if k < KE else xl[:, k - KE, :]
                    mm(pf, fw1t[:, k, m * 128:(m + 1) * 128], rhs,
                       start=(k == 0), stop=(k == 2 * KE - 1))
                f1 = pc.tile([128, C], BF16, tag="a1")
                sc.activation(f1, pf, AF.Relu, bias=fb1_col[:, m:m + 1])
                v.tensor_scalar_min(f1t[:, m, :], f1, 6.0)
            ft = pa.tile([128, KE, C], BF16, tag="a4")
            for m in range(KE):
                pf2 = ps.tile([128, C], F32, tag="mm")
                for k in range(2):
                    mm(pf2, fw2t[:, k, m * 128:(m + 1) * 128], f1t[:, k, :],
                       start=(k == 0), stop=(k == 1))
                sc.activation(ft[:, m, :], pf2, AF.Identity,
                              bias=fb2_col[:, m:m + 1])
            frt = pa.tile([128, KE, C], BF16, tag="a4")
            ln_apply(ft, flng_col, flnb_col, frt)
            # relu after LN
            frf = frt.rearrange("p k c -> p (k c)")
            v.tensor_scalar_max(frf, frf, 0.0)
            p1 = ps.tile([128, C], F32, tag="mm")
            for k in range(KE):
                mm(p1, rw1t[:, k, :], frt[:, k, :], start=(k == 0),
                   stop=(k == KE - 1))
            h1f = pc.tile([128, C], BF16, tag="a1")
            sc.activation(h1f, p1, AF.Relu, bias=rb1_col[:, 0:1])
            h1t = pc.tile([128, C], BF16, tag="a1")
            v.tensor_scalar_min(h1t, h1f, 6.0)
            p2 = ps.tile([E8, C], F32, tag="mm")
            mm(p2, rw2t, h1t, start=True, stop=True)
            h2f = pc.tile([E8, C], BF16, tag="a1")
            sc.activation(h2f, p2, AF.Relu, bias=rb2_col[:, 0:1])
            h2t = pc.tile([E8, C], BF16, tag="a1")
            v.tensor_scalar_min(h2t, h2f, 6.0)
            ot = pc.tile([128, NTT, c.OUT], F32, tag="a1")
            for tt in range(NTT):
                p3 = ps.tile([128, 16], F32, tag="mm")
                mm(p3, h2t[:, tt * 128:(tt + 1) * 128], rw3t,
                   start=True, stop=False)
                mm(p3, ONES_ROW, rb3_row, start=False, stop=True)
                sc.activation(ot[:, tt, :], p3[:, 0:c.OUT], AF.Copy)
            nc.sync.dma_start(
                out=out_ap[ci * C:(ci + 1) * C, :].rearrange(
                    "(tt p) o -> p tt o", p=128),
                in_=ot)

    return din, out_dram


# ======================================================================
# kernel() entry point: full inputs in, full outputs out (8-core SPMD).
# ======================================================================
import concourse.bacc as _bacc
from concourse.bass_utils import run_bass_kernel_spmd as _run_spmd

_N_CORES = 8
_CACHE = {}


def _get_nc():
    if "nc" not in _CACHE:
        nc = _bacc.Bacc("TRN2", target_bir_lowering=False, debug=False)
        build(nc, Cfg())
        nc.finalize()
        _CACHE["nc"] = nc
    return _CACHE["nc"]


def _bf16(x):
    import ml_dtypes
    return np.asarray(x, dtype=np.float32).astype(ml_dtypes.bfloat16)


def host_prep(inputs):
    """Host-side weight preprocessing: compose QKV, fold BN, convert bf16."""
    c = Cfg()
    E, X, H, L = c.E, c.X, c.H, c.L
    E4, E2, E8 = E // 4, E // 2, E // 8
    f = {k: np.asarray(v, dtype=np.float32) for k, v in inputs.items()}
    dw, uw, ub = f["dw"], f["uw"], f["ub"]
    # composed q and k|v projection weights
    wq = np.matmul(dw[:, :, 0], uw[:, :, 0])          # (L,4,E,E)
    wk = np.matmul(dw[:, :, 1], uw[:, :, 1])
    wv = np.matmul(dw[:, :, 2], uw[:, :, 2])
    wkv = np.concatenate([wk, wv], axis=-1)           # (L,4,E,2E)
    ubq = ub[:, :, 0]                                 # (L,4,E)
    ubkv = np.concatenate([ub[:, :, 1], ub[:, :, 2]], axis=-1)
    rsq = np.float32(1.0 / np.sqrt(1.0 + BN_EPS))
    A = f["bng"] * rsq                                # (L,2,X)
    B = f["cb"] * A + f["bnb"]
    cwf = f["cw"].transpose(0, 1, 3, 2).copy()        # (L,2,3,X)
    gwd = f["gw2"][:, :, 0] - f["gw2"][:, :, 1]       # (L,E4)
    gb2d = (f["gb2"][:, 0] - f["gb2"][:, 1])[:, None]  # (L,1)
    rw3p = np.zeros((E8, 16), np.float32)
    rw3p[:, :c.OUT] = f["rw3"]
    rb3p = np.zeros((16,), np.float32)
    rb3p[:c.OUT] = f["rb3"]
    dh = E // H
    ident = np.eye(128, dtype=np.float32)
    ones = np.ones((128, 128), dtype=np.float32)
    hmask = np.zeros((E, H), dtype=np.float32)
    for ff in range(E):
        hmask[ff, ff // dh] = 1.0
    cmask = hmask.T.copy()

    b16 = dict(wq=wq, wkv=wkv, ubkv=ubkv, ow=f["ow"],
               w1=f["w1"], w2=f["w2"],
               gw1=f["gw1"], gwd=gwd, fw1=f["fw1"], fw2=f["fw2"],
               rw1=f["rw1"], rw2=f["rw2"], rw3p=rw3p,
               rb3p=rb3p, ident=ident, ones128=ones, hmask=hmask,
               cmask=cmask)
    f32 = dict(ubq=ubq, ob=f["ob"], b1=f["b1"], b2=f["b2"], fb2=f["fb2"],
               cwf=cwf, bnA=A, bnB=B,
               lng=f["lng"], lnb=f["lnb"],
               gb1=f["gb1"], gb2d=gb2d, fb1=f["fb1"], flng=f["flng"],
               flnb=f["flnb"], rb1=f["rb1"], rb2=f["rb2"])
    shared = {k: _bf16(v) for k, v in b16.items()}
    shared.update({k: np.ascontiguousarray(v, dtype=np.float32)
                   for k, v in f32.items()})
    return shared, f["body_feats"], f["limb_feats"]


def kernel(**inputs):
    nc = _get_nc()
    shared, body, limb = host_prep(inputs)
    in_maps = []
    for i in range(_N_CORES):
        m = dict(shared)
        m["body_feats"] = _bf16(body[i])
        m["limb_feats"] = _bf16(limb[i])
        in_maps.append(m)
    res = run_kernel_spmd_cached(nc, in_maps)
    out = np.stack([res[i]["out"] for i in range(_N_CORES)], axis=0)
    return out.astype(np.float32)


def run_kernel_spmd_cached(nc, in_maps, **kw):
    r = _run_spmd(nc, in_maps, list(range(_N_CORES)), **kw)
    _CACHE["last_result"] = r
    return r.results


# revision 5
# speedup vs baseline: 1.1508x; 1.0034x over previous
"""Dual-stream linear-attention transformer — bf16 redesign (per-core).

Layout convention (same as baseline):
  - "layout 1" activation: [E, N] feature-major; SBUF tiles [128, KE, C]
    (feature f = 128*k + p -> partition p, k-th slice; tokens on free dim).
  - alpha k/v are produced token-major per 128-token tile [128, E].
  - Residual streams live in internal DRAM as [E, N] bf16.

Key changes vs baseline:
  - All matmul operands + SBUF activations bf16 (same PE rate as f32r>=256,
    but DVE tensor_tensor 2x / tensor_scalar 4x, half DMA bytes).
  - QKV down+up projections composed into single E x E / E x 2E weights on
    the host (removes the low-rank intermediate copies).
  - Biases folded into matmuls as rank-1 accumulates (ones_row x bias_row).
  - elu+1 via 3 ops: ACT Exp, ACT Relu, DVE scalar_tensor_tensor(min,add).
  - LayerNorm apply via 3 bf16 DVE ops per slice (TT,TT,TS) instead of
    gpsimd tensor_tensor pairs.
  - FFN dwconv via TS/STT chain on DVE; BN folded on host into A,B.
  - relu6 of FFN h on gpsimd (idle engine) straight from PSUM.
  - m-outer matmul groups -> 1 PSUM bank live per group, fewer stalls.
"""

from dataclasses import dataclass
from contextlib import ExitStack

import numpy as np

import concourse.bass as bass
import concourse.mybir as mybir
import concourse.tile as tile

F32 = mybir.dt.float32
BF16 = mybir.dt.bfloat16
AF = mybir.ActivationFunctionType
ALU = mybir.AluOpType

LN_EPS = 1e-5
BN_EPS = 1e-5


@dataclass
class Cfg:
    N: int = 2048
    E: int = 512
    R: int = 256
    X: int = 1024
    H: int = 8
    L: int = 3
    OUT: int = 15
    C: int = 512

    @property
    def KE(self):
        return self.E // 128

    @property
    def KX(self):
        return self.X // 128

    @property
    def NC(self):
        return self.N // self.C

    @property
    def NTT(self):
        return self.C // 128


PHASES = []


def build(nc, cfg):
    c = cfg
    E, X, H, N, C, L = c.E, c.X, c.H, c.N, c.C, c.L
    KE, KX, NC, NTT = c.KE, c.KX, c.NC, c.NTT
    E4, E2, E8 = E // 4, E // 2, E // 8

    din = {}

    def inp(name, shape, dt=BF16):
        din[name] = nc.dram_tensor(name, list(shape), dt, kind="ExternalInput")
        return din[name].ap()

    # activations (host converts to bf16)
    body_feats = inp("body_feats", (N, E))
    limb_feats = inp("limb_feats", (N, E))
    # attention weights (host-composed)
    wq = inp("wq", (L, 4, E, E))
    wkv = inp("wkv", (L, 4, E, 2 * E))
    ubq = inp("ubq", (L, 4, E), F32)
    ubkv = inp("ubkv", (L, 4, 2 * E))
    ow = inp("ow", (L, 4, E, E))
    obf = inp("ob", (L, 4, E), F32)
    # FFN
    w1 = inp("w1", (L, 2, E, X))
    b1f = inp("b1", (L, 2, X), F32)
    cwf = inp("cwf", (L, 2, 3, X), F32)     # conv taps, tap-major
    bnA = inp("bnA", (L, 2, X), F32)        # bng*rsqrt(1+eps)
    bnB = inp("bnB", (L, 2, X), F32)        # cb*A + bnb
    w2 = inp("w2", (L, 2, X, E))
    b2f = inp("b2", (L, 2, E), F32)
    lng = inp("lng", (L, 5, E), F32)
    lnb = inp("lnb", (L, 5, E), F32)
    # gating
    gw1 = inp("gw1", (L, 2 * E, E4))
    gb1 = inp("gb1", (L, E4), F32)
    gwd = inp("gwd", (L, E4))               # gw2[:,0]-gw2[:,1]
    gb2d = inp("gb2d", (L, 1), F32)         # gb2[0]-gb2[1]
    # final head
    fw1 = inp("fw1", (2 * E, E2))
    fb1 = inp("fb1", (E2,), F32)
    fw2 = inp("fw2", (E2, E))
    fb2f = inp("fb2", (E,), F32)
    flng = inp("flng", (E,), F32)
    flnb = inp("flnb", (E,), F32)
    rw1 = inp("rw1", (E, E4))
    rb1 = inp("rb1", (E4,), F32)
    rw2 = inp("rw2", (E4, E8))
    rb2 = inp("rb2", (E8,), F32)
    rw3p = inp("rw3p", (E8, 16))            # zero-padded to 16
    rb3p = inp("rb3p", (16,))               # zero-padded
    ident_in = inp("ident", (128, 128))
    ones_in = inp("ones128", (128, 128))
    hmask_in = inp("hmask", (E, H))
    cmask_in = inp("cmask", (H, E))

    out_dram = nc.dram_tensor("out", [N, c.OUT], F32, kind="ExternalOutput")

    def idram(name):
        return nc.dram_tensor(name, [E, N], BF16).ap().rearrange(
            "(k p) n -> p k n", p=128)

    rs = {}
    for s in ("b", "l"):
        rs[s, 0] = idram(f"r{s}0")
        for l in range(L):
            for st in (1, 2, 3):
                rs[s, (l, st)] = idram(f"r{s}_{l}_{st}")

    lowp = nc.allow_low_precision("bf16 activations within rel-err budget")

    with tile.TileContext(nc) as tc, ExitStack() as ctx, lowp:
        p_ = ctx.enter_context
        cst = p_(tc.tile_pool(name="cst", bufs=1))
        wbig = p_(tc.tile_pool(name="wbig", bufs=3))
        wsm = p_(tc.tile_pool(name="wsm", bufs=2))
        wcol = p_(tc.tile_pool(name="wcol", bufs=10))
        wrow = p_(tc.tile_pool(name="wrow", bufs=6))
        pa = p_(tc.tile_pool(name="pa", bufs=9))      # 4KB bf16 chunk tiles
        pb = p_(tc.tile_pool(name="pb", bufs=5))      # 8KB ht tiles
        pc = p_(tc.tile_pool(name="pc", bufs=12))      # 1KB bf16 / rows
        pat = p_(tc.tile_pool(name="pat", bufs=3))    # per-attn persistents
        phl = p_(tc.tile_pool(name="phl", bufs=6))    # conv halos
        ps = p_(tc.tile_pool(name="ps", bufs=5, space="PSUM"))
        psr = p_(tc.tile_pool(name="psr", bufs=3, space="PSUM"))

        v, sc, gp = nc.vector, nc.scalar, nc.gpsimd

        def mm(out, lhsT, rhs, start, stop):
            nc.tensor.matmul(out, lhsT, rhs, start=start, stop=stop)

        # ---- constants ----
        ident_t = cst.tile([128, 128], BF16, tag="ident")
        nc.sync.dma_start(out=ident_t, in_=ident_in)
        ones_t = cst.tile([128, 128], BF16, tag="ones")
        nc.sync.dma_start(out=ones_t, in_=ones_in)
        hmask_t = cst.tile([128, KE, H], BF16, tag="hmask")
        nc.sync.dma_start(out=hmask_t,
                          in_=hmask_in.rearrange("(k p) h -> p k h", p=128))
        cmask_t = cst.tile([H, KE, 128], BF16, tag="cmask")
        nc.sync.dma_start(out=cmask_t,
                          in_=cmask_in.rearrange("h (k p) -> h k p", p=128))
        ONES_COL = ones_t[:, 0:1]
        ONES_ROW = ones_t[0:1, :]
        onesc_t = cst.tile([1, C], BF16, tag="onesc")
        v.memset(onesc_t, 1.0)
        ONES_C = onesc_t[0:1, :]
        eps_ln = cst.tile([1, 1], F32, tag="epsl")
        v.memset(eps_ln, LN_EPS)

        def col_tile(src_ap, m, tag="col", bufs=None):
            t = wcol.tile([128, m], F32, tag=tag,
                          bufs=(12 if tag == "col" else bufs))
            nc.sync.dma_start(out=t, in_=src_ap.rearrange("(m p) -> p m", p=128))
            return t

        def row_tile(src_ap, n, tag="row", pool=None):
            t = (pool or wrow).tile([1, n], BF16, tag=tag)
            nc.sync.dma_start(out=t, in_=src_ap[None, :])
            return t

        def ln_apply(xs, g_col, b_col, outt, extra_tt=None):
            """LayerNorm over features (layout 1). xs: [128, KE, C] bf16 tile.
            outt: [128, KE, C] bf16 out. extra_tt(m): None."""
            sq = pa.tile([128, KE, C], BF16, tag="a4")
            xf = xs.rearrange("p k c -> p (k c)")
            v.tensor_tensor(out=sq.rearrange("p k c -> p (k c)"),
                            in0=xf, in1=xf, op=ALU.mult)
            ps_s = psr.tile([1, C], F32, tag="row")
            ps_ss = psr.tile([1, C], F32, tag="row")
            for m in range(KE):
                mm(ps_s, ONES_COL, xs[:, m, :], start=(m == 0),
                   stop=(m == KE - 1))
                mm(ps_ss, ONES_COL, sq[:, m, :], start=(m == 0),
                   stop=(m == KE - 1))
            msq = pc.tile([1, C], F32, tag="row", bufs=8)
            sc.activation(msq, ps_ss, AF.Copy, scale=1.0 / E)
            m2 = pc.tile([1, C], F32, tag="row", bufs=8)
            sc.activation(m2, ps_s, AF.Square, scale=1.0 / E)
            var = pc.tile([1, C], F32, tag="row", bufs=8)
            v.tensor_tensor(out=var, in0=msq, in1=m2, op=ALU.subtract)
            srow = pc.tile([1, C], BF16, tag="rowh", bufs=8)
            sc.activation(srow, var, AF.Abs_reciprocal_sqrt,
                          bias=eps_ln[0:1, 0:1])
            trow = pc.tile([1, C], BF16, tag="rowh", bufs=8)
            v.scalar_tensor_tensor(out=trow, in0=ps_s, scalar=1.0 / E,
                                   in1=srow, op0=ALU.mult, op1=ALU.mult)
            sb_s = pc.tile([128, C], BF16, tag="a1")
            gp.partition_broadcast(sb_s, srow)
            sb_t = pc.tile([128, C], BF16, tag="a1")
            gp.partition_broadcast(sb_t, trow)
            for m in range(KE):
                u = pc.tile([128, C], BF16, tag="a1")
                v.tensor_tensor(out=u, in0=xs[:, m, :], in1=sb_s, op=ALU.mult)
                v.tensor_tensor(out=u, in0=u, in1=sb_t, op=ALU.subtract)
                v.tensor_scalar(out=outt[:, m, :], in0=u,
                                scalar1=g_col[:, m:m + 1],
                                scalar2=b_col[:, m:m + 1],
                                op0=ALU.mult, op1=ALU.add)

        def load_x_chunk(dram_l1, ci, tag="a4"):
            xt = pa.tile([128, KE, C], BF16, tag=tag)
            nc.sync.dma_start(out=xt, in_=dram_l1[:, :, ci * C:(ci + 1) * C])
            return xt

        def store_chunk(dram_l1, ci, t):
            gp.dma_start(out=dram_l1[:, :, ci * C:(ci + 1) * C], in_=t)

        # ---- entry transpose ----
        def entry(x_ap, dst):
            for ttk in range(N // 128):
                x2 = pa.tile([128, E], BF16, tag="a4")
                nc.sync.dma_start(out=x2, in_=x_ap[ttk * 128:(ttk + 1) * 128, :])
                xt = pa.tile([128, KE, 128], BF16, tag="a4")
                for f in range(KE):
                    pt = ps.tile([128, 128], BF16, tag="mm")
                    nc.tensor.transpose(pt, x2[:, f * 128:(f + 1) * 128],
                                        ident_t)
                    sc.activation(xt[:, f, :], pt, AF.Copy)
                nc.sync.dma_start(out=dst[:, :, ttk * 128:(ttk + 1) * 128],
                                  in_=xt)

        PHASES.append(("entry", len(nc.inst_map)))
        entry(body_feats, rs["b", 0])
        entry(limb_feats, rs["l", 0])

        # ---- linear attention ----
        def attn_gen(l, a, xq_dram, xkv_dram, tail_m, tail_post):
            """Generator: yields after weight loads, after each alpha chunk
            (bd/kmm ride with the last), and after each beta chunk."""
            wqt = wbig.tile([128, KE, E], BF16, tag="w", bufs=6)
            nc.sync.dma_start(
                out=wqt, in_=wq[l, a].rearrange("(k p) e -> p k e", p=128))
            wkvt = wbig.tile([128, KE, 2 * E], BF16, tag="w2x", bufs=4)
            nc.sync.dma_start(
                out=wkvt, in_=wkv[l, a].rearrange("(k p) e -> p k e", p=128))
            owt = wbig.tile([128, KE, E], BF16, tag="w", bufs=6)
            nc.sync.dma_start(
                out=owt, in_=ow[l, a].rearrange("(k p) e -> p k e", p=128))
            ubq_col = col_tile(ubq[l, a], KE)
            ubkv_row = row_tile(ubkv[l, a], 2 * E)
            ob_col = col_tile(obf[l, a], KE)
            yield

            PHASES.append((f"attn{l}.{a}.alpha", len(nc.inst_map)))
            kv_acc = pat.tile([128, KE, 258], F32, tag="kva", bufs=2)

            for ci in range(NC):
                xt = load_x_chunk(xkv_dram, ci)
                k2f = pa.tile([128, NTT, E], BF16, tag="a4")
                v2x = pa.tile([128, NTT, 2, 258], BF16, tag="a4")
                v.memset(v2x[:, :, :, 256:258], 1.0)
                for tt in range(NTT):
                    xs = xt[:, :, tt * 128:(tt + 1) * 128]
                    pk = ps.tile([128, E], F32, tag="mm")
                    pv = ps.tile([128, E], F32, tag="mm")
                    for k in range(KE):
                        mm(pk, xs[:, k, :], wkvt[:, k, 0:E],
                           start=(k == 0), stop=False)
                        mm(pv, xs[:, k, :], wkvt[:, k, E:2 * E],
                           start=(k == 0), stop=False)
                    mm(pk, ONES_ROW, ubkv_row[:, 0:E], start=False, stop=True)
                    mm(pv, ONES_ROW, ubkv_row[:, E:2 * E], start=False,
                       stop=True)
                    ee = pc.tile([128, E], BF16, tag="a1")
                    rr = pc.tile([128, E], BF16, tag="a1")
                    sc.activation(ee, pk, AF.Exp)
                    sc.activation(rr, pk, AF.Relu)
                    v.scalar_tensor_tensor(out=k2f[:, tt, :], in0=ee,
                                           scalar=1.0, in1=rr,
                                           op0=ALU.min, op1=ALU.add)
                    v.tensor_copy(v2x[:, tt, 0, 0:256], pv[:, 0:256])
                    v.tensor_copy(v2x[:, tt, 1, 0:256], pv[:, 256:512])
                yield "a1"
                for p in range(4):
                    pkv = ps.tile([128, 258], F32, tag="mm")
                    for tt in range(NTT):
                        mm(pkv, k2f[:, tt, p * 128:(p + 1) * 128],
                           v2x[:, tt, p // 2, :],
                           start=(tt == 0), stop=(tt == NTT - 1))
                    if ci == 0:
                        sc.activation(kv_acc[:, p, :], pkv, AF.Copy)
                    else:
                        v.tensor_tensor(out=kv_acc[:, p, :],
                                        in0=kv_acc[:, p, :], in1=pkv,
                                        op=ALU.add)
                yield "a2"

            bd = pat.tile([128, KE, 128], BF16, tag="bd", bufs=2)
            v.memset(bd, 0.0)
            for p in range(4):
                h0c = (2 * p % 4) * 64
                h1c = ((2 * p + 1) % 4) * 64
                v.tensor_copy(bd[0:64, p, 0:64], kv_acc[0:64, p, h0c:h0c + 64])
                v.tensor_copy(bd[64:128, p, 64:128],
                              kv_acc[64:128, p, h1c:h1c + 64])
            kmm = pat.tile([128, KE, H], BF16, tag="km")
            for k in range(KE):
                v.tensor_scalar_mul(kmm[:, k, :], hmask_t[:, k, :],
                                    kv_acc[:, k, 256:257])
            yield "bd"

            PHASES.append((f"attn{l}.{a}.beta", len(nc.inst_map)))
            for ci in range(NC):
                xq = load_x_chunk(xq_dram, ci)
                qf = pa.tile([128, KE, C], BF16, tag="a4")
                for m in range(KE):
                    pq = ps.tile([128, C], F32, tag="mm")
                    for k in range(KE):
                        mm(pq, wqt[:, k, m * 128:(m + 1) * 128], xq[:, k, :],
                           start=(k == 0), stop=(k == KE - 1))
                    ee = pc.tile([128, C], BF16, tag="a1")
                    rr = pc.tile([128, C], BF16, tag="a1")
                    sc.activation(ee, pq, AF.Exp, bias=ubq_col[:, m:m + 1])
                    sc.activation(rr, pq, AF.Relu, bias=ubq_col[:, m:m + 1])
                    v.scalar_tensor_tensor(out=qf[:, m, :], in0=ee, scalar=1.0,
                                           in1=rr, op0=ALU.min, op1=ALU.add)
                yield "b1"
                pd = psr.tile([8, C], F32, tag="row")
                for k in range(KE):
                    mm(pd, kmm[:, k, :], qf[:, k, :], start=(k == 0),
                       stop=(k == KE - 1))
                rec = pc.tile([8, C], BF16, tag="a1")
                v.reciprocal(out=rec, in_=pd)
                att = pa.tile([128, KE, C], BF16, tag="a4")
                for m in range(KE):
                    pn = ps.tile([128, C], F32, tag="mm")
                    mm(pn, bd[:, m, :], qf[:, m, :], start=True, stop=True)
                    pr = ps.tile([128, C], F32, tag="mm")
                    mm(pr, cmask_t[:, m, :], rec, start=True, stop=True)
                    rb = pc.tile([128, C], BF16, tag="a1")
                    sc.activation(rb, pr, AF.Copy)
                    v.tensor_tensor(out=att[:, m, :], in0=pn, in1=rb,
                                    op=ALU.mult)
                yield "b2"
                for m in range(KE):
                    pos = ps.tile([128, C], F32, tag="mm")
                    for k in range(KE):
                        mm(pos, owt[:, k, m * 128:(m + 1) * 128],
                           att[:, k, :], start=(k == 0), stop=(k == KE - 1))
                    tail_m(ci, m, pos, xq, ob_col)
                tail_post(ci, xq)
                yield "b3"

        # ---- tails ----
        def make_self_tail(l, s, dst):
            g_col = col_tile(lng[l, 0 if s == "b" else 1], KE, tag="lncol", bufs=16)
            b_col = col_tile(lnb[l, 0 if s == "b" else 1], KE, tag="lncol", bufs=16)
            rt_box = [None]

            def tail_m(ci, m, pos, xq, ob_col):
                if m == 0:
                    rt_box[0] = pa.tile([128, KE, C], BF16, tag="a4",
                                        name="rt")
                v.scalar_tensor_tensor(out=rt_box[0][:, m, :], in0=pos,
                                       scalar=ob_col[:, m:m + 1],
                                       in1=xq[:, m, :],
                                       op0=ALU.add, op1=ALU.add)

            def tail_post(ci, xq):
                rt = rt_box[0]
                outt = pa.tile([128, KE, C], BF16, tag="a4")
                ln_apply(rt, g_col, b_col, outt)
                store_chunk(dst, ci, outt)

            return tail_m, tail_post

        def make_cross_tail(l, s, dst):
            gw1t = wsm.tile([128, 2 * KE, E4], BF16, tag="ws")
            nc.sync.dma_start(out=gw1t,
                              in_=gw1[l].rearrange("(k p) g -> p k g", p=128))
            gwd_col = wcol.tile([128, 1], BF16, tag="gwd")
            nc.sync.dma_start(out=gwd_col, in_=gwd[l][:, None])
            gb1_col = col_tile(gb1[l], 1, tag="lncol", bufs=16)
            gb2d_t = pat.tile([1, 1], F32, tag="gb2d")
            nc.sync.dma_start(out=gb2d_t, in_=gb2d[l][None, :])
            g_col = col_tile(lng[l, 2], KE, tag="lncol", bufs=16)
            b_col = col_tile(lnb[l, 2], KE, tag="lncol", bufs=16)
            proj_box = [None]

            def tail_m(ci, m, pos, xq, ob_col):
                if m == 0:
                    proj_box[0] = pa.tile([128, KE, C], BF16, tag="a4",
                                          name="proj")
                sc.activation(proj_box[0][:, m, :], pos, AF.Identity,
                              bias=ob_col[:, m:m + 1])

            def tail_post(ci, xq):
                proj = proj_box[0]
                pg = ps.tile([128, C], F32, tag="mm")
                for k in range(2 * KE):
                    rhs = xq[:, k, :] if k < KE else proj[:, k - KE, :]
                    mm(pg, gw1t[:, k, :], rhs, start=(k == 0),
                       stop=(k == 2 * KE - 1))
                g1 = pc.tile([128, C], BF16, tag="a1")
                sc.activation(g1, pg, AF.Relu, bias=gb1_col[:, 0:1])
                g1t = pc.tile([128, C], BF16, tag="a1")
                v.tensor_scalar_min(g1t, g1, 6.0)
                pg2 = psr.tile([1, C], F32, tag="row")
                mm(pg2, gwd_col, g1t, start=True, stop=True)
                bg = pc.tile([1, C], BF16, tag="rowh", bufs=8)
                sc.activation(bg, pg2, AF.Sigmoid, bias=gb2d_t[0:1, 0:1])
                bgb = pc.tile([128, C], BF16, tag="a1")
                gp.partition_broadcast(bgb, bg)
                mt = pa.tile([128, KE, C], BF16, tag="a4")
                for m in range(KE):
                    dtmp = pc.tile([128, C], BF16, tag="a1")
                    v.tensor_tensor(out=dtmp, in0=xq[:, m, :],
                                    in1=proj[:, m, :], op=ALU.subtract)
                    v.tensor_tensor(out=dtmp, in0=dtmp, in1=bgb, op=ALU.mult)
                    v.tensor_tensor(out=mt[:, m, :], in0=dtmp,
                                    in1=proj[:, m, :], op=ALU.add)
                outt = pa.tile([128, KE, C], BF16, tag="a4")
                ln_apply(mt, g_col, b_col, outt)
                store_chunk(dst, ci, outt)

            return tail_m, tail_post

        # ---- FFN ----
        def ffn_gen(l, s, src, dst):
            si = 0 if s == "b" else 1
            w1t = wbig.tile([128, KE, X], BF16, tag="w2x", bufs=4)
            nc.sync.dma_start(
                out=w1t, in_=w1[l, si].rearrange("(k p) x -> p k x", p=128))
            w2t = wbig.tile([128, KX, E], BF16, tag="w2x", bufs=4)
            nc.sync.dma_start(
                out=w2t, in_=w2[l, si].rearrange("(k p) e -> p k e", p=128))
            b1_col = col_tile(b1f[l, si], KX, tag="ffcol", bufs=16)
            b2_col = col_tile(b2f[l, si], KE, tag="ffcol", bufs=16)
            w0_col = col_tile(cwf[l, si, 0], KX, tag="ffcol", bufs=16)
            w1c_col = col_tile(cwf[l, si, 1], KX, tag="ffcol", bufs=16)
            w2_col = col_tile(cwf[l, si, 2], KX, tag="ffcol", bufs=16)
            A_col = col_tile(bnA[l, si], KX, tag="ffcol", bufs=16)
            B_col = col_tile(bnB[l, si], KX, tag="ffcol", bufs=16)
            g_col = col_tile(lng[l, 3 if s == "b" else 4], KE, tag="lncol", bufs=16)
            bb_col = col_tile(lnb[l, 3 if s == "b" else 4], KE, tag="lncol", bufs=16)
            yield
            PHASES.append((f"ffn{l}.{s}", len(nc.inst_map)))

            hts = [None] * NC
            xts = [None] * NC
            hl0 = [None] * NC
            hf2 = [None] * NC

            def compute_h(ci):
                xt = load_x_chunk(src, ci)
                xts[ci] = xt
                ht = pb.tile([128, KX, C], BF16, tag="a8")
                for m in range(KX):
                    ph = ps.tile([128, C], F32, tag="mm")
                    for k in range(KE):
                        mm(ph, w1t[:, k, m * 128:(m + 1) * 128], xt[:, k, :],
                           start=(k == 0), stop=(k == KE - 1))
                    hf = pc.tile([128, C], BF16, tag="a1")
                    sc.activation(hf, ph, AF.Relu, bias=b1_col[:, m:m + 1])
                    v.tensor_scalar_min(ht[:, m, :], hf, 6.0)
                hts[ci] = ht
                l0 = phl.tile([128, KX, 1], BF16, tag="hl")
                f2 = phl.tile([128, KX, 1], BF16, tag="hf")
                for m in range(KX):
                    v.tensor_scalar_mul(l0[:, m, :], ht[:, m, C - 1:C],
                                        w0_col[:, m:m + 1])
                    v.tensor_scalar_mul(f2[:, m, :], ht[:, m, 0:1],
                                        w2_col[:, m:m + 1])
                hl0[ci], hf2[ci] = l0, f2

            def conv_elem(ci):
                ht = hts[ci]
                h2 = pb.tile([128, KX, C], BF16, tag="a8")
                for m in range(KX):
                    acc = pc.tile([128, C], BF16, tag="a1")
                    v.tensor_scalar_mul(acc, ht[:, m, :], w1c_col[:, m:m + 1])
                    v.scalar_tensor_tensor(out=acc[:, 1:C],
                                           in0=ht[:, m, 0:C - 1],
                                           scalar=w0_col[:, m:m + 1],
                                           in1=acc[:, 1:C],
                                           op0=ALU.mult, op1=ALU.add)
                    if ci > 0:
                        v.tensor_tensor(out=acc[:, 0:1], in0=acc[:, 0:1],
                                        in1=hl0[ci - 1][:, m, :], op=ALU.add)
                    v.scalar_tensor_tensor(out=acc[:, 0:C - 1],
                                           in0=ht[:, m, 1:C],
                                           scalar=w2_col[:, m:m + 1],
                                           in1=acc[:, 0:C - 1],
                                           op0=ALU.mult, op1=ALU.add)
                    if ci < NC - 1:
                        v.tensor_tensor(out=acc[:, C - 1:C],
                                        in0=acc[:, C - 1:C],
                                        in1=hf2[ci + 1][:, m, :], op=ALU.add)
                    a2 = pc.tile([128, C], BF16, tag="a1")
                    sc.activation(a2, acc, AF.Relu, scale=A_col[:, m:m + 1],
                                  bias=B_col[:, m:m + 1])
                    v.tensor_scalar_min(h2[:, m, :], a2, 6.0)
                return h2

            def conv_pw(ci, h2):
                rt = pa.tile([128, KE, C], BF16, tag="a4")
                for m in range(KE):
                    pw = ps.tile([128, C], F32, tag="mm")
                    for k in range(KX):
                        mm(pw, w2t[:, k, m * 128:(m + 1) * 128], h2[:, k, :],
                           start=(k == 0), stop=(k == KX - 1))
                    v.scalar_tensor_tensor(out=rt[:, m, :], in0=pw,
                                           scalar=b2_col[:, m:m + 1],
                                           in1=xts[ci][:, m, :],
                                           op0=ALU.add, op1=ALU.add)
                outt = pa.tile([128, KE, C], BF16, tag="a4")
                ln_apply(rt, g_col, bb_col, outt)
                store_chunk(dst, ci, outt)
                hts[ci] = xts[ci] = None

            compute_h(0)
            yield "h"
            compute_h(1)
            yield "h"
            for ci in range(NC - 1):
                h2 = conv_elem(ci)
                yield "cv"
                conv_pw(ci, h2)
                yield "pw"
                if ci + 2 < NC:
                    compute_h(ci + 2)
                    yield "h"
            h2 = conv_elem(NC - 1)
            yield "cv"
            conv_pw(NC - 1, h2)
            yield "pw"

        # ---- layers (chunk-interleaved across independent streams) ----
        def adv(g, n=1):
            for _ in range(n):
                next(g, None)

        for l in range(L):
            bsrc = rs["b", 0] if l == 0 else rs["b", (l - 1, 3)]
            lsrc = rs["l", 0] if l == 0 else rs["l", (l - 1, 3)]
            g0 = attn_gen(l, 0, bsrc, bsrc,
                          *make_self_tail(l, "b", rs["b", (l, 1)]))
            g1 = attn_gen(l, 1, lsrc, lsrc,
                          *make_self_tail(l, "l", rs["l", (l, 1)]))
            g2 = attn_gen(l, 2, rs["b", (l, 1)], rs["l", (l, 1)],
                          *make_cross_tail(l, "b", rs["b", (l, 2)]))
            g3 = attn_gen(l, 3, rs["l", (l, 1)], rs["b", (l, 1)],
                          *make_cross_tail(l, "l", rs["l", (l, 2)]))
            gb = ffn_gen(l, "b", rs["b", (l, 2)], rs["b", (l, 3)])
            gl = ffn_gen(l, "l", rs["l", (l, 2)], rs["l", (l, 3)])
            adv(g0)                     # weights
            adv(g1)
            for _ in range(2 * NC):     # self alphas: a1/a2 staged
                adv(g0)
                adv(g1)
            adv(g0)                     # bd/kmm
            adv(g1)
            adv(g2)                     # prefetch cross weights
            adv(g3)
            for _ in range(3 * NC):     # self betas: b1/b2/b3 staged
                adv(g0)
                adv(g1)
            for _ in range(2 * NC):     # cross alphas staged
                adv(g2)
                adv(g3)
            adv(g2)                     # bd/kmm
            adv(g3)
            adv(gb)                     # prefetch ffn weights
            adv(gl)
            for _ in range(3 * NC):     # cross betas staged
                adv(g2)
                adv(g3)
            for _ in range(12):         # ffn h/cv/pw staged
                adv(gb)
                adv(gl)

        PHASES.append(("final", len(nc.inst_map)))
        # ---- final head ----
        fw1t = wbig.tile([128, 2 * KE, E2], BF16, tag="w", bufs=6)
        nc.sync.dma_start(out=fw1t,
                          in_=fw1.rearrange("(k p) g -> p k g", p=128))
        fw2t = wsm.tile([128, 2, E], BF16, tag="wfin", bufs=4)
        nc.sync.dma_start(out=fw2t,
                          in_=fw2.rearrange("(k p) e -> p k e", p=128))
        rw1t = wsm.tile([128, KE, E4], BF16, tag="wfin", bufs=4)
        nc.sync.dma_start(out=rw1t,
                          in_=rw1.rearrange("(k p) g -> p k g", p=128))
        rw2t = wsm.tile([128, E8], BF16, tag="wfin", bufs=4)
        nc.sync.dma_start(out=rw2t, in_=rw2)
        rw3t = wsm.tile([E8, 16], BF16, tag="wfin", bufs=4)
        nc.sync.dma_start(out=rw3t, in_=rw3p)
        rb3_row = row_tile(rb3p, 16)
        fb2_col = col_tile(fb2f, KE, tag="fcol")
        fb1_col = col_tile(fb1, 2, tag="fcol")
        flng_col = col_tile(flng, KE, tag="fcol")
        flnb_col = col_tile(flnb, KE, tag="fcol")
        rb1_col = col_tile(rb1, 1, tag="fcol")
        rb2_col = wcol.tile([E8, 1], F32, tag="fcol")
        nc.sync.dma_start(out=rb2_col, in_=rb2[:, None])
        out_ap = out_dram.ap()

        bsrc, lsrc = rs["b", (L - 1, 3)], rs["l", (L - 1, 3)]
        for ci in range(NC):
            xb = load_x_chunk(bsrc, ci)
            xl = load_x_chunk(lsrc, ci)
            f1t = pa.tile([128, 2, C], BF16, tag="a4")
            for m in range(2):
                pf = ps.tile([128, C], F32, tag="mm")
                for k in range(2 * KE):
                    rhs = xb[:, k, :] if k < KE else xl[:, k - KE, :]
                    mm(pf, fw1t[:, k, m * 128:(m + 1) * 128], rhs,
                       start=(k == 0), stop=(k == 2 * KE - 1))
                f1 = pc.tile([128, C], BF16, tag="a1")
                sc.activation(f1, pf, AF.Relu, bias=fb1_col[:, m:m + 1])
                v.tensor_scalar_min(f1t[:, m, :], f1, 6.0)
            ft = pa.tile([128, KE, C], BF16, tag="a4")
            for m in range(KE):
                pf2 = ps.tile([128, C], F32, tag="mm")
                for k in range(2):
                    mm(pf2, fw2t[:, k, m * 128:(m + 1) * 128], f1t[:, k, :],
                       start=(k == 0), stop=(k == 1))
                sc.activation(ft[:, m, :], pf2, AF.Identity,
                              bias=fb2_col[:, m:m + 1])
            frt = pa.tile([128, KE, C], BF16, tag="a4")
            ln_apply(ft, flng_col, flnb_col, frt)
            # relu after LN
            frf = frt.rearrange("p k c -> p (k c)")
            v.tensor_scalar_max(frf, frf, 0.0)
            p1 = ps.tile([128, C], F32, tag="mm")
            for k in range(KE):
                mm(p1, rw1t[:, k, :], frt[:, k, :], start=(k == 0),
                   stop=(k == KE - 1))
            h1f = pc.tile([128, C], BF16, tag="a1")
            sc.activation(h1f, p1, AF.Relu, bias=rb1_col[:, 0:1])
            h1t = pc.tile([128, C], BF16, tag="a1")
            v.tensor_scalar_min(h1t, h1f, 6.0)
            p2 = ps.tile([E8, C], F32, tag="mm")
            mm(p2, rw2t, h1t, start=True, stop=True)
            h2f = pc.tile([E8, C], BF16, tag="a1")
            sc.activation(h2f, p2, AF.Relu, bias=rb2_col[:, 0:1])
            h2t = pc.tile([E8, C], BF16, tag="a1")
            v.tensor_scalar_min(h2t, h2f, 6.0)
            ot = pc.tile([128, NTT, c.OUT], F32, tag="a1")
            for tt in range(NTT):
                p3 = ps.tile([128, 16], F32, tag="mm")
                mm(p3, h2t[:, tt * 128:(tt + 1) * 128], rw3t,
                   start=True, stop=False)
                mm(p3, ONES_ROW, rb3_row, start=False, stop=True)
                sc.activation(ot[:, tt, :], p3[:, 0:c.OUT], AF.Copy)
            nc.sync.dma_start(
                out=out_ap[ci * C:(ci + 1) * C, :].rearrange(
                    "(tt p) o -> p tt o", p=128),
                in_=ot)

    return din, out_dram


# ======================================================================
# kernel() entry point: full inputs in, full outputs out (8-core SPMD).
# ======================================================================
import concourse.bacc as _bacc
from concourse.bass_utils import run_bass_kernel_spmd as _run_spmd

_N_CORES = 8
_CACHE = {}


def _get_nc():
    if "nc" not in _CACHE:
        nc = _bacc.Bacc("TRN2", target_bir_lowering=False, debug=False)
        build(nc, Cfg())
        nc.finalize()
        _CACHE["nc"] = nc
    return _CACHE["nc"]


def _bf16(x):
    import ml_dtypes
    return np.asarray(x, dtype=np.float32).astype(ml_dtypes.bfloat16)


def host_prep(inputs):
    """Host-side weight preprocessing: compose QKV, fold BN, convert bf16."""
    c = Cfg()
    E, X, H, L = c.E, c.X, c.H, c.L
    E4, E2, E8 = E // 4, E // 2, E // 8
    f = {k: np.asarray(v, dtype=np.float32) for k, v in inputs.items()}
    dw, uw, ub = f["dw"], f["uw"], f["ub"]
    # composed q and k|v projection weights
    wq = np.matmul(dw[:, :, 0], uw[:, :, 0])          # (L,4,E,E)
    wk = np.matmul(dw[:, :, 1], uw[:, :, 1])
    wv = np.matmul(dw[:, :, 2], uw[:, :, 2])
    wkv = np.concatenate([wk, wv], axis=-1)           # (L,4,E,2E)
    ubq = ub[:, :, 0]                                 # (L,4,E)
    ubkv = np.concatenate([ub[:, :, 1], ub[:, :, 2]], axis=-1)
    rsq = np.float32(1.0 / np.sqrt(1.0 + BN_EPS))
    A = f["bng"] * rsq                                # (L,2,X)
    B = f["cb"] * A + f["bnb"]
    cwf = f["cw"].transpose(0, 1, 3, 2).copy()        # (L,2,3,X)
    gwd = f["gw2"][:, :, 0] - f["gw2"][:, :, 1]       # (L,E4)
    gb2d = (f["gb2"][:, 0] - f["gb2"][:, 1])[:, None]  # (L,1)
    rw3p = np.zeros((E8, 16), np.float32)
    rw3p[:, :c.OUT] = f["rw3"]
    rb3p = np.zeros((16,), np.float32)
    rb3p[:c.OUT] = f["rb3"]
    dh = E // H
    ident = np.eye(128, dtype=np.float32)
    ones = np.ones((128, 128), dtype=np.float32)
    hmask = np.zeros((E, H), dtype=np.float32)
    for ff in range(E):
        hmask[ff, ff // dh] = 1.0
    cmask = hmask.T.copy()

    b16 = dict(wq=wq, wkv=wkv, ubkv=ubkv, ow=f["ow"],
               w1=f["w1"], w2=f["w2"],
               gw1=f["gw1"], gwd=gwd, fw1=f["fw1"], fw2=f["fw2"],
               rw1=f["rw1"], rw2=f["rw2"], rw3p=rw3p,
               rb3p=rb3p, ident=ident, ones128=ones, hmask=hmask,
               cmask=cmask)
    f32 = dict(ubq=ubq, ob=f["ob"], b1=f["b1"], b2=f["b2"], fb2=f["fb2"],
               cwf=cwf, bnA=A, bnB=B,
               lng=f["lng"], lnb=f["lnb"],
               gb1=f["gb1"], gb2d=gb2d, fb1=f["fb1"], flng=f["flng"],
               flnb=f["flnb"], rb1=f["rb1"], rb2=f["rb2"])
    shared = {k: _bf16(v) for k, v in b16.items()}
    shared.update({k: np.ascontiguousarray(v, dtype=np.float32)
                   for k, v in f32.items()})
    return shared, f["body_feats"], f["limb_feats"]


def kernel(**inputs):
    nc = _get_nc()
    shared, body, limb = host_prep(inputs)
    in_maps = []
    for i in range(_N_CORES):
        m = dict(shared)
        m["body_feats"] = _bf16(body[i])
        m["limb_feats"] = _bf16(limb[i])
        in_maps.append(m)
    res = run_kernel_spmd_cached(nc, in_maps)
    out = np.stack([res[i]["out"] for i in range(_N_CORES)], axis=0)
    return out.astype(np.float32)


def run_kernel_spmd_cached(nc, in_maps, **kw):
    r = _run_spmd(nc, in_maps, list(range(_N_CORES)), **kw)
    _CACHE["last_result"] = r
    return r.results


# revision 6
# speedup vs baseline: 1.1530x; 1.0018x over previous
"""Dual-stream linear-attention transformer — bf16 redesign (per-core).

Layout convention (same as baseline):
  - "layout 1" activation: [E, N] feature-major; SBUF tiles [128, KE, C]
    (feature f = 128*k + p -> partition p, k-th slice; tokens on free dim).
  - alpha k/v are produced token-major per 128-token tile [128, E].
  - Residual streams live in internal DRAM as [E, N] bf16.

Key changes vs baseline:
  - All matmul operands + SBUF activations bf16 (same PE rate as f32r>=256,
    but DVE tensor_tensor 2x / tensor_scalar 4x, half DMA bytes).
  - QKV down+up projections composed into single E x E / E x 2E weights on
    the host (removes the low-rank intermediate copies).
  - Biases folded into matmuls as rank-1 accumulates (ones_row x bias_row).
  - elu+1 via 3 ops: ACT Exp, ACT Relu, DVE scalar_tensor_tensor(min,add).
  - LayerNorm apply via 3 bf16 DVE ops per slice (TT,TT,TS) instead of
    gpsimd tensor_tensor pairs.
  - FFN dwconv via TS/STT chain on DVE; BN folded on host into A,B.
  - relu6 of FFN h on gpsimd (idle engine) straight from PSUM.
  - m-outer matmul groups -> 1 PSUM bank live per group, fewer stalls.
"""

from dataclasses import dataclass
from contextlib import ExitStack

import numpy as np

import concourse.bass as bass
import concourse.mybir as mybir
import concourse.tile as tile

F32 = mybir.dt.float32
BF16 = mybir.dt.bfloat16
AF = mybir.ActivationFunctionType
ALU = mybir.AluOpType

LN_EPS = 1e-5
BN_EPS = 1e-5


@dataclass
class Cfg:
    N: int = 2048
    E: int = 512
    R: int = 256
    X: int = 1024
    H: int = 8
    L: int = 3
    OUT: int = 15
    C: int = 512

    @property
    def KE(self):
        return self.E // 128

    @property
    def KX(self):
        return self.X // 128

    @property
    def NC(self):
        return self.N // self.C

    @property
    def NTT(self):
        return self.C // 128


PHASES = []


def build(nc, cfg):
    c = cfg
    E, X, H, N, C, L = c.E, c.X, c.H, c.N, c.C, c.L
    KE, KX, NC, NTT = c.KE, c.KX, c.NC, c.NTT
    E4, E2, E8 = E // 4, E // 2, E // 8

    din = {}

    def inp(name, shape, dt=BF16):
        din[name] = nc.dram_tensor(name, list(shape), dt, kind="ExternalInput")
        return din[name].ap()

    # activations (host converts to bf16)
    body_feats = inp("body_feats", (N, E))
    limb_feats = inp("limb_feats", (N, E))
    # attention weights (host-composed)
    wq = inp("wq", (L, 4, E, E))
    wkv = inp("wkv", (L, 4, E, 2 * E))
    ubq = inp("ubq", (L, 4, E), F32)
    ubkv = inp("ubkv", (L, 4, 2 * E))
    ow = inp("ow", (L, 4, E, E))
    obf = inp("ob", (L, 4, E), F32)
    # FFN
    w1 = inp("w1", (L, 2, E, X))
    b1f = inp("b1", (L, 2, X), F32)
    cwf = inp("cwf", (L, 2, 3, X), F32)     # conv taps, tap-major
    bnA = inp("bnA", (L, 2, X), F32)        # bng*rsqrt(1+eps)
    bnB = inp("bnB", (L, 2, X), F32)        # cb*A + bnb
    w2 = inp("w2", (L, 2, X, E))
    b2f = inp("b2", (L, 2, E), F32)
    lng = inp("lng", (L, 5, E), F32)
    lnb = inp("lnb", (L, 5, E), F32)
    # gating
    gw1 = inp("gw1", (L, 2 * E, E4))
    gb1 = inp("gb1", (L, E4), F32)
    gwd = inp("gwd", (L, E4))               # gw2[:,0]-gw2[:,1]
    gb2d = inp("gb2d", (L, 1), F32)         # gb2[0]-gb2[1]
    # final head
    fw1 = inp("fw1", (2 * E, E2))
    fb1 = inp("fb1", (E2,), F32)
    fw2 = inp("fw2", (E2, E))
    fb2f = inp("fb2", (E,), F32)
    flng = inp("flng", (E,), F32)
    flnb = inp("flnb", (E,), F32)
    rw1 = inp("rw1", (E, E4))
    rb1 = inp("rb1", (E4,), F32)
    rw2 = inp("rw2", (E4, E8))
    rb2 = inp("rb2", (E8,), F32)
    rw3p = inp("rw3p", (E8, 16))            # zero-padded to 16
    rb3p = inp("rb3p", (16,))               # zero-padded
    ident_in = inp("ident", (128, 128))
    ones_in = inp("ones128", (128, 128))
    hmask_in = inp("hmask", (E, H))
    cmask_in = inp("cmask", (H, E))

    out_dram = nc.dram_tensor("out", [N, c.OUT], F32, kind="ExternalOutput")

    def idram(name):
        return nc.dram_tensor(name, [E, N], BF16).ap().rearrange(
            "(k p) n -> p k n", p=128)

    rs = {}
    for s in ("b", "l"):
        rs[s, 0] = idram(f"r{s}0")
        for l in range(L):
            for st in (1, 2, 3):
                rs[s, (l, st)] = idram(f"r{s}_{l}_{st}")

    lowp = nc.allow_low_precision("bf16 activations within rel-err budget")

    with tile.TileContext(nc) as tc, ExitStack() as ctx, lowp:
        p_ = ctx.enter_context
        cst = p_(tc.tile_pool(name="cst", bufs=1))
        wbig = p_(tc.tile_pool(name="wbig", bufs=3))
        wsm = p_(tc.tile_pool(name="wsm", bufs=2))
        wcol = p_(tc.tile_pool(name="wcol", bufs=10))
        wrow = p_(tc.tile_pool(name="wrow", bufs=6))
        pa = p_(tc.tile_pool(name="pa", bufs=9))      # 4KB bf16 chunk tiles
        pb = p_(tc.tile_pool(name="pb", bufs=5))      # 8KB ht tiles
        pc = p_(tc.tile_pool(name="pc", bufs=12))      # 1KB bf16 / rows
        pat = p_(tc.tile_pool(name="pat", bufs=3))    # per-attn persistents
        phl = p_(tc.tile_pool(name="phl", bufs=6))    # conv halos
        ps = p_(tc.tile_pool(name="ps", bufs=6, space="PSUM"))
        psr = p_(tc.tile_pool(name="psr", bufs=2, space="PSUM"))

        v, sc, gp = nc.vector, nc.scalar, nc.gpsimd

        def mm(out, lhsT, rhs, start, stop):
            nc.tensor.matmul(out, lhsT, rhs, start=start, stop=stop)

        # ---- constants ----
        ident_t = cst.tile([128, 128], BF16, tag="ident")
        nc.sync.dma_start(out=ident_t, in_=ident_in)
        ones_t = cst.tile([128, 128], BF16, tag="ones")
        nc.sync.dma_start(out=ones_t, in_=ones_in)
        hmask_t = cst.tile([128, KE, H], BF16, tag="hmask")
        nc.sync.dma_start(out=hmask_t,
                          in_=hmask_in.rearrange("(k p) h -> p k h", p=128))
        cmask_t = cst.tile([H, KE, 128], BF16, tag="cmask")
        nc.sync.dma_start(out=cmask_t,
                          in_=cmask_in.rearrange("h (k p) -> h k p", p=128))
        ONES_COL = ones_t[:, 0:1]
        ONES_ROW = ones_t[0:1, :]
        onesc_t = cst.tile([1, C], BF16, tag="onesc")
        v.memset(onesc_t, 1.0)
        ONES_C = onesc_t[0:1, :]
        eps_ln = cst.tile([1, 1], F32, tag="epsl")
        v.memset(eps_ln, LN_EPS)

        def col_tile(src_ap, m, tag="col", bufs=None):
            t = wcol.tile([128, m], F32, tag=tag,
                          bufs=(12 if tag == "col" else bufs))
            nc.sync.dma_start(out=t, in_=src_ap.rearrange("(m p) -> p m", p=128))
            return t

        def row_tile(src_ap, n, tag="row", pool=None):
            t = (pool or wrow).tile([1, n], BF16, tag=tag)
            nc.sync.dma_start(out=t, in_=src_ap[None, :])
            return t

        def ln_apply(xs, g_col, b_col, outt, extra_tt=None):
            """LayerNorm over features (layout 1). xs: [128, KE, C] bf16 tile.
            outt: [128, KE, C] bf16 out. extra_tt(m): None."""
            sq = pa.tile([128, KE, C], BF16, tag="a4")
            xf = xs.rearrange("p k c -> p (k c)")
            v.tensor_tensor(out=sq.rearrange("p k c -> p (k c)"),
                            in0=xf, in1=xf, op=ALU.mult)
            ps_s = psr.tile([1, C], F32, tag="row")
            ps_ss = psr.tile([1, C], F32, tag="row")
            for m in range(KE):
                mm(ps_s, ONES_COL, xs[:, m, :], start=(m == 0),
                   stop=(m == KE - 1))
                mm(ps_ss, ONES_COL, sq[:, m, :], start=(m == 0),
                   stop=(m == KE - 1))
            msq = pc.tile([1, C], F32, tag="row", bufs=8)
            sc.activation(msq, ps_ss, AF.Copy, scale=1.0 / E)
            m2 = pc.tile([1, C], F32, tag="row", bufs=8)
            sc.activation(m2, ps_s, AF.Square, scale=1.0 / E)
            var = pc.tile([1, C], F32, tag="row", bufs=8)
            v.tensor_tensor(out=var, in0=msq, in1=m2, op=ALU.subtract)
            srow = pc.tile([1, C], BF16, tag="rowh", bufs=8)
            sc.activation(srow, var, AF.Abs_reciprocal_sqrt,
                          bias=eps_ln[0:1, 0:1])
            trow = pc.tile([1, C], BF16, tag="rowh", bufs=8)
            v.scalar_tensor_tensor(out=trow, in0=ps_s, scalar=1.0 / E,
                                   in1=srow, op0=ALU.mult, op1=ALU.mult)
            sb_s = pc.tile([128, C], BF16, tag="a1")
            gp.partition_broadcast(sb_s, srow)
            sb_t = pc.tile([128, C], BF16, tag="a1")
            gp.partition_broadcast(sb_t, trow)
            for m in range(KE):
                u = pc.tile([128, C], BF16, tag="a1")
                v.tensor_tensor(out=u, in0=xs[:, m, :], in1=sb_s, op=ALU.mult)
                v.tensor_tensor(out=u, in0=u, in1=sb_t, op=ALU.subtract)
                v.tensor_scalar(out=outt[:, m, :], in0=u,
                                scalar1=g_col[:, m:m + 1],
                                scalar2=b_col[:, m:m + 1],
                                op0=ALU.mult, op1=ALU.add)

        def load_x_chunk(dram_l1, ci, tag="a4"):
            xt = pa.tile([128, KE, C], BF16, tag=tag)
            nc.sync.dma_start(out=xt, in_=dram_l1[:, :, ci * C:(ci + 1) * C])
            return xt

        def store_chunk(dram_l1, ci, t):
            gp.dma_start(out=dram_l1[:, :, ci * C:(ci + 1) * C], in_=t)

        # ---- entry transpose ----
        def entry(x_ap, dst):
            for ttk in range(N // 128):
                x2 = pa.tile([128, E], BF16, tag="a4")
                nc.sync.dma_start(out=x2, in_=x_ap[ttk * 128:(ttk + 1) * 128, :])
                xt = pa.tile([128, KE, 128], BF16, tag="a4")
                for f in range(KE):
                    pt = ps.tile([128, 128], BF16, tag="mm")
                    nc.tensor.transpose(pt, x2[:, f * 128:(f + 1) * 128],
                                        ident_t)
                    sc.activation(xt[:, f, :], pt, AF.Copy)
                nc.sync.dma_start(out=dst[:, :, ttk * 128:(ttk + 1) * 128],
                                  in_=xt)

        PHASES.append(("entry", len(nc.inst_map)))
        entry(body_feats, rs["b", 0])
        entry(limb_feats, rs["l", 0])

        # ---- linear attention ----
        def attn_gen(l, a, xq_dram, xkv_dram, tail_m, tail_post):
            """Generator: yields after weight loads, after each alpha chunk
            (bd/kmm ride with the last), and after each beta chunk."""
            wqt = wbig.tile([128, KE, E], BF16, tag="w", bufs=6)
            nc.sync.dma_start(
                out=wqt, in_=wq[l, a].rearrange("(k p) e -> p k e", p=128))
            wkvt = wbig.tile([128, KE, 2 * E], BF16, tag="w2x", bufs=4)
            nc.sync.dma_start(
                out=wkvt, in_=wkv[l, a].rearrange("(k p) e -> p k e", p=128))
            owt = wbig.tile([128, KE, E], BF16, tag="w", bufs=6)
            nc.sync.dma_start(
                out=owt, in_=ow[l, a].rearrange("(k p) e -> p k e", p=128))
            ubq_col = col_tile(ubq[l, a], KE)
            ubkv_row = row_tile(ubkv[l, a], 2 * E)
            ob_col = col_tile(obf[l, a], KE)
            yield

            PHASES.append((f"attn{l}.{a}.alpha", len(nc.inst_map)))
            kv_acc = pat.tile([128, KE, 258], F32, tag="kva", bufs=2)

            for ci in range(NC):
                xt = load_x_chunk(xkv_dram, ci)
                k2f = pa.tile([128, NTT, E], BF16, tag="a4")
                v2x = pa.tile([128, NTT, 2, 258], BF16, tag="a4")
                v.memset(v2x[:, :, :, 256:258], 1.0)
                for tt in range(NTT):
                    xs = xt[:, :, tt * 128:(tt + 1) * 128]
                    pk = ps.tile([128, E], F32, tag="mm")
                    pv = ps.tile([128, E], F32, tag="mm")
                    for k in range(KE):
                        mm(pk, xs[:, k, :], wkvt[:, k, 0:E],
                           start=(k == 0), stop=False)
                        mm(pv, xs[:, k, :], wkvt[:, k, E:2 * E],
                           start=(k == 0), stop=False)
                    mm(pk, ONES_ROW, ubkv_row[:, 0:E], start=False, stop=True)
                    mm(pv, ONES_ROW, ubkv_row[:, E:2 * E], start=False,
                       stop=True)
                    ee = pc.tile([128, E], BF16, tag="a1")
                    rr = pc.tile([128, E], BF16, tag="a1")
                    sc.activation(ee, pk, AF.Exp)
                    sc.activation(rr, pk, AF.Relu)
                    v.scalar_tensor_tensor(out=k2f[:, tt, :], in0=ee,
                                           scalar=1.0, in1=rr,
                                           op0=ALU.min, op1=ALU.add)
                    v.tensor_copy(v2x[:, tt, 0, 0:256], pv[:, 0:256])
                    v.tensor_copy(v2x[:, tt, 1, 0:256], pv[:, 256:512])
                yield "a1"
                for p in range(4):
                    pkv = ps.tile([128, 258], F32, tag="mm")
                    for tt in range(NTT):
                        mm(pkv, k2f[:, tt, p * 128:(p + 1) * 128],
                           v2x[:, tt, p // 2, :],
                           start=(tt == 0), stop=(tt == NTT - 1))
                    if ci == 0:
                        sc.activation(kv_acc[:, p, :], pkv, AF.Copy)
                    else:
                        v.tensor_tensor(out=kv_acc[:, p, :],
                                        in0=kv_acc[:, p, :], in1=pkv,
                                        op=ALU.add)
                yield "a2"

            bd = pat.tile([128, KE, 128], BF16, tag="bd", bufs=2)
            v.memset(bd, 0.0)
            for p in range(4):
                h0c = (2 * p % 4) * 64
                h1c = ((2 * p + 1) % 4) * 64
                v.tensor_copy(bd[0:64, p, 0:64], kv_acc[0:64, p, h0c:h0c + 64])
                v.tensor_copy(bd[64:128, p, 64:128],
                              kv_acc[64:128, p, h1c:h1c + 64])
            kmm = pat.tile([128, KE, H], BF16, tag="km")
            for k in range(KE):
                v.tensor_scalar_mul(kmm[:, k, :], hmask_t[:, k, :],
                                    kv_acc[:, k, 256:257])
            yield "bd"

            PHASES.append((f"attn{l}.{a}.beta", len(nc.inst_map)))
            for ci in range(NC):
                xq = load_x_chunk(xq_dram, ci)
                qf = pa.tile([128, KE, C], BF16, tag="a4")
                for m in range(KE):
                    pq = ps.tile([128, C], F32, tag="mm")
                    for k in range(KE):
                        mm(pq, wqt[:, k, m * 128:(m + 1) * 128], xq[:, k, :],
                           start=(k == 0), stop=(k == KE - 1))
                    ee = pc.tile([128, C], BF16, tag="a1")
                    rr = pc.tile([128, C], BF16, tag="a1")
                    sc.activation(ee, pq, AF.Exp, bias=ubq_col[:, m:m + 1])
                    sc.activation(rr, pq, AF.Relu, bias=ubq_col[:, m:m + 1])
                    v.scalar_tensor_tensor(out=qf[:, m, :], in0=ee, scalar=1.0,
                                           in1=rr, op0=ALU.min, op1=ALU.add)
                yield "b1"
                pd = psr.tile([8, C], F32, tag="row")
                for k in range(KE):
                    mm(pd, kmm[:, k, :], qf[:, k, :], start=(k == 0),
                       stop=(k == KE - 1))
                rec = pc.tile([8, C], BF16, tag="a1")
                v.reciprocal(out=rec, in_=pd)
                att = pa.tile([128, KE, C], BF16, tag="a4")
                for m in range(KE):
                    pn = ps.tile([128, C], F32, tag="mm")
                    mm(pn, bd[:, m, :], qf[:, m, :], start=True, stop=True)
                    pr = ps.tile([128, C], F32, tag="mm")
                    mm(pr, cmask_t[:, m, :], rec, start=True, stop=True)
                    rb = pc.tile([128, C], BF16, tag="a1")
                    sc.activation(rb, pr, AF.Copy)
                    v.tensor_tensor(out=att[:, m, :], in0=pn, in1=rb,
                                    op=ALU.mult)
                yield "b2"
                for m in range(KE):
                    pos = ps.tile([128, C], F32, tag="mm")
                    for k in range(KE):
                        mm(pos, owt[:, k, m * 128:(m + 1) * 128],
                           att[:, k, :], start=(k == 0), stop=(k == KE - 1))
                    tail_m(ci, m, pos, xq, ob_col)
                tail_post(ci, xq)
                yield "b3"

        # ---- tails ----
        def make_self_tail(l, s, dst):
            g_col = col_tile(lng[l, 0 if s == "b" else 1], KE, tag="lncol", bufs=16)
            b_col = col_tile(lnb[l, 0 if s == "b" else 1], KE, tag="lncol", bufs=16)
            rt_box = [None]

            def tail_m(ci, m, pos, xq, ob_col):
                if m == 0:
                    rt_box[0] = pa.tile([128, KE, C], BF16, tag="a4",
                                        name="rt")
                v.scalar_tensor_tensor(out=rt_box[0][:, m, :], in0=pos,
                                       scalar=ob_col[:, m:m + 1],
                                       in1=xq[:, m, :],
                                       op0=ALU.add, op1=ALU.add)

            def tail_post(ci, xq):
                rt = rt_box[0]
                outt = pa.tile([128, KE, C], BF16, tag="a4")
                ln_apply(rt, g_col, b_col, outt)
                store_chunk(dst, ci, outt)

            return tail_m, tail_post

        def make_cross_tail(l, s, dst):
            gw1t = wsm.tile([128, 2 * KE, E4], BF16, tag="ws")
            nc.sync.dma_start(out=gw1t,
                              in_=gw1[l].rearrange("(k p) g -> p k g", p=128))
            gwd_col = wcol.tile([128, 1], BF16, tag="gwd")
            nc.sync.dma_start(out=gwd_col, in_=gwd[l][:, None])
            gb1_col = col_tile(gb1[l], 1, tag="lncol", bufs=16)
            gb2d_t = pat.tile([1, 1], F32, tag="gb2d")
            nc.sync.dma_start(out=gb2d_t, in_=gb2d[l][None, :])
            g_col = col_tile(lng[l, 2], KE, tag="lncol", bufs=16)
            b_col = col_tile(lnb[l, 2], KE, tag="lncol", bufs=16)
            proj_box = [None]

            def tail_m(ci, m, pos, xq, ob_col):
                if m == 0:
                    proj_box[0] = pa.tile([128, KE, C], BF16, tag="a4",
                                          name="proj")
                sc.activation(proj_box[0][:, m, :], pos, AF.Identity,
                              bias=ob_col[:, m:m + 1])

            def tail_post(ci, xq):
                proj = proj_box[0]
                pg = ps.tile([128, C], F32, tag="mm")
                for k in range(2 * KE):
                    rhs = xq[:, k, :] if k < KE else proj[:, k - KE, :]
                    mm(pg, gw1t[:, k, :], rhs, start=(k == 0),
                       stop=(k == 2 * KE - 1))
                g1 = pc.tile([128, C], BF16, tag="a1")
                sc.activation(g1, pg, AF.Relu, bias=gb1_col[:, 0:1])
                g1t = pc.tile([128, C], BF16, tag="a1")
                v.tensor_scalar_min(g1t, g1, 6.0)
                pg2 = psr.tile([1, C], F32, tag="row")
                mm(pg2, gwd_col, g1t, start=True, stop=True)
                bg = pc.tile([1, C], BF16, tag="rowh", bufs=8)
                sc.activation(bg, pg2, AF.Sigmoid, bias=gb2d_t[0:1, 0:1])
                bgb = pc.tile([128, C], BF16, tag="a1")
                gp.partition_broadcast(bgb, bg)
                mt = pa.tile([128, KE, C], BF16, tag="a4")
                for m in range(KE):
                    dtmp = pc.tile([128, C], BF16, tag="a1")
                    v.tensor_tensor(out=dtmp, in0=xq[:, m, :],
                                    in1=proj[:, m, :], op=ALU.subtract)
                    v.tensor_tensor(out=dtmp, in0=dtmp, in1=bgb, op=ALU.mult)
                    v.tensor_tensor(out=mt[:, m, :], in0=dtmp,
                                    in1=proj[:, m, :], op=ALU.add)
                outt = pa.tile([128, KE, C], BF16, tag="a4")
                ln_apply(mt, g_col, b_col, outt)
                store_chunk(dst, ci, outt)

            return tail_m, tail_post

        # ---- FFN ----
        def ffn_gen(l, s, src, dst):
            si = 0 if s == "b" else 1
            w1t = wbig.tile([128, KE, X], BF16, tag="w2x", bufs=4)
            nc.sync.dma_start(
                out=w1t, in_=w1[l, si].rearrange("(k p) x -> p k x", p=128))
            w2t = wbig.tile([128, KX, E], BF16, tag="w2x", bufs=4)
            nc.sync.dma_start(
                out=w2t, in_=w2[l, si].rearrange("(k p) e -> p k e", p=128))
            b1_col = col_tile(b1f[l, si], KX, tag="ffcol", bufs=16)
            b2_col = col_tile(b2f[l, si], KE, tag="ffcol", bufs=16)
            w0_col = col_tile(cwf[l, si, 0], KX, tag="ffcol", bufs=16)
            w1c_col = col_tile(cwf[l, si, 1], KX, tag="ffcol", bufs=16)
            w2_col = col_tile(cwf[l, si, 2], KX, tag="ffcol", bufs=16)
            A_col = col_tile(bnA[l, si], KX, tag="ffcol", bufs=16)
            B_col = col_tile(bnB[l, si], KX, tag="ffcol", bufs=16)
            g_col = col_tile(lng[l, 3 if s == "b" else 4], KE, tag="lncol", bufs=16)
            bb_col = col_tile(lnb[l, 3 if s == "b" else 4], KE, tag="lncol", bufs=16)
            yield
            PHASES.append((f"ffn{l}.{s}", len(nc.inst_map)))

            hts = [None] * NC
            xts = [None] * NC
            hl0 = [None] * NC
            hf2 = [None] * NC

            def compute_h(ci):
                xt = load_x_chunk(src, ci)
                xts[ci] = xt
                ht = pb.tile([128, KX, C], BF16, tag="a8")
                for m in range(KX):
                    ph = ps.tile([128, C], F32, tag="mm")
                    for k in range(KE):
                        mm(ph, w1t[:, k, m * 128:(m + 1) * 128], xt[:, k, :],
                           start=(k == 0), stop=(k == KE - 1))
                    hf = pc.tile([128, C], BF16, tag="a1")
                    sc.activation(hf, ph, AF.Relu, bias=b1_col[:, m:m + 1])
                    v.tensor_scalar_min(ht[:, m, :], hf, 6.0)
                hts[ci] = ht
                l0 = phl.tile([128, KX, 1], BF16, tag="hl")
                f2 = phl.tile([128, KX, 1], BF16, tag="hf")
                for m in range(KX):
                    v.tensor_scalar_mul(l0[:, m, :], ht[:, m, C - 1:C],
                                        w0_col[:, m:m + 1])
                    v.tensor_scalar_mul(f2[:, m, :], ht[:, m, 0:1],
                                        w2_col[:, m:m + 1])
                hl0[ci], hf2[ci] = l0, f2

            def conv_elem(ci):
                ht = hts[ci]
                h2 = pb.tile([128, KX, C], BF16, tag="a8")
                for m in range(KX):
                    acc = pc.tile([128, C], BF16, tag="a1")
                    v.tensor_scalar_mul(acc, ht[:, m, :], w1c_col[:, m:m + 1])
                    v.scalar_tensor_tensor(out=acc[:, 1:C],
                                           in0=ht[:, m, 0:C - 1],
                                           scalar=w0_col[:, m:m + 1],
                                           in1=acc[:, 1:C],
                                           op0=ALU.mult, op1=ALU.add)
                    if ci > 0:
                        v.tensor_tensor(out=acc[:, 0:1], in0=acc[:, 0:1],
                                        in1=hl0[ci - 1][:, m, :], op=ALU.add)
                    v.scalar_tensor_tensor(out=acc[:, 0:C - 1],
                                           in0=ht[:, m, 1:C],
                                           scalar=w2_col[:, m:m + 1],
                                           in1=acc[:, 0:C - 1],
                                           op0=ALU.mult, op1=ALU.add)
                    if ci < NC - 1:
                        v.tensor_tensor(out=acc[:, C - 1:C],
                                        in0=acc[:, C - 1:C],
                                        in1=hf2[ci + 1][:, m, :], op=ALU.add)
                    a2 = pc.tile([128, C], BF16, tag="a1")
                    sc.activation(a2, acc, AF.Relu, scale=A_col[:, m:m + 1],
                                  bias=B_col[:, m:m + 1])
                    v.tensor_scalar_min(h2[:, m, :], a2, 6.0)
                return h2

            def conv_pw(ci, h2):
                rt = pa.tile([128, KE, C], BF16, tag="a4")
                for m in range(KE):
                    pw = ps.tile([128, C], F32, tag="mm")
                    for k in range(KX):
                        mm(pw, w2t[:, k, m * 128:(m + 1) * 128], h2[:, k, :],
                           start=(k == 0), stop=(k == KX - 1))
                    v.scalar_tensor_tensor(out=rt[:, m, :], in0=pw,
                                           scalar=b2_col[:, m:m + 1],
                                           in1=xts[ci][:, m, :],
                                           op0=ALU.add, op1=ALU.add)
                outt = pa.tile([128, KE, C], BF16, tag="a4")
                ln_apply(rt, g_col, bb_col, outt)
                store_chunk(dst, ci, outt)
                hts[ci] = xts[ci] = None

            compute_h(0)
            yield "h"
            compute_h(1)
            yield "h"
            for ci in range(NC - 1):
                h2 = conv_elem(ci)
                yield "cv"
                conv_pw(ci, h2)
                yield "pw"
                if ci + 2 < NC:
                    compute_h(ci + 2)
                    yield "h"
            h2 = conv_elem(NC - 1)
            yield "cv"
            conv_pw(NC - 1, h2)
            yield "pw"

        # ---- layers (chunk-interleaved across independent streams) ----
        def adv(g, n=1):
            for _ in range(n):
                next(g, None)

        for l in range(L):
            bsrc = rs["b", 0] if l == 0 else rs["b", (l - 1, 3)]
            lsrc = rs["l", 0] if l == 0 else rs["l", (l - 1, 3)]
            g0 = attn_gen(l, 0, bsrc, bsrc,
                          *make_self_tail(l, "b", rs["b", (l, 1)]))
            g1 = attn_gen(l, 1, lsrc, lsrc,
                          *make_self_tail(l, "l", rs["l", (l, 1)]))
            g2 = attn_gen(l, 2, rs["b", (l, 1)], rs["l", (l, 1)],
                          *make_cross_tail(l, "b", rs["b", (l, 2)]))
            g3 = attn_gen(l, 3, rs["l", (l, 1)], rs["b", (l, 1)],
                          *make_cross_tail(l, "l", rs["l", (l, 2)]))
            gb = ffn_gen(l, "b", rs["b", (l, 2)], rs["b", (l, 3)])
            gl = ffn_gen(l, "l", rs["l", (l, 2)], rs["l", (l, 3)])
            adv(g0)                     # weights
            adv(g1)
            for _ in range(2 * NC):     # self alphas: a1/a2 staged
                adv(g0)
                adv(g1)
            adv(g0)                     # bd/kmm
            adv(g1)
            adv(g2)                     # prefetch cross weights
            adv(g3)
            for _ in range(3 * NC):     # self betas: b1/b2/b3 staged
                adv(g0)
                adv(g1)
            for _ in range(2 * NC):     # cross alphas staged
                adv(g2)
                adv(g3)
            adv(g2)                     # bd/kmm
            adv(g3)
            adv(gb)                     # prefetch ffn weights
            adv(gl)
            for _ in range(3 * NC):     # cross betas staged
                adv(g2)
                adv(g3)
            for _ in range(12):         # ffn h/cv/pw staged
                adv(gb)
                adv(gl)

        PHASES.append(("final", len(nc.inst_map)))
        # ---- final head ----
        fw1t = wbig.tile([128, 2 * KE, E2], BF16, tag="w", bufs=6)
        nc.sync.dma_start(out=fw1t,
                          in_=fw1.rearrange("(k p) g -> p k g", p=128))
        fw2t = wsm.tile([128, 2, E], BF16, tag="wfin", bufs=4)
        nc.sync.dma_start(out=fw2t,
                          in_=fw2.rearrange("(k p) e -> p k e", p=128))
        rw1t = wsm.tile([128, KE, E4], BF16, tag="wfin", bufs=4)
        nc.sync.dma_start(out=rw1t,
                          in_=rw1.rearrange("(k p) g -> p k g", p=128))
        rw2t = wsm.tile([128, E8], BF16, tag="wfin", bufs=4)
        nc.sync.dma_start(out=rw2t, in_=rw2)
        rw3t = wsm.tile([E8, 16], BF16, tag="wfin", bufs=4)
        nc.sync.dma_start(out=rw3t, in_=rw3p)
        rb3_row = row_tile(rb3p, 16)
        fb2_col = col_tile(fb2f, KE, tag="fcol")
        fb1_col = col_tile(fb1, 2, tag="fcol")
        flng_col = col_tile(flng, KE, tag="fcol")
        flnb_col = col_tile(flnb, KE, tag="fcol")
        rb1_col = col_tile(rb1, 1, tag="fcol")
        rb2_col = wcol.tile([E8, 1], F32, tag="fcol")
        nc.sync.dma_start(out=rb2_col, in_=rb2[:, None])
        out_ap = out_dram.ap()

        bsrc, lsrc = rs["b", (L - 1, 3)], rs["l", (L - 1, 3)]
        for ci in range(NC):
            xb = load_x_chunk(bsrc, ci)
            xl = load_x_chunk(lsrc, ci)
            f1t = pa.tile([128, 2, C], BF16, tag="a4")
            for m in range(2):
                pf = ps.tile([128, C], F32, tag="mm")
                for k in range(2 * KE):
                    rhs = xb[:, k, :] if k < KE else xl[:, k - KE, :]
                    mm(pf, fw1t[:, k, m * 128:(m + 1) * 128], rhs,
                       start=(k == 0), stop=(k == 2 * KE - 1))
                f1 = pc.tile([128, C], BF16, tag="a1")
                sc.activation(f1, pf, AF.Relu, bias=fb1_col[:, m:m + 1])
                v.tensor_scalar_min(f1t[:, m, :], f1, 6.0)
            ft = pa.tile([128, KE, C], BF16, tag="a4")
            for m in range(KE):
                pf2 = ps.tile([128, C], F32, tag="mm")
                for k in range(2):
                    mm(pf2, fw2t[:, k, m * 128:(m + 1) * 128], f1t[:, k, :],
                       start=(k == 0), stop=(k == 1))
                sc.activation(ft[:, m, :], pf2, AF.Identity,
                              bias=fb2_col[:, m:m + 1])
            frt = pa.tile([128, KE, C], BF16, tag="a4")
            ln_apply(ft, flng_col, flnb_col, frt)
            # relu after LN
            frf = frt.rearrange("p k c -> p (k c)")
            v.tensor_scalar_max(frf, frf, 0.0)
            p1 = ps.tile([128, C], F32, tag="mm")
            for k in range(KE):
                mm(p1, rw1t[:, k, :], frt[:, k, :], start=(k == 0),
                   stop=(k == KE - 1))
            h1f = pc.tile([128, C], BF16, tag="a1")
            sc.activation(h1f, p1, AF.Relu, bias=rb1_col[:, 0:1])
            h1t = pc.tile([128, C], BF16, tag="a1")
            v.tensor_scalar_min(h1t, h1f, 6.0)
            p2 = ps.tile([E8, C], F32, tag="mm")
            mm(p2, rw2t, h1t, start=True, stop=True)
            h2f = pc.tile([E8, C], BF16, tag="a1")
            sc.activation(h2f, p2, AF.Relu, bias=rb2_col[:, 0:1])
            h2t = pc.tile([E8, C], BF16, tag="a1")
            v.tensor_scalar_min(h2t, h2f, 6.0)
            ot = pc.tile([128, NTT, c.OUT], F32, tag="a1")
            for tt in range(NTT):
                p3 = ps.tile([128, 16], F32, tag="mm")
                mm(p3, h2t[:, tt * 128:(tt + 1) * 128], rw3t,
                   start=True, stop=False)
                mm(p3, ONES_ROW, rb3_row, start=False, stop=True)
                sc.activation(ot[:, tt, :], p3[:, 0:c.OUT], AF.Copy)
            nc.sync.dma_start(
                out=out_ap[ci * C:(ci + 1) * C, :].rearrange(
                    "(tt p) o -> p tt o", p=128),
                in_=ot)

    return din, out_dram


# ======================================================================
# kernel() entry point: full inputs in, full outputs out (8-core SPMD).
# ======================================================================
import concourse.bacc as _bacc
from concourse.bass_utils import run_bass_kernel_spmd as _run_spmd

_N_CORES = 8
_CACHE = {}


def _get_nc():
    if "nc" not in _CACHE:
        nc = _bacc.Bacc("TRN2", target_bir_lowering=False, debug=False)
        build(nc, Cfg())
        nc.finalize()
        _CACHE["nc"] = nc
    return _CACHE["nc"]


def _bf16(x):
    import ml_dtypes
    return np.asarray(x, dtype=np.float32).astype(ml_dtypes.bfloat16)


def host_prep(inputs):
    """Host-side weight preprocessing: compose QKV, fold BN, convert bf16."""
    c = Cfg()
    E, X, H, L = c.E, c.X, c.H, c.L
    E4, E2, E8 = E // 4, E // 2, E // 8
    f = {k: np.asarray(v, dtype=np.float32) for k, v in inputs.items()}
    dw, uw, ub = f["dw"], f["uw"], f["ub"]
    # composed q and k|v projection weights
    wq = np.matmul(dw[:, :, 0], uw[:, :, 0])          # (L,4,E,E)
    wk = np.matmul(dw[:, :, 1], uw[:, :, 1])
    wv = np.matmul(dw[:, :, 2], uw[:, :, 2])
    wkv = np.concatenate([wk, wv], axis=-1)           # (L,4,E,2E)
    ubq = ub[:, :, 0]                                 # (L,4,E)
    ubkv = np.concatenate([ub[:, :, 1], ub[:, :, 2]], axis=-1)
    rsq = np.float32(1.0 / np.sqrt(1.0 + BN_EPS))
    A = f["bng"] * rsq                                # (L,2,X)
    B = f["cb"] * A + f["bnb"]
    cwf = f["cw"].transpose(0, 1, 3, 2).copy()        # (L,2,3,X)
    gwd = f["gw2"][:, :, 0] - f["gw2"][:, :, 1]       # (L,E4)
    gb2d = (f["gb2"][:, 0] - f["gb2"][:, 1])[:, None]  # (L,1)
    rw3p = np.zeros((E8, 16), np.float32)
    rw3p[:, :c.OUT] = f["rw3"]
    rb3p = np.zeros((16,), np.float32)
    rb3p[:c.OUT] = f["rb3"]
    dh = E // H
    ident = np.eye(128, dtype=np.float32)
    ones = np.ones((128, 128), dtype=np.float32)
    hmask = np.zeros((E, H), dtype=np.float32)
    for ff in range(E):
        hmask[ff, ff // dh] = 1.0
    cmask = hmask.T.copy()

    b16 = dict(wq=wq, wkv=wkv, ubkv=ubkv, ow=f["ow"],
               w1=f["w1"], w2=f["w2"],
               gw1=f["gw1"], gwd=gwd, fw1=f["fw1"], fw2=f["fw2"],
               rw1=f["rw1"], rw2=f["rw2"], rw3p=rw3p,
               rb3p=rb3p, ident=ident, ones128=ones, hmask=hmask,
               cmask=cmask)
    f32 = dict(ubq=ubq, ob=f["ob"], b1=f["b1"], b2=f["b2"], fb2=f["fb2"],
               cwf=cwf, bnA=A, bnB=B,
               lng=f["lng"], lnb=f["lnb"],
               gb1=f["gb1"], gb2d=gb2d, fb1=f["fb1"], flng=f["flng"],
               flnb=f["flnb"], rb1=f["rb1"], rb2=f["rb2"])
    shared = {k: _bf16(v) for k, v in b16.items()}
    shared.update({k: np.ascontiguousarray(v, dtype=np.float32)
                   for k, v in f32.items()})
    return shared, f["body_feats"], f["limb_feats"]


def kernel(**inputs):
    nc = _get_nc()
    shared, body, limb = host_prep(inputs)
    in_maps = []
    for i in range(_N_CORES):
        m = dict(shared)
        m["body_feats"] = _bf16(body[i])
        m["limb_feats"] = _bf16(limb[i])
        in_maps.append(m)
    res = run_kernel_spmd_cached(nc, in_maps)
    out = np.stack([res[i]["out"] for i in range(_N_CORES)], axis=0)
    return out.astype(np.float32)


def run_kernel_spmd_cached(nc, in_maps, **kw):
    r = _run_spmd(nc, in_maps, list(range(_N_CORES)), **kw)
    _CACHE["last_result"] = r
    return r.results


# revision 7
# speedup vs baseline: 1.1797x; 1.0232x over previous
"""Dual-stream linear-attention transformer — bf16 redesign (per-core).

Layout convention (same as baseline):
  - "layout 1" activation: [E, N] feature-major; SBUF tiles [128, KE, C]
    (feature f = 128*k + p -> partition p, k-th slice; tokens on free dim).
  - alpha k/v are produced token-major per 128-token tile [128, E].
  - Residual streams live in internal DRAM as [E, N] bf16.

Key changes vs baseline:
  - All matmul operands + SBUF activations bf16 (same PE rate as f32r>=256,
    but DVE tensor_tensor 2x / tensor_scalar 4x, half DMA bytes).
  - QKV down+up projections composed into single E x E / E x 2E weights on
    the host (removes the low-rank intermediate copies).
  - Biases folded into matmuls as rank-1 accumulates (ones_row x bias_row).
  - elu+1 via 3 ops: ACT Exp, ACT Relu, DVE scalar_tensor_tensor(min,add).
  - LayerNorm apply via 3 bf16 DVE ops per slice (TT,TT,TS) instead of
    gpsimd tensor_tensor pairs.
  - FFN dwconv via TS/STT chain on DVE; BN folded on host into A,B.
  - relu6 of FFN h on gpsimd (idle engine) straight from PSUM.
  - m-outer matmul groups -> 1 PSUM bank live per group, fewer stalls.
"""

from dataclasses import dataclass
from contextlib import ExitStack

import numpy as np

import concourse.bass as bass
import concourse.mybir as mybir
import concourse.tile as tile

F32 = mybir.dt.float32
BF16 = mybir.dt.bfloat16
AF = mybir.ActivationFunctionType
ALU = mybir.AluOpType

LN_EPS = 1e-5
BN_EPS = 1e-5


@dataclass
class Cfg:
    N: int = 2048
    E: int = 512
    R: int = 256
    X: int = 1024
    H: int = 8
    L: int = 3
    OUT: int = 15
    C: int = 512

    @property
    def KE(self):
        return self.E // 128

    @property
    def KX(self):
        return self.X // 128

    @property
    def NC(self):
        return self.N // self.C

    @property
    def NTT(self):
        return self.C // 128


PHASES = []


def build(nc, cfg):
    c = cfg
    E, X, H, N, C, L = c.E, c.X, c.H, c.N, c.C, c.L
    KE, KX, NC, NTT = c.KE, c.KX, c.NC, c.NTT
    E4, E2, E8 = E // 4, E // 2, E // 8

    din = {}

    def inp(name, shape, dt=BF16):
        din[name] = nc.dram_tensor(name, list(shape), dt, kind="ExternalInput")
        return din[name].ap()

    # activations (host converts to bf16)
    body_feats = inp("body_feats", (N, E))
    limb_feats = inp("limb_feats", (N, E))
    # attention weights (host-composed)
    wq = inp("wq", (L, 4, E, E))
    wkv = inp("wkv", (L, 4, E, 2 * E))
    ubq = inp("ubq", (L, 4, E), F32)
    ubkv = inp("ubkv", (L, 4, 2 * E))
    ow = inp("ow", (L, 4, E, E))
    obf = inp("ob", (L, 4, E), F32)
    # FFN
    w1 = inp("w1", (L, 2, E, X))
    b1f = inp("b1", (L, 2, X), F32)
    cwf = inp("cwf", (L, 2, 3, X), F32)     # conv taps, tap-major
    bnA = inp("bnA", (L, 2, X), F32)        # bng*rsqrt(1+eps)
    bnB = inp("bnB", (L, 2, X), F32)        # cb*A + bnb
    w2 = inp("w2", (L, 2, X, E))
    b2f = inp("b2", (L, 2, E), F32)
    lng = inp("lng", (L, 5, E), F32)
    lnb = inp("lnb", (L, 5, E), F32)
    # gating
    gw1 = inp("gw1", (L, 2 * E, E4))
    gb1 = inp("gb1", (L, E4), F32)
    gwd = inp("gwd", (L, E4))               # gw2[:,0]-gw2[:,1]
    gb2d = inp("gb2d", (L, 1), F32)         # gb2[0]-gb2[1]
    # final head
    fw1 = inp("fw1", (2 * E, E2))
    fb1 = inp("fb1", (E2,), F32)
    fw2 = inp("fw2", (E2, E))
    fb2f = inp("fb2", (E,), F32)
    flng = inp("flng", (E,), F32)
    flnb = inp("flnb", (E,), F32)
    rw1 = inp("rw1", (E, E4))
    rb1 = inp("rb1", (E4,), F32)
    rw2 = inp("rw2", (E4, E8))
    rb2 = inp("rb2", (E8,), F32)
    rw3p = inp("rw3p", (E8, 16))            # zero-padded to 16
    rb3p = inp("rb3p", (16,))               # zero-padded
    ident_in = inp("ident", (128, 128))
    ones_in = inp("ones128", (128, 128))
    hmask_in = inp("hmask", (E, H))
    cmask_in = inp("cmask", (H, E))

    out_dram = nc.dram_tensor("out", [N, c.OUT], F32, kind="ExternalOutput")

    def idram(name):
        return nc.dram_tensor(name, [E, N], BF16).ap().rearrange(
            "(k p) n -> p k n", p=128)

    rs = {}
    for s in ("b", "l"):
        rs[s, 0] = idram(f"r{s}0")
        for l in range(L):
            for st in (1, 2, 3):
                rs[s, (l, st)] = idram(f"r{s}_{l}_{st}")

    lowp = nc.allow_low_precision("bf16 activations within rel-err budget")

    with tile.TileContext(nc) as tc, ExitStack() as ctx, lowp:
        p_ = ctx.enter_context
        cst = p_(tc.tile_pool(name="cst", bufs=1))
        wbig = p_(tc.tile_pool(name="wbig", bufs=3))
        wsm = p_(tc.tile_pool(name="wsm", bufs=2))
        wcol = p_(tc.tile_pool(name="wcol", bufs=10))
        wrow = p_(tc.tile_pool(name="wrow", bufs=6))
        pa = p_(tc.tile_pool(name="pa", bufs=9))      # 4KB bf16 chunk tiles
        pb = p_(tc.tile_pool(name="pb", bufs=5))      # 8KB ht tiles
        pc = p_(tc.tile_pool(name="pc", bufs=12))      # 1KB bf16 / rows
        pat = p_(tc.tile_pool(name="pat", bufs=3))    # per-attn persistents
        phl = p_(tc.tile_pool(name="phl", bufs=6))    # conv halos
        ps = p_(tc.tile_pool(name="ps", bufs=6, space="PSUM"))
        psr = p_(tc.tile_pool(name="psr", bufs=2, space="PSUM"))

        v, sc, gp = nc.vector, nc.scalar, nc.gpsimd

        def mm(out, lhsT, rhs, start, stop):
            nc.tensor.matmul(out, lhsT, rhs, start=start, stop=stop)

        # ---- constants ----
        ident_t = cst.tile([128, 128], BF16, tag="ident")
        nc.sync.dma_start(out=ident_t, in_=ident_in)
        ones_t = cst.tile([128, 128], BF16, tag="ones")
        nc.sync.dma_start(out=ones_t, in_=ones_in)
        hmask_t = cst.tile([128, KE, H], BF16, tag="hmask")
        nc.sync.dma_start(out=hmask_t,
                          in_=hmask_in.rearrange("(k p) h -> p k h", p=128))
        cmask_t = cst.tile([H, KE, 128], BF16, tag="cmask")
        nc.sync.dma_start(out=cmask_t,
                          in_=cmask_in.rearrange("h (k p) -> h k p", p=128))
        ONES_COL = ones_t[:, 0:1]
        ONES_ROW = ones_t[0:1, :]
        onesc_t = cst.tile([1, C], BF16, tag="onesc")
        v.memset(onesc_t, 1.0)
        ONES_C = onesc_t[0:1, :]
        eps_ln = cst.tile([1, 1], F32, tag="epsl")
        v.memset(eps_ln, LN_EPS)

        def col_tile(src_ap, m, tag="col", bufs=None):
            t = wcol.tile([128, m], F32, tag=tag,
                          bufs=(12 if tag == "col" else bufs))
            nc.sync.dma_start(out=t, in_=src_ap.rearrange("(m p) -> p m", p=128))
            return t

        def row_tile(src_ap, n, tag="row", pool=None):
            t = (pool or wrow).tile([1, n], BF16, tag=tag)
            nc.sync.dma_start(out=t, in_=src_ap[None, :])
            return t

        def ln_apply(xs, g_col, b_col, outt, extra_tt=None):
            """LayerNorm over features (layout 1). xs: [128, KE, C] bf16 tile.
            outt: [128, KE, C] bf16 out. extra_tt(m): None."""
            sq = pa.tile([128, KE, C], BF16, tag="a4")
            xf = xs.rearrange("p k c -> p (k c)")
            v.tensor_tensor(out=sq.rearrange("p k c -> p (k c)"),
                            in0=xf, in1=xf, op=ALU.mult)
            ps_s = psr.tile([1, C], F32, tag="row")
            ps_ss = psr.tile([1, C], F32, tag="row")
            for m in range(KE):
                mm(ps_s, ONES_COL, xs[:, m, :], start=(m == 0),
                   stop=(m == KE - 1))
                mm(ps_ss, ONES_COL, sq[:, m, :], start=(m == 0),
                   stop=(m == KE - 1))
            msq = pc.tile([1, C], F32, tag="row", bufs=8)
            sc.activation(msq, ps_ss, AF.Copy, scale=1.0 / E)
            m2 = pc.tile([1, C], F32, tag="row", bufs=8)
            sc.activation(m2, ps_s, AF.Square, scale=1.0 / E)
            var = pc.tile([1, C], F32, tag="row", bufs=8)
            v.tensor_tensor(out=var, in0=msq, in1=m2, op=ALU.subtract)
            srow = pc.tile([1, C], BF16, tag="rowh", bufs=8)
            sc.activation(srow, var, AF.Abs_reciprocal_sqrt,
                          bias=eps_ln[0:1, 0:1])
            trow = pc.tile([1, C], BF16, tag="rowh", bufs=8)
            v.scalar_tensor_tensor(out=trow, in0=ps_s, scalar=1.0 / E,
                                   in1=srow, op0=ALU.mult, op1=ALU.mult)
            sb_s = pc.tile([128, C], BF16, tag="a1")
            gp.partition_broadcast(sb_s, srow)
            sb_t = pc.tile([128, C], BF16, tag="a1")
            gp.partition_broadcast(sb_t, trow)
            for m in range(KE):
                u = pc.tile([128, C], BF16, tag="a1")
                v.tensor_tensor(out=u, in0=xs[:, m, :], in1=sb_s, op=ALU.mult)
                v.tensor_tensor(out=u, in0=u, in1=sb_t, op=ALU.subtract)
                v.tensor_scalar(out=outt[:, m, :], in0=u,
                                scalar1=g_col[:, m:m + 1],
                                scalar2=b_col[:, m:m + 1],
                                op0=ALU.mult, op1=ALU.add)

        def load_x_chunk(dram_l1, ci, tag="a4"):
            xt = pa.tile([128, KE, C], BF16, tag=tag)
            nc.sync.dma_start(out=xt, in_=dram_l1[:, :, ci * C:(ci + 1) * C])
            return xt

        def store_chunk(dram_l1, ci, t):
            gp.dma_start(out=dram_l1[:, :, ci * C:(ci + 1) * C], in_=t)

        # ---- entry transpose ----
        def entry(x_ap, dst):
            for ttk in range(N // 128):
                x2 = pa.tile([128, E], BF16, tag="a4")
                nc.sync.dma_start(out=x2, in_=x_ap[ttk * 128:(ttk + 1) * 128, :])
                xt = pa.tile([128, KE, 128], BF16, tag="a4")
                for f in range(KE):
                    pt = ps.tile([128, 128], BF16, tag="mm")
                    nc.tensor.transpose(pt, x2[:, f * 128:(f + 1) * 128],
                                        ident_t)
                    sc.activation(xt[:, f, :], pt, AF.Copy)
                nc.sync.dma_start(out=dst[:, :, ttk * 128:(ttk + 1) * 128],
                                  in_=xt)

        PHASES.append(("entry", len(nc.inst_map)))
        entry(body_feats, rs["b", 0])
        entry(limb_feats, rs["l", 0])

        # ---- linear attention ----
        def attn_gen(l, a, xq_dram, xkv_dram, tail_m, tail_post):
            """Generator: yields after weight loads, after each alpha chunk
            (bd/kmm ride with the last), and after each beta chunk."""
            wqt = wbig.tile([128, KE, E], BF16, tag="w", bufs=6)
            nc.sync.dma_start(
                out=wqt, in_=wq[l, a].rearrange("(k p) e -> p k e", p=128))
            wkvt = wbig.tile([128, KE, 2 * E], BF16, tag="w2x", bufs=4)
            nc.sync.dma_start(
                out=wkvt, in_=wkv[l, a].rearrange("(k p) e -> p k e", p=128))
            owt = wbig.tile([128, KE, E], BF16, tag="w", bufs=6)
            nc.sync.dma_start(
                out=owt, in_=ow[l, a].rearrange("(k p) e -> p k e", p=128))
            ubq_col = col_tile(ubq[l, a], KE)
            ubkv_row = row_tile(ubkv[l, a], 2 * E)
            ob_col = col_tile(obf[l, a], KE)
            yield

            PHASES.append((f"attn{l}.{a}.alpha", len(nc.inst_map)))
            kv_acc = pat.tile([128, KE, 258], F32, tag="kva", bufs=2)

            for ci in range(NC):
                xt = load_x_chunk(xkv_dram, ci)
                k2f = pa.tile([128, NTT, E], BF16, tag="a4")
                v2x = pa.tile([128, NTT, 2, 258], BF16, tag="a4")
                v.memset(v2x[:, :, :, 256:258], 1.0)
                for tt in range(NTT):
                    xs = xt[:, :, tt * 128:(tt + 1) * 128]
                    pk = ps.tile([128, E], F32, tag="mm")
                    pv = ps.tile([128, E], F32, tag="mm")
                    for k in range(KE):
                        mm(pk, xs[:, k, :], wkvt[:, k, 0:E],
                           start=(k == 0), stop=False)
                        mm(pv, xs[:, k, :], wkvt[:, k, E:2 * E],
                           start=(k == 0), stop=False)
                    mm(pk, ONES_ROW, ubkv_row[:, 0:E], start=False, stop=True)
                    mm(pv, ONES_ROW, ubkv_row[:, E:2 * E], start=False,
                       stop=True)
                    ee = pc.tile([128, E], BF16, tag="a1")
                    rr = pc.tile([128, E], BF16, tag="a1")
                    sc.activation(ee, pk, AF.Exp)
                    sc.activation(rr, pk, AF.Relu)
                    v.scalar_tensor_tensor(out=k2f[:, tt, :], in0=ee,
                                           scalar=1.0, in1=rr,
                                           op0=ALU.min, op1=ALU.add)
                    v.tensor_copy(v2x[:, tt, 0, 0:256], pv[:, 0:256])
                    v.tensor_copy(v2x[:, tt, 1, 0:256], pv[:, 256:512])
                yield "a1"
                for p in range(4):
                    pkv = ps.tile([128, 258], F32, tag="mm")
                    for tt in range(NTT):
                        mm(pkv, k2f[:, tt, p * 128:(p + 1) * 128],
                           v2x[:, tt, p // 2, :],
                           start=(tt == 0), stop=(tt == NTT - 1))
                    if ci == 0:
                        sc.activation(kv_acc[:, p, :], pkv, AF.Copy)
                    else:
                        v.tensor_tensor(out=kv_acc[:, p, :],
                                        in0=kv_acc[:, p, :], in1=pkv,
                                        op=ALU.add)
                yield "a2"

            bd = pat.tile([128, KE, 128], BF16, tag="bd", bufs=2)
            v.memset(bd, 0.0)
            for p in range(4):
                h0c = (2 * p % 4) * 64
                h1c = ((2 * p + 1) % 4) * 64
                v.tensor_copy(bd[0:64, p, 0:64], kv_acc[0:64, p, h0c:h0c + 64])
                v.tensor_copy(bd[64:128, p, 64:128],
                              kv_acc[64:128, p, h1c:h1c + 64])
            kmm = pat.tile([128, KE, H], BF16, tag="km")
            for k in range(KE):
                v.tensor_scalar_mul(kmm[:, k, :], hmask_t[:, k, :],
                                    kv_acc[:, k, 256:257])
            yield "bd"

            PHASES.append((f"attn{l}.{a}.beta", len(nc.inst_map)))
            for ci in range(NC):
                xq = load_x_chunk(xq_dram, ci)
                qf = pa.tile([128, KE, C], BF16, tag="a4")
                for m in range(KE):
                    pq = ps.tile([128, C], F32, tag="mm")
                    for k in range(KE):
                        mm(pq, wqt[:, k, m * 128:(m + 1) * 128], xq[:, k, :],
                           start=(k == 0), stop=(k == KE - 1))
                    ee = pc.tile([128, C], BF16, tag="a1")
                    rr = pc.tile([128, C], BF16, tag="a1")
                    sc.activation(ee, pq, AF.Exp, bias=ubq_col[:, m:m + 1])
                    sc.activation(rr, pq, AF.Relu, bias=ubq_col[:, m:m + 1])
                    v.scalar_tensor_tensor(out=qf[:, m, :], in0=ee, scalar=1.0,
                                           in1=rr, op0=ALU.min, op1=ALU.add)
                yield "b1"
                pd = psr.tile([8, C], F32, tag="row")
                for k in range(KE):
                    mm(pd, kmm[:, k, :], qf[:, k, :], start=(k == 0),
                       stop=(k == KE - 1))
                rec = pc.tile([8, C], BF16, tag="a1")
                v.reciprocal(out=rec, in_=pd)
                att = pa.tile([128, KE, C], BF16, tag="a4")
                for m in range(KE):
                    pn = ps.tile([128, C], F32, tag="mm")
                    mm(pn, bd[:, m, :], qf[:, m, :], start=True, stop=True)
                    pr = ps.tile([128, C], F32, tag="mm")
                    mm(pr, cmask_t[:, m, :], rec, start=True, stop=True)
                    rb = pc.tile([128, C], BF16, tag="a1")
                    sc.activation(rb, pr, AF.Copy)
                    v.tensor_tensor(out=att[:, m, :], in0=pn, in1=rb,
                                    op=ALU.mult)
                yield "b2"
                for m in range(KE):
                    pos = ps.tile([128, C], F32, tag="mm")
                    for k in range(KE):
                        mm(pos, owt[:, k, m * 128:(m + 1) * 128],
                           att[:, k, :], start=(k == 0), stop=(k == KE - 1))
                    tail_m(ci, m, pos, xq, ob_col)
                tail_post(ci, xq)
                yield "b3"

        # ---- tails ----
        def make_self_tail(l, s, dst):
            g_col = col_tile(lng[l, 0 if s == "b" else 1], KE, tag="lncol", bufs=16)
            b_col = col_tile(lnb[l, 0 if s == "b" else 1], KE, tag="lncol", bufs=16)
            rt_box = [None]

            def tail_m(ci, m, pos, xq, ob_col):
                if m == 0:
                    rt_box[0] = pa.tile([128, KE, C], BF16, tag="a4",
                                        name="rt")
                v.scalar_tensor_tensor(out=rt_box[0][:, m, :], in0=pos,
                                       scalar=ob_col[:, m:m + 1],
                                       in1=xq[:, m, :],
                                       op0=ALU.add, op1=ALU.add)

            def tail_post(ci, xq):
                rt = rt_box[0]
                outt = pa.tile([128, KE, C], BF16, tag="a4")
                ln_apply(rt, g_col, b_col, outt)
                store_chunk(dst, ci, outt)

            return tail_m, tail_post

        def make_cross_tail(l, s, dst):
            gw1t = wsm.tile([128, 2 * KE, E4], BF16, tag="ws")
            nc.sync.dma_start(out=gw1t,
                              in_=gw1[l].rearrange("(k p) g -> p k g", p=128))
            gwd_col = wcol.tile([128, 1], BF16, tag="gwd")
            nc.sync.dma_start(out=gwd_col, in_=gwd[l][:, None])
            gb1_col = col_tile(gb1[l], 1, tag="lncol", bufs=16)
            gb2d_t = pat.tile([1, 1], F32, tag="gb2d")
            nc.sync.dma_start(out=gb2d_t, in_=gb2d[l][None, :])
            g_col = col_tile(lng[l, 2], KE, tag="lncol", bufs=16)
            b_col = col_tile(lnb[l, 2], KE, tag="lncol", bufs=16)
            proj_box = [None]

            def tail_m(ci, m, pos, xq, ob_col):
                if m == 0:
                    proj_box[0] = pa.tile([128, KE, C], BF16, tag="a4",
                                          name="proj")
                sc.activation(proj_box[0][:, m, :], pos, AF.Identity,
                              bias=ob_col[:, m:m + 1])

            def tail_post(ci, xq):
                proj = proj_box[0]
                pg = ps.tile([128, C], F32, tag="mm")
                for k in range(2 * KE):
                    rhs = xq[:, k, :] if k < KE else proj[:, k - KE, :]
                    mm(pg, gw1t[:, k, :], rhs, start=(k == 0),
                       stop=(k == 2 * KE - 1))
                g1 = pc.tile([128, C], BF16, tag="a1")
                sc.activation(g1, pg, AF.Relu, bias=gb1_col[:, 0:1])
                g1t = pc.tile([128, C], BF16, tag="a1")
                v.tensor_scalar_min(g1t, g1, 6.0)
                pg2 = psr.tile([1, C], F32, tag="row")
                mm(pg2, gwd_col, g1t, start=True, stop=True)
                bg = pc.tile([1, C], BF16, tag="rowh", bufs=8)
                sc.activation(bg, pg2, AF.Sigmoid, bias=gb2d_t[0:1, 0:1])
                bgb = pc.tile([128, C], BF16, tag="a1")
                gp.partition_broadcast(bgb, bg)
                mt = pa.tile([128, KE, C], BF16, tag="a4")
                for m in range(KE):
                    dtmp = pc.tile([128, C], BF16, tag="a1")
                    v.tensor_tensor(out=dtmp, in0=xq[:, m, :],
                                    in1=proj[:, m, :], op=ALU.subtract)
                    v.tensor_tensor(out=dtmp, in0=dtmp, in1=bgb, op=ALU.mult)
                    v.tensor_tensor(out=mt[:, m, :], in0=dtmp,
                                    in1=proj[:, m, :], op=ALU.add)
                outt = pa.tile([128, KE, C], BF16, tag="a4")
                ln_apply(mt, g_col, b_col, outt)
                store_chunk(dst, ci, outt)

            return tail_m, tail_post

        # ---- FFN ----
        def ffn_gen(l, s, src, dst):
            si = 0 if s == "b" else 1
            w1t = wbig.tile([128, KE, X], BF16, tag="w2x", bufs=4)
            nc.sync.dma_start(
                out=w1t, in_=w1[l, si].rearrange("(k p) x -> p k x", p=128))
            w2t = wbig.tile([128, KX, E], BF16, tag="w2x", bufs=4)
            nc.sync.dma_start(
                out=w2t, in_=w2[l, si].rearrange("(k p) e -> p k e", p=128))
            b1_col = col_tile(b1f[l, si], KX, tag="ffcol", bufs=16)
            b2_col = col_tile(b2f[l, si], KE, tag="ffcol", bufs=16)
            w0_col = col_tile(cwf[l, si, 0], KX, tag="ffcol", bufs=16)
            w1c_col = col_tile(cwf[l, si, 1], KX, tag="ffcol", bufs=16)
            w2_col = col_tile(cwf[l, si, 2], KX, tag="ffcol", bufs=16)
            A_col = col_tile(bnA[l, si], KX, tag="ffcol", bufs=16)
            B_col = col_tile(bnB[l, si], KX, tag="ffcol", bufs=16)
            g_col = col_tile(lng[l, 3 if s == "b" else 4], KE, tag="lncol", bufs=16)
            bb_col = col_tile(lnb[l, 3 if s == "b" else 4], KE, tag="lncol", bufs=16)
            yield
            PHASES.append((f"ffn{l}.{s}", len(nc.inst_map)))

            hts = [None] * NC
            xts = [None] * NC
            hl0 = [None] * NC
            hf2 = [None] * NC

            def compute_h(ci):
                xt = load_x_chunk(src, ci)
                xts[ci] = xt
                ht = pb.tile([128, KX, C], BF16, tag="a8")
                for m in range(KX):
                    ph = ps.tile([128, C], F32, tag="mm")
                    for k in range(KE):
                        mm(ph, w1t[:, k, m * 128:(m + 1) * 128], xt[:, k, :],
                           start=(k == 0), stop=(k == KE - 1))
                    hf = pc.tile([128, C], BF16, tag="a1")
                    sc.activation(hf, ph, AF.Relu, bias=b1_col[:, m:m + 1])
                    v.tensor_scalar_min(ht[:, m, :], hf, 6.0)
                hts[ci] = ht
                l0 = phl.tile([128, KX, 1], BF16, tag="hl")
                f2 = phl.tile([128, KX, 1], BF16, tag="hf")
                for m in range(KX):
                    v.tensor_scalar_mul(l0[:, m, :], ht[:, m, C - 1:C],
                                        w0_col[:, m:m + 1])
                    v.tensor_scalar_mul(f2[:, m, :], ht[:, m, 0:1],
                                        w2_col[:, m:m + 1])
                hl0[ci], hf2[ci] = l0, f2

            def conv_elem(ci):
                ht = hts[ci]
                h2 = pb.tile([128, KX, C], BF16, tag="a8")
                for m in range(KX):
                    acc = pc.tile([128, C], BF16, tag="a1")
                    v.tensor_scalar_mul(acc, ht[:, m, :], w1c_col[:, m:m + 1])
                    v.scalar_tensor_tensor(out=acc[:, 1:C],
                                           in0=ht[:, m, 0:C - 1],
                                           scalar=w0_col[:, m:m + 1],
                                           in1=acc[:, 1:C],
                                           op0=ALU.mult, op1=ALU.add)
                    if ci > 0:
                        v.tensor_tensor(out=acc[:, 0:1], in0=acc[:, 0:1],
                                        in1=hl0[ci - 1][:, m, :], op=ALU.add)
                    v.scalar_tensor_tensor(out=acc[:, 0:C - 1],
                                           in0=ht[:, m, 1:C],
                                           scalar=w2_col[:, m:m + 1],
                                           in1=acc[:, 0:C - 1],
                                           op0=ALU.mult, op1=ALU.add)
                    if ci < NC - 1:
                        v.tensor_tensor(out=acc[:, C - 1:C],
                                        in0=acc[:, C - 1:C],
                                        in1=hf2[ci + 1][:, m, :], op=ALU.add)
                    a2 = pc.tile([128, C], BF16, tag="a1")
                    sc.activation(a2, acc, AF.Relu, scale=A_col[:, m:m + 1],
                                  bias=B_col[:, m:m + 1])
                    v.tensor_scalar_min(h2[:, m, :], a2, 6.0)
                return h2

            def conv_pw(ci, h2):
                rt = pa.tile([128, KE, C], BF16, tag="a4")
                for m in range(KE):
                    pw = ps.tile([128, C], F32, tag="mm")
                    for k in range(KX):
                        mm(pw, w2t[:, k, m * 128:(m + 1) * 128], h2[:, k, :],
                           start=(k == 0), stop=(k == KX - 1))
                    v.scalar_tensor_tensor(out=rt[:, m, :], in0=pw,
                                           scalar=b2_col[:, m:m + 1],
                                           in1=xts[ci][:, m, :],
                                           op0=ALU.add, op1=ALU.add)
                outt = pa.tile([128, KE, C], BF16, tag="a4")
                ln_apply(rt, g_col, bb_col, outt)
                store_chunk(dst, ci, outt)
                hts[ci] = xts[ci] = None

            compute_h(0)
            yield "h"
            compute_h(1)
            yield "h"
            for ci in range(NC - 1):
                h2 = conv_elem(ci)
                yield "cv"
                conv_pw(ci, h2)
                yield "pw"
                if ci + 2 < NC:
                    compute_h(ci + 2)
                    yield "h"
            h2 = conv_elem(NC - 1)
            yield "cv"
            conv_pw(NC - 1, h2)
            yield "pw"

        # ---- layers (chunk-interleaved across independent streams) ----
        def adv(g, n=1):
            for _ in range(n):
                next(g, None)

        for l in range(L):
            bsrc = rs["b", 0] if l == 0 else rs["b", (l - 1, 3)]
            lsrc = rs["l", 0] if l == 0 else rs["l", (l - 1, 3)]
            g0 = attn_gen(l, 0, bsrc, bsrc,
                          *make_self_tail(l, "b", rs["b", (l, 1)]))
            g1 = attn_gen(l, 1, lsrc, lsrc,
                          *make_self_tail(l, "l", rs["l", (l, 1)]))
            g2 = attn_gen(l, 2, rs["b", (l, 1)], rs["l", (l, 1)],
                          *make_cross_tail(l, "b", rs["b", (l, 2)]))
            g3 = attn_gen(l, 3, rs["l", (l, 1)], rs["b", (l, 1)],
                          *make_cross_tail(l, "l", rs["l", (l, 2)]))
            gb = ffn_gen(l, "b", rs["b", (l, 2)], rs["b", (l, 3)])
            gl = ffn_gen(l, "l", rs["l", (l, 2)], rs["l", (l, 3)])
            adv(g0)                     # weights
            adv(g1)
            for _ in range(2 * NC):     # self alphas: a1/a2 staged
                adv(g0)
                adv(g1)
            adv(g0)                     # bd/kmm
            adv(g1)
            adv(g2)                     # prefetch cross weights
            adv(g3)
            for _ in range(3 * NC):     # self betas: b1/b2/b3 staged
                adv(g0)
                adv(g1)
            for _ in range(2 * NC):     # cross alphas staged
                adv(g2)
                adv(g3)
            adv(g2)                     # bd/kmm
            adv(g3)
            adv(gb)                     # prefetch ffn weights
            adv(gl)
            for _ in range(3 * NC):     # cross betas staged
                adv(g2)
                adv(g3)
            for _ in range(12):         # ffn h/cv/pw staged
                adv(gb)
                adv(gl)

        PHASES.append(("final", len(nc.inst_map)))
        # ---- final head ----
        fw1t = wbig.tile([128, 2 * KE, E2], BF16, tag="w", bufs=6)
        nc.sync.dma_start(out=fw1t,
                          in_=fw1.rearrange("(k p) g -> p k g", p=128))
        fw2t = wsm.tile([128, 2, E], BF16, tag="wfin", bufs=4)
        nc.sync.dma_start(out=fw2t,
                          in_=fw2.rearrange("(k p) e -> p k e", p=128))
        rw1t = wsm.tile([128, KE, E4], BF16, tag="wfin", bufs=4)
        nc.sync.dma_start(out=rw1t,
                          in_=rw1.rearrange("(k p) g -> p k g", p=128))
        rw2t = wsm.tile([128, E8], BF16, tag="wfin", bufs=4)
        nc.sync.dma_start(out=rw2t, in_=rw2)
        rw3t = wsm.tile([E8, 16], BF16, tag="wfin", bufs=4)
        nc.sync.dma_start(out=rw3t, in_=rw3p)
        rb3_row = row_tile(rb3p, 16)
        fb2_col = col_tile(fb2f, KE, tag="fcol")
        fb1_col = col_tile(fb1, 2, tag="fcol")
        flng_col = col_tile(flng, KE, tag="fcol")
        flnb_col = col_tile(flnb, KE, tag="fcol")
        rb1_col = col_tile(rb1, 1, tag="fcol")
        rb2_col = wcol.tile([E8, 1], F32, tag="fcol")
        nc.sync.dma_start(out=rb2_col, in_=rb2[:, None])
        out_ap = out_dram.ap()

        bsrc, lsrc = rs["b", (L - 1, 3)], rs["l", (L - 1, 3)]

        def final_stage1(ci):
            xb = load_x_chunk(bsrc, ci)
            xl = load_x_chunk(lsrc, ci)
            f1t = pa.tile([128, 2, C], BF16, tag="a4")
            for m in range(2):
                pf = ps.tile([128, C], F32, tag="mm")
                for k in range(2 * KE):
                    rhs = xb[:, k, :] if k < KE else xl[:, k - KE, :]
                    mm(pf, fw1t[:, k, m * 128:(m + 1) * 128], rhs,
                       start=(k == 0), stop=(k == 2 * KE - 1))
                f1 = pc.tile([128, C], BF16, tag="a1")
                sc.activation(f1, pf, AF.Relu, bias=fb1_col[:, m:m + 1])
                v.tensor_scalar_min(f1t[:, m, :], f1, 6.0)
            ft = pa.tile([128, KE, C], BF16, tag="a4")
            for m in range(KE):
                pf2 = ps.tile([128, C], F32, tag="mm")
                for k in range(2):
                    mm(pf2, fw2t[:, k, m * 128:(m + 1) * 128], f1t[:, k, :],
                       start=(k == 0), stop=(k == 1))
                sc.activation(ft[:, m, :], pf2, AF.Identity,
                              bias=fb2_col[:, m:m + 1])
            frt = pa.tile([128, KE, C], BF16, tag="a4")
            ln_apply(ft, flng_col, flnb_col, frt)
            # relu after LN
            frf = frt.rearrange("p k c -> p (k c)")
            v.tensor_scalar_max(frf, frf, 0.0)
            return frt

        def final_stage2(ci, frt):
            p1 = ps.tile([128, C], F32, tag="mm")
            for k in range(KE):
                mm(p1, rw1t[:, k, :], frt[:, k, :], start=(k == 0),
                   stop=(k == KE - 1))
            h1f = pc.tile([128, C], BF16, tag="a1")
            sc.activation(h1f, p1, AF.Relu, bias=rb1_col[:, 0:1])
            h1t = pc.tile([128, C], BF16, tag="a1")
            v.tensor_scalar_min(h1t, h1f, 6.0)
            p2 = ps.tile([E8, C], F32, tag="mm")
            mm(p2, rw2t, h1t, start=True, stop=True)
            h2f = pc.tile([E8, C], BF16, tag="a1")
            sc.activation(h2f, p2, AF.Relu, bias=rb2_col[:, 0:1])
            h2t = pc.tile([E8, C], BF16, tag="a1")
            v.tensor_scalar_min(h2t, h2f, 6.0)
            ot = pc.tile([128, NTT, c.OUT], F32, tag="a1")
            for tt in range(NTT):
                p3 = ps.tile([128, 16], F32, tag="mm")
                mm(p3, h2t[:, tt * 128:(tt + 1) * 128], rw3t,
                   start=True, stop=False)
                mm(p3, ONES_ROW, rb3_row, start=False, stop=True)
                sc.activation(ot[:, tt, :], p3[:, 0:c.OUT], AF.Copy)
            nc.sync.dma_start(
                out=out_ap[ci * C:(ci + 1) * C, :].rearrange(
                    "(tt p) o -> p tt o", p=128),
                in_=ot)

        frts = [None] * NC
        for ci in range(NC):
            frts[ci] = final_stage1(ci)
            if ci >= 1:
                final_stage2(ci - 1, frts[ci - 1])
        final_stage2(NC - 1, frts[NC - 1])

    return din, out_dram


# ======================================================================
# kernel() entry point: full inputs in, full outputs out (8-core SPMD).
# ======================================================================
import concourse.bacc as _bacc
from concourse.bass_utils import run_bass_kernel_spmd as _run_spmd

_N_CORES = 8
_CACHE = {}


def _get_nc():
    if "nc" not in _CACHE:
        nc = _bacc.Bacc("TRN2", target_bir_lowering=False, debug=False)
        build(nc, Cfg())
        nc.finalize()
        _CACHE["nc"] = nc
    return _CACHE["nc"]


def _bf16(x):
    import ml_dtypes
    return np.asarray(x, dtype=np.float32).astype(ml_dtypes.bfloat16)


def host_prep(inputs):
    """Host-side weight preprocessing: compose QKV, fold BN, convert bf16."""
    c = Cfg()
    E, X, H, L = c.E, c.X, c.H, c.L
    E4, E2, E8 = E // 4, E // 2, E // 8
    f = {k: np.asarray(v, dtype=np.float32) for k, v in inputs.items()}
    dw, uw, ub = f["dw"], f["uw"], f["ub"]
    # composed q and k|v projection weights
    wq = np.matmul(dw[:, :, 0], uw[:, :, 0])          # (L,4,E,E)
    wk = np.matmul(dw[:, :, 1], uw[:, :, 1])
    wv = np.matmul(dw[:, :, 2], uw[:, :, 2])
    wkv = np.concatenate([wk, wv], axis=-1)           # (L,4,E,2E)
    ubq = ub[:, :, 0]                                 # (L,4,E)
    ubkv = np.concatenate([ub[:, :, 1], ub[:, :, 2]], axis=-1)
    rsq = np.float32(1.0 / np.sqrt(1.0 + BN_EPS))
    A = f["bng"] * rsq                                # (L,2,X)
    B = f["cb"] * A + f["bnb"]
    cwf = f["cw"].transpose(0, 1, 3, 2).copy()        # (L,2,3,X)
    gwd = f["gw2"][:, :, 0] - f["gw2"][:, :, 1]       # (L,E4)
    gb2d = (f["gb2"][:, 0] - f["gb2"][:, 1])[:, None]  # (L,1)
    rw3p = np.zeros((E8, 16), np.float32)
    rw3p[:, :c.OUT] = f["rw3"]
    rb3p = np.zeros((16,), np.float32)
    rb3p[:c.OUT] = f["rb3"]
    dh = E // H
    ident = np.eye(128, dtype=np.float32)
    ones = np.ones((128, 128), dtype=np.float32)
    hmask = np.zeros((E, H), dtype=np.float32)
    for ff in range(E):
        hmask[ff, ff // dh] = 1.0
    cmask = hmask.T.copy()

    b16 = dict(wq=wq, wkv=wkv, ubkv=ubkv, ow=f["ow"],
               w1=f["w1"], w2=f["w2"],
               gw1=f["gw1"], gwd=gwd, fw1=f["fw1"], fw2=f["fw2"],
               rw1=f["rw1"], rw2=f["rw2"], rw3p=rw3p,
               rb3p=rb3p, ident=ident, ones128=ones, hmask=hmask,
               cmask=cmask)
    f32 = dict(ubq=ubq, ob=f["ob"], b1=f["b1"], b2=f["b2"], fb2=f["fb2"],
               cwf=cwf, bnA=A, bnB=B,
               lng=f["lng"], lnb=f["lnb"],
               gb1=f["gb1"], gb2d=gb2d, fb1=f["fb1"], flng=f["flng"],
               flnb=f["flnb"], rb1=f["rb1"], rb2=f["rb2"])
    shared = {k: _bf16(v) for k, v in b16.items()}
    shared.update({k: np.ascontiguousarray(v, dtype=np.float32)
                   for k, v in f32.items()})
    return shared, f["body_feats"], f["limb_feats"]


def kernel(**inputs):
    nc = _get_nc()
    shared, body, limb = host_prep(inputs)
    in_maps = []
    for i in range(_N_CORES):
        m = dict(shared)
        m["body_feats"] = _bf16(body[i])
        m["limb_feats"] = _bf16(limb[i])
        in_maps.append(m)
    res = run_kernel_spmd_cached(nc, in_maps)
    out = np.stack([res[i]["out"] for i in range(_N_CORES)], axis=0)
    return out.astype(np.float32)


def run_kernel_spmd_cached(nc, in_maps, **kw):
    r = _run_spmd(nc, in_maps, list(range(_N_CORES)), **kw)
    _CACHE["last_result"] = r
    return r.results


# revision 8
# speedup vs baseline: 1.1815x; 1.0015x over previous
"""Dual-stream linear-attention transformer — bf16 redesign (per-core).

Layout convention (same as baseline):
  - "layout 1" activation: [E, N] feature-major; SBUF tiles [128, KE, C]
    (feature f = 128*k + p -> partition p, k-th slice; tokens on free dim).
  - alpha k/v are produced token-major per 128-token tile [128, E].
  - Residual streams live in internal DRAM as [E, N] bf16.

Key changes vs baseline:
  - All matmul operands + SBUF activations bf16 (same PE rate as f32r>=256,
    but DVE tensor_tensor 2x / tensor_scalar 4x, half DMA bytes).
  - QKV down+up projections composed into single E x E / E x 2E weights on
    the host (removes the low-rank intermediate copies).
  - Biases folded into matmuls as rank-1 accumulates (ones_row x bias_row).
  - elu+1 via 3 ops: ACT Exp, ACT Relu, DVE scalar_tensor_tensor(min,add).
  - LayerNorm apply via 3 bf16 DVE ops per slice (TT,TT,TS) instead of
    gpsimd tensor_tensor pairs.
  - FFN dwconv via TS/STT chain on DVE; BN folded on host into A,B.
  - relu6 of FFN h on gpsimd (idle engine) straight from PSUM.
  - m-outer matmul groups -> 1 PSUM bank live per group, fewer stalls.
"""

from dataclasses import dataclass
from contextlib import ExitStack

import numpy as np

import concourse.bass as bass
import concourse.mybir as mybir
import concourse.tile as tile

F32 = mybir.dt.float32
BF16 = mybir.dt.bfloat16
AF = mybir.ActivationFunctionType
ALU = mybir.AluOpType

LN_EPS = 1e-5
BN_EPS = 1e-5


@dataclass
class Cfg:
    N: int = 2048
    E: int = 512
    R: int = 256
    X: int = 1024
    H: int = 8
    L: int = 3
    OUT: int = 15
    C: int = 512

    @property
    def KE(self):
        return self.E // 128

    @property
    def KX(self):
        return self.X // 128

    @property
    def NC(self):
        return self.N // self.C

    @property
    def NTT(self):
        return self.C // 128


PHASES = []


def build(nc, cfg):
    c = cfg
    E, X, H, N, C, L = c.E, c.X, c.H, c.N, c.C, c.L
    KE, KX, NC, NTT = c.KE, c.KX, c.NC, c.NTT
    E4, E2, E8 = E // 4, E // 2, E // 8

    din = {}

    def inp(name, shape, dt=BF16):
        din[name] = nc.dram_tensor(name, list(shape), dt, kind="ExternalInput")
        return din[name].ap()

    # activations (host converts to bf16)
    body_feats = inp("body_feats", (N, E))
    limb_feats = inp("limb_feats", (N, E))
    # attention weights (host-composed)
    wq = inp("wq", (L, 4, E, E))
    wkv = inp("wkv", (L, 4, E, 2 * E))
    ubq = inp("ubq", (L, 4, E), F32)
    ubkv = inp("ubkv", (L, 4, 2 * E))
    ow = inp("ow", (L, 4, E, E))
    obf = inp("ob", (L, 4, E), F32)
    # FFN
    w1 = inp("w1", (L, 2, E, X))
    b1f = inp("b1", (L, 2, X), F32)
    cwf = inp("cwf", (L, 2, 3, X), F32)     # conv taps, tap-major
    bnA = inp("bnA", (L, 2, X), F32)        # bng*rsqrt(1+eps)
    bnB = inp("bnB", (L, 2, X), F32)        # cb*A + bnb
    w2 = inp("w2", (L, 2, X, E))
    b2f = inp("b2", (L, 2, E), F32)
    lng = inp("lng", (L, 5, E), F32)
    lnb = inp("lnb", (L, 5, E), F32)
    # gating
    gw1 = inp("gw1", (L, 2 * E, E4))
    gb1 = inp("gb1", (L, E4), F32)
    gwd = inp("gwd", (L, E4))               # gw2[:,0]-gw2[:,1]
    gb2d = inp("gb2d", (L, 1), F32)         # gb2[0]-gb2[1]
    # final head
    fw1 = inp("fw1", (2 * E, E2))
    fb1 = inp("fb1", (E2,), F32)
    fw2 = inp("fw2", (E2, E))
    fb2f = inp("fb2", (E,), F32)
    flng = inp("flng", (E,), F32)
    flnb = inp("flnb", (E,), F32)
    rw1 = inp("rw1", (E, E4))
    rb1 = inp("rb1", (E4,), F32)
    rw2 = inp("rw2", (E4, E8))
    rb2 = inp("rb2", (E8,), F32)
    rw3p = inp("rw3p", (E8, 16))            # zero-padded to 16
    rb3p = inp("rb3p", (16,))               # zero-padded
    ident_in = inp("ident", (128, 128))
    ones_in = inp("ones128", (128, 128))
    hmask_in = inp("hmask", (E, H))
    cmask_in = inp("cmask", (H, E))

    out_dram = nc.dram_tensor("out", [N, c.OUT], F32, kind="ExternalOutput")

    def idram(name):
        return nc.dram_tensor(name, [E, N], BF16).ap().rearrange(
            "(k p) n -> p k n", p=128)

    rs = {}
    for s in ("b", "l"):
        rs[s, 0] = idram(f"r{s}0")
        for l in range(L):
            for st in (1, 2, 3):
                rs[s, (l, st)] = idram(f"r{s}_{l}_{st}")

    lowp = nc.allow_low_precision("bf16 activations within rel-err budget")

    with tile.TileContext(nc) as tc, ExitStack() as ctx, lowp:
        p_ = ctx.enter_context
        cst = p_(tc.tile_pool(name="cst", bufs=1))
        wbig = p_(tc.tile_pool(name="wbig", bufs=3))
        wsm = p_(tc.tile_pool(name="wsm", bufs=2))
        wcol = p_(tc.tile_pool(name="wcol", bufs=10))
        wrow = p_(tc.tile_pool(name="wrow", bufs=6))
        pa = p_(tc.tile_pool(name="pa", bufs=9))      # 4KB bf16 chunk tiles
        pb = p_(tc.tile_pool(name="pb", bufs=5))      # 8KB ht tiles
        pc = p_(tc.tile_pool(name="pc", bufs=12))      # 1KB bf16 / rows
        pat = p_(tc.tile_pool(name="pat", bufs=3))    # per-attn persistents
        phl = p_(tc.tile_pool(name="phl", bufs=6))    # conv halos
        ps = p_(tc.tile_pool(name="ps", bufs=6, space="PSUM"))
        psr = p_(tc.tile_pool(name="psr", bufs=2, space="PSUM"))

        v, sc, gp = nc.vector, nc.scalar, nc.gpsimd

        def mm(out, lhsT, rhs, start, stop):
            nc.tensor.matmul(out, lhsT, rhs, start=start, stop=stop)

        # ---- constants ----
        ident_t = cst.tile([128, 128], BF16, tag="ident")
        nc.sync.dma_start(out=ident_t, in_=ident_in)
        ones_t = cst.tile([128, 128], BF16, tag="ones")
        nc.sync.dma_start(out=ones_t, in_=ones_in)
        hmask_t = cst.tile([128, KE, H], BF16, tag="hmask")
        nc.sync.dma_start(out=hmask_t,
                          in_=hmask_in.rearrange("(k p) h -> p k h", p=128))
        cmask_t = cst.tile([H, KE, 128], BF16, tag="cmask")
        nc.sync.dma_start(out=cmask_t,
                          in_=cmask_in.rearrange("h (k p) -> h k p", p=128))
        ONES_COL = ones_t[:, 0:1]
        ONES_ROW = ones_t[0:1, :]
        onesc_t = cst.tile([1, C], BF16, tag="onesc")
        v.memset(onesc_t, 1.0)
        ONES_C = onesc_t[0:1, :]
        eps_ln = cst.tile([1, 1], F32, tag="epsl")
        v.memset(eps_ln, LN_EPS)

        def col_tile(src_ap, m, tag="col", bufs=None):
            t = wcol.tile([128, m], F32, tag=tag,
                          bufs=(12 if tag == "col" else bufs))
            nc.sync.dma_start(out=t, in_=src_ap.rearrange("(m p) -> p m", p=128))
            return t

        def row_tile(src_ap, n, tag="row", pool=None):
            t = (pool or wrow).tile([1, n], BF16, tag=tag)
            nc.sync.dma_start(out=t, in_=src_ap[None, :])
            return t

        def ln_apply(xs, g_col, b_col, outt, extra_tt=None):
            """LayerNorm over features (layout 1). xs: [128, KE, C] bf16 tile.
            outt: [128, KE, C] bf16 out. extra_tt(m): None."""
            sq = pa.tile([128, KE, C], BF16, tag="a4")
            xf = xs.rearrange("p k c -> p (k c)")
            v.tensor_tensor(out=sq.rearrange("p k c -> p (k c)"),
                            in0=xf, in1=xf, op=ALU.mult)
            ps_s = psr.tile([1, C], F32, tag="row")
            ps_ss = psr.tile([1, C], F32, tag="row")
            for m in range(KE):
                mm(ps_s, ONES_COL, xs[:, m, :], start=(m == 0),
                   stop=(m == KE - 1))
                mm(ps_ss, ONES_COL, sq[:, m, :], start=(m == 0),
                   stop=(m == KE - 1))
            msq = pc.tile([1, C], F32, tag="row", bufs=8)
            sc.activation(msq, ps_ss, AF.Copy, scale=1.0 / E)
            m2 = pc.tile([1, C], F32, tag="row", bufs=8)
            sc.activation(m2, ps_s, AF.Square, scale=1.0 / E)
            var = pc.tile([1, C], F32, tag="row", bufs=8)
            v.tensor_tensor(out=var, in0=msq, in1=m2, op=ALU.subtract)
            srow = pc.tile([1, C], BF16, tag="rowh", bufs=8)
            sc.activation(srow, var, AF.Abs_reciprocal_sqrt,
                          bias=eps_ln[0:1, 0:1])
            trow = pc.tile([1, C], BF16, tag="rowh", bufs=8)
            v.scalar_tensor_tensor(out=trow, in0=ps_s, scalar=1.0 / E,
                                   in1=srow, op0=ALU.mult, op1=ALU.mult)
            sb_s = pc.tile([128, C], BF16, tag="a1")
            gp.partition_broadcast(sb_s, srow)
            sb_t = pc.tile([128, C], BF16, tag="a1")
            gp.partition_broadcast(sb_t, trow)
            for m in range(KE):
                u = pc.tile([128, C], BF16, tag="a1")
                v.tensor_tensor(out=u, in0=xs[:, m, :], in1=sb_s, op=ALU.mult)
                v.tensor_tensor(out=u, in0=u, in1=sb_t, op=ALU.subtract)
                v.tensor_scalar(out=outt[:, m, :], in0=u,
                                scalar1=g_col[:, m:m + 1],
                                scalar2=b_col[:, m:m + 1],
                                op0=ALU.mult, op1=ALU.add)

        def load_x_chunk(dram_l1, ci, tag="a4"):
            xt = pa.tile([128, KE, C], BF16, tag=tag)
            nc.sync.dma_start(out=xt, in_=dram_l1[:, :, ci * C:(ci + 1) * C])
            return xt

        def store_chunk(dram_l1, ci, t):
            gp.dma_start(out=dram_l1[:, :, ci * C:(ci + 1) * C], in_=t)

        # ---- entry transpose ----
        def entry(x_ap, dst):
            for ttk in range(N // 128):
                x2 = pa.tile([128, E], BF16, tag="a4")
                nc.sync.dma_start(out=x2, in_=x_ap[ttk * 128:(ttk + 1) * 128, :])
                xt = pa.tile([128, KE, 128], BF16, tag="a4")
                for f in range(KE):
                    pt = ps.tile([128, 128], BF16, tag="mm")
                    nc.tensor.transpose(pt, x2[:, f * 128:(f + 1) * 128],
                                        ident_t)
                    sc.activation(xt[:, f, :], pt, AF.Copy)
                nc.sync.dma_start(out=dst[:, :, ttk * 128:(ttk + 1) * 128],
                                  in_=xt)

        PHASES.append(("entry", len(nc.inst_map)))
        entry(body_feats, rs["b", 0])
        entry(limb_feats, rs["l", 0])

        # ---- linear attention ----
        def attn_gen(l, a, xq_dram, xkv_dram, tail_m, tail_post):
            """Generator: yields after weight loads, after each alpha chunk
            (bd/kmm ride with the last), and after each beta chunk."""
            wqt = wbig.tile([128, KE, E], BF16, tag="w", bufs=6)
            nc.sync.dma_start(
                out=wqt, in_=wq[l, a].rearrange("(k p) e -> p k e", p=128))
            wkvt = wbig.tile([128, KE, 2 * E], BF16, tag="w2x", bufs=4)
            nc.sync.dma_start(
                out=wkvt, in_=wkv[l, a].rearrange("(k p) e -> p k e", p=128))
            owt = wbig.tile([128, KE, E], BF16, tag="w", bufs=6)
            nc.sync.dma_start(
                out=owt, in_=ow[l, a].rearrange("(k p) e -> p k e", p=128))
            ubq_col = col_tile(ubq[l, a], KE)
            ubkv_row = row_tile(ubkv[l, a], 2 * E)
            ob_col = col_tile(obf[l, a], KE)
            yield

            PHASES.append((f"attn{l}.{a}.alpha", len(nc.inst_map)))
            kv_acc = pat.tile([128, KE, 258], F32, tag="kva", bufs=2)

            xpf = [None] * NC
            xpf[0] = load_x_chunk(xkv_dram, 0)
            for ci in range(NC):
                if ci + 1 < NC:
                    xpf[ci + 1] = load_x_chunk(xkv_dram, ci + 1)
                xt = xpf[ci]
                xpf[ci] = None
                k2f = pa.tile([128, NTT, E], BF16, tag="a4")
                v2x = pa.tile([128, NTT, 2, 258], BF16, tag="a4")
                v.memset(v2x[:, :, :, 256:258], 1.0)
                for tt in range(NTT):
                    xs = xt[:, :, tt * 128:(tt + 1) * 128]
                    pk = ps.tile([128, E], F32, tag="mm")
                    pv = ps.tile([128, E], F32, tag="mm")
                    for k in range(KE):
                        mm(pk, xs[:, k, :], wkvt[:, k, 0:E],
                           start=(k == 0), stop=False)
                        mm(pv, xs[:, k, :], wkvt[:, k, E:2 * E],
                           start=(k == 0), stop=False)
                    mm(pk, ONES_ROW, ubkv_row[:, 0:E], start=False, stop=True)
                    mm(pv, ONES_ROW, ubkv_row[:, E:2 * E], start=False,
                       stop=True)
                    ee = pc.tile([128, E], BF16, tag="a1")
                    rr = pc.tile([128, E], BF16, tag="a1")
                    sc.activation(ee, pk, AF.Exp)
                    sc.activation(rr, pk, AF.Relu)
                    v.scalar_tensor_tensor(out=k2f[:, tt, :], in0=ee,
                                           scalar=1.0, in1=rr,
                                           op0=ALU.min, op1=ALU.add)
                    v.tensor_copy(v2x[:, tt, 0, 0:256], pv[:, 0:256])
                    v.tensor_copy(v2x[:, tt, 1, 0:256], pv[:, 256:512])
                yield "a1"
                for p in range(4):
                    pkv = ps.tile([128, 258], F32, tag="mm")
                    for tt in range(NTT):
                        mm(pkv, k2f[:, tt, p * 128:(p + 1) * 128],
                           v2x[:, tt, p // 2, :],
                           start=(tt == 0), stop=(tt == NTT - 1))
                    if ci == 0:
                        sc.activation(kv_acc[:, p, :], pkv, AF.Copy)
                    else:
                        v.tensor_tensor(out=kv_acc[:, p, :],
                                        in0=kv_acc[:, p, :], in1=pkv,
                                        op=ALU.add)
                yield "a2"

            bd = pat.tile([128, KE, 128], BF16, tag="bd", bufs=2)
            v.memset(bd, 0.0)
            for p in range(4):
                h0c = (2 * p % 4) * 64
                h1c = ((2 * p + 1) % 4) * 64
                v.tensor_copy(bd[0:64, p, 0:64], kv_acc[0:64, p, h0c:h0c + 64])
                v.tensor_copy(bd[64:128, p, 64:128],
                              kv_acc[64:128, p, h1c:h1c + 64])
            kmm = pat.tile([128, KE, H], BF16, tag="km")
            for k in range(KE):
                v.tensor_scalar_mul(kmm[:, k, :], hmask_t[:, k, :],
                                    kv_acc[:, k, 256:257])
            yield "bd"

            PHASES.append((f"attn{l}.{a}.beta", len(nc.inst_map)))
            qpf = [None] * NC
            qpf[0] = load_x_chunk(xq_dram, 0)
            for ci in range(NC):
                if ci + 1 < NC:
                    qpf[ci + 1] = load_x_chunk(xq_dram, ci + 1)
                xq = qpf[ci]
                qpf[ci] = None
                qf = pa.tile([128, KE, C], BF16, tag="a4")
                for m in range(KE):
                    pq = ps.tile([128, C], F32, tag="mm")
                    for k in range(KE):
                        mm(pq, wqt[:, k, m * 128:(m + 1) * 128], xq[:, k, :],
                           start=(k == 0), stop=(k == KE - 1))
                    ee = pc.tile([128, C], BF16, tag="a1")
                    rr = pc.tile([128, C], BF16, tag="a1")
                    sc.activation(ee, pq, AF.Exp, bias=ubq_col[:, m:m + 1])
                    sc.activation(rr, pq, AF.Relu, bias=ubq_col[:, m:m + 1])
                    v.scalar_tensor_tensor(out=qf[:, m, :], in0=ee, scalar=1.0,
                                           in1=rr, op0=ALU.min, op1=ALU.add)
                yield "b1"
                pd = psr.tile([8, C], F32, tag="row")
                for k in range(KE):
                    mm(pd, kmm[:, k, :], qf[:, k, :], start=(k == 0),
                       stop=(k == KE - 1))
                rec = pc.tile([8, C], BF16, tag="a1")
                v.reciprocal(out=rec, in_=pd)
                att = pa.tile([128, KE, C], BF16, tag="a4")
                for m in range(KE):
                    pn = ps.tile([128, C], F32, tag="mm")
                    mm(pn, bd[:, m, :], qf[:, m, :], start=True, stop=True)
                    pr = ps.tile([128, C], F32, tag="mm")
                    mm(pr, cmask_t[:, m, :], rec, start=True, stop=True)
                    rb = pc.tile([128, C], BF16, tag="a1")
                    sc.activation(rb, pr, AF.Copy)
                    v.tensor_tensor(out=att[:, m, :], in0=pn, in1=rb,
                                    op=ALU.mult)
                yield "b2"
                for m in range(KE):
                    pos = ps.tile([128, C], F32, tag="mm")
                    for k in range(KE):
                        mm(pos, owt[:, k, m * 128:(m + 1) * 128],
                           att[:, k, :], start=(k == 0), stop=(k == KE - 1))
                    tail_m(ci, m, pos, xq, ob_col)
                tail_post(ci, xq)
                yield "b3"

        # ---- tails ----
        def make_self_tail(l, s, dst):
            g_col = col_tile(lng[l, 0 if s == "b" else 1], KE, tag="lncol", bufs=16)
            b_col = col_tile(lnb[l, 0 if s == "b" else 1], KE, tag="lncol", bufs=16)
            rt_box = [None]

            def tail_m(ci, m, pos, xq, ob_col):
                if m == 0:
                    rt_box[0] = pa.tile([128, KE, C], BF16, tag="a4",
                                        name="rt")
                v.scalar_tensor_tensor(out=rt_box[0][:, m, :], in0=pos,
                                       scalar=ob_col[:, m:m + 1],
                                       in1=xq[:, m, :],
                                       op0=ALU.add, op1=ALU.add)

            def tail_post(ci, xq):
                rt = rt_box[0]
                outt = pa.tile([128, KE, C], BF16, tag="a4")
                ln_apply(rt, g_col, b_col, outt)
                store_chunk(dst, ci, outt)

            return tail_m, tail_post

        def make_cross_tail(l, s, dst):
            gw1t = wsm.tile([128, 2 * KE, E4], BF16, tag="ws")
            nc.sync.dma_start(out=gw1t,
                              in_=gw1[l].rearrange("(k p) g -> p k g", p=128))
            gwd_col = wcol.tile([128, 1], BF16, tag="gwd")
            nc.sync.dma_start(out=gwd_col, in_=gwd[l][:, None])
            gb1_col = col_tile(gb1[l], 1, tag="lncol", bufs=16)
            gb2d_t = pat.tile([1, 1], F32, tag="gb2d")
            nc.sync.dma_start(out=gb2d_t, in_=gb2d[l][None, :])
            g_col = col_tile(lng[l, 2], KE, tag="lncol", bufs=16)
            b_col = col_tile(lnb[l, 2], KE, tag="lncol", bufs=16)
            proj_box = [None]

            def tail_m(ci, m, pos, xq, ob_col):
                if m == 0:
                    proj_box[0] = pa.tile([128, KE, C], BF16, tag="a4",
                                          name="proj")
                sc.activation(proj_box[0][:, m, :], pos, AF.Identity,
                              bias=ob_col[:, m:m + 1])

            def tail_post(ci, xq):
                proj = proj_box[0]
                pg = ps.tile([128, C], F32, tag="mm")
                for k in range(2 * KE):
                    rhs = xq[:, k, :] if k < KE else proj[:, k - KE, :]
                    mm(pg, gw1t[:, k, :], rhs, start=(k == 0),
                       stop=(k == 2 * KE - 1))
                g1 = pc.tile([128, C], BF16, tag="a1")
                sc.activation(g1, pg, AF.Relu, bias=gb1_col[:, 0:1])
                g1t = pc.tile([128, C], BF16, tag="a1")
                v.tensor_scalar_min(g1t, g1, 6.0)
                pg2 = psr.tile([1, C], F32, tag="row")
                mm(pg2, gwd_col, g1t, start=True, stop=True)
                bg = pc.tile([1, C], BF16, tag="rowh", bufs=8)
                sc.activation(bg, pg2, AF.Sigmoid, bias=gb2d_t[0:1, 0:1])
                bgb = pc.tile([128, C], BF16, tag="a1")
                gp.partition_broadcast(bgb, bg)
                mt = pa.tile([128, KE, C], BF16, tag="a4")
                for m in range(KE):
                    dtmp = pc.tile([128, C], BF16, tag="a1")
                    v.tensor_tensor(out=dtmp, in0=xq[:, m, :],
                                    in1=proj[:, m, :], op=ALU.subtract)
                    v.tensor_tensor(out=dtmp, in0=dtmp, in1=bgb, op=ALU.mult)
                    v.tensor_tensor(out=mt[:, m, :], in0=dtmp,
                                    in1=proj[:, m, :], op=ALU.add)
                outt = pa.tile([128, KE, C], BF16, tag="a4")
                ln_apply(mt, g_col, b_col, outt)
                store_chunk(dst, ci, outt)

            return tail_m, tail_post

        # ---- FFN ----
        def ffn_gen(l, s, src, dst):
            si = 0 if s == "b" else 1
            w1t = wbig.tile([128, KE, X], BF16, tag="w2x", bufs=4)
            nc.sync.dma_start(
                out=w1t, in_=w1[l, si].rearrange("(k p) x -> p k x", p=128))
            w2t = wbig.tile([128, KX, E], BF16, tag="w2x", bufs=4)
            nc.sync.dma_start(
                out=w2t, in_=w2[l, si].rearrange("(k p) e -> p k e", p=128))
            b1_col = col_tile(b1f[l, si], KX, tag="ffcol", bufs=16)
            b2_col = col_tile(b2f[l, si], KE, tag="ffcol", bufs=16)
            w0_col = col_tile(cwf[l, si, 0], KX, tag="ffcol", bufs=16)
            w1c_col = col_tile(cwf[l, si, 1], KX, tag="ffcol", bufs=16)
            w2_col = col_tile(cwf[l, si, 2], KX, tag="ffcol", bufs=16)
            A_col = col_tile(bnA[l, si], KX, tag="ffcol", bufs=16)
            B_col = col_tile(bnB[l, si], KX, tag="ffcol", bufs=16)
            g_col = col_tile(lng[l, 3 if s == "b" else 4], KE, tag="lncol", bufs=16)
            bb_col = col_tile(lnb[l, 3 if s == "b" else 4], KE, tag="lncol", bufs=16)
            yield
            PHASES.append((f"ffn{l}.{s}", len(nc.inst_map)))

            hts = [None] * NC
            xts = [None] * NC
            hl0 = [None] * NC
            hf2 = [None] * NC

            fpf = [None] * NC

            def compute_h(ci):
                if fpf[ci] is None:
                    fpf[ci] = load_x_chunk(src, ci)
                if ci + 1 < NC:
                    fpf[ci + 1] = load_x_chunk(src, ci + 1)
                xt = fpf[ci]
                xts[ci] = xt
                ht = pb.tile([128, KX, C], BF16, tag="a8")
                for m in range(KX):
                    ph = ps.tile([128, C], F32, tag="mm")
                    for k in range(KE):
                        mm(ph, w1t[:, k, m * 128:(m + 1) * 128], xt[:, k, :],
                           start=(k == 0), stop=(k == KE - 1))
                    hf = pc.tile([128, C], BF16, tag="a1")
                    sc.activation(hf, ph, AF.Relu, bias=b1_col[:, m:m + 1])
                    v.tensor_scalar_min(ht[:, m, :], hf, 6.0)
                hts[ci] = ht
                l0 = phl.tile([128, KX, 1], BF16, tag="hl")
                f2 = phl.tile([128, KX, 1], BF16, tag="hf")
                for m in range(KX):
                    v.tensor_scalar_mul(l0[:, m, :], ht[:, m, C - 1:C],
                                        w0_col[:, m:m + 1])
                    v.tensor_scalar_mul(f2[:, m, :], ht[:, m, 0:1],
                                        w2_col[:, m:m + 1])
                hl0[ci], hf2[ci] = l0, f2

            def conv_elem(ci):
                ht = hts[ci]
                h2 = pb.tile([128, KX, C], BF16, tag="a8")
                for m in range(KX):
                    acc = pc.tile([128, C], BF16, tag="a1")
                    v.tensor_scalar_mul(acc, ht[:, m, :], w1c_col[:, m:m + 1])
                    v.scalar_tensor_tensor(out=acc[:, 1:C],
                                           in0=ht[:, m, 0:C - 1],
                                           scalar=w0_col[:, m:m + 1],
                                           in1=acc[:, 1:C],
                                           op0=ALU.mult, op1=ALU.add)
                    if ci > 0:
                        v.tensor_tensor(out=acc[:, 0:1], in0=acc[:, 0:1],
                                        in1=hl0[ci - 1][:, m, :], op=ALU.add)
                    v.scalar_tensor_tensor(out=acc[:, 0:C - 1],
                                           in0=ht[:, m, 1:C],
                                           scalar=w2_col[:, m:m + 1],
                                           in1=acc[:, 0:C - 1],
                                           op0=ALU.mult, op1=ALU.add)
                    if ci < NC - 1:
                        v.tensor_tensor(out=acc[:, C - 1:C],
                                        in0=acc[:, C - 1:C],
                                        in1=hf2[ci + 1][:, m, :], op=ALU.add)
                    a2 = pc.tile([128, C], BF16, tag="a1")
                    sc.activation(a2, acc, AF.Relu, scale=A_col[:, m:m + 1],
                                  bias=B_col[:, m:m + 1])
                    v.tensor_scalar_min(h2[:, m, :], a2, 6.0)
                return h2

            def conv_pw(ci, h2):
                rt = pa.tile([128, KE, C], BF16, tag="a4")
                for m in range(KE):
                    pw = ps.tile([128, C], F32, tag="mm")
                    for k in range(KX):
                        mm(pw, w2t[:, k, m * 128:(m + 1) * 128], h2[:, k, :],
                           start=(k == 0), stop=(k == KX - 1))
                    v.scalar_tensor_tensor(out=rt[:, m, :], in0=pw,
                                           scalar=b2_col[:, m:m + 1],
                                           in1=xts[ci][:, m, :],
                                           op0=ALU.add, op1=ALU.add)
                outt = pa.tile([128, KE, C], BF16, tag="a4")
                ln_apply(rt, g_col, bb_col, outt)
                store_chunk(dst, ci, outt)
                hts[ci] = xts[ci] = None

            compute_h(0)
            yield "h"
            compute_h(1)
            yield "h"
            for ci in range(NC - 1):
                h2 = conv_elem(ci)
                yield "cv"
                conv_pw(ci, h2)
                yield "pw"
                if ci + 2 < NC:
                    compute_h(ci + 2)
                    yield "h"
            h2 = conv_elem(NC - 1)
            yield "cv"
            conv_pw(NC - 1, h2)
            yield "pw"

        # ---- layers (chunk-interleaved across independent streams) ----
        def adv(g, n=1):
            for _ in range(n):
                next(g, None)

        for l in range(L):
            bsrc = rs["b", 0] if l == 0 else rs["b", (l - 1, 3)]
            lsrc = rs["l", 0] if l == 0 else rs["l", (l - 1, 3)]
            g0 = attn_gen(l, 0, bsrc, bsrc,
                          *make_self_tail(l, "b", rs["b", (l, 1)]))
            g1 = attn_gen(l, 1, lsrc, lsrc,
                          *make_self_tail(l, "l", rs["l", (l, 1)]))
            g2 = attn_gen(l, 2, rs["b", (l, 1)], rs["l", (l, 1)],
                          *make_cross_tail(l, "b", rs["b", (l, 2)]))
            g3 = attn_gen(l, 3, rs["l", (l, 1)], rs["b", (l, 1)],
                          *make_cross_tail(l, "l", rs["l", (l, 2)]))
            gb = ffn_gen(l, "b", rs["b", (l, 2)], rs["b", (l, 3)])
            gl = ffn_gen(l, "l", rs["l", (l, 2)], rs["l", (l, 3)])
            adv(g0)                     # weights
            adv(g1)
            for _ in range(2 * NC):     # self alphas: a1/a2 staged
                adv(g0)
                adv(g1)
            adv(g0)                     # bd/kmm
            adv(g1)
            adv(g2)                     # prefetch cross weights
            adv(g3)
            for _ in range(3 * NC):     # self betas: b1/b2/b3 staged
                adv(g0)
                adv(g1)
            for _ in range(2 * NC):     # cross alphas staged
                adv(g2)
                adv(g3)
            adv(g2)                     # bd/kmm
            adv(g3)
            adv(gb)                     # prefetch ffn weights
            adv(gl)
            for _ in range(3 * NC):     # cross betas staged
                adv(g2)
                adv(g3)
            for _ in range(12):         # ffn h/cv/pw staged
                adv(gb)
                adv(gl)

        PHASES.append(("final", len(nc.inst_map)))
        # ---- final head ----
        fw1t = wbig.tile([128, 2 * KE, E2], BF16, tag="w", bufs=6)
        nc.sync.dma_start(out=fw1t,
                          in_=fw1.rearrange("(k p) g -> p k g", p=128))
        fw2t = wsm.tile([128, 2, E], BF16, tag="wfin", bufs=4)
        nc.sync.dma_start(out=fw2t,
                          in_=fw2.rearrange("(k p) e -> p k e", p=128))
        rw1t = wsm.tile([128, KE, E4], BF16, tag="wfin", bufs=4)
        nc.sync.dma_start(out=rw1t,
                          in_=rw1.rearrange("(k p) g -> p k g", p=128))
        rw2t = wsm.tile([128, E8], BF16, tag="wfin", bufs=4)
        nc.sync.dma_start(out=rw2t, in_=rw2)
        rw3t = wsm.tile([E8, 16], BF16, tag="wfin", bufs=4)
        nc.sync.dma_start(out=rw3t, in_=rw3p)
        rb3_row = row_tile(rb3p, 16)
        fb2_col = col_tile(fb2f, KE, tag="fcol")
        fb1_col = col_tile(fb1, 2, tag="fcol")
        flng_col = col_tile(flng, KE, tag="fcol")
        flnb_col = col_tile(flnb, KE, tag="fcol")
        rb1_col = col_tile(rb1, 1, tag="fcol")
        rb2_col = wcol.tile([E8, 1], F32, tag="fcol")
        nc.sync.dma_start(out=rb2_col, in_=rb2[:, None])
        out_ap = out_dram.ap()

        bsrc, lsrc = rs["b", (L - 1, 3)], rs["l", (L - 1, 3)]

        def final_stage1(ci):
            xb = load_x_chunk(bsrc, ci)
            xl = load_x_chunk(lsrc, ci)
            f1t = pa.tile([128, 2, C], BF16, tag="a4")
            for m in range(2):
                pf = ps.tile([128, C], F32, tag="mm")
                for k in range(2 * KE):
                    rhs = xb[:, k, :] if k < KE else xl[:, k - KE, :]
                    mm(pf, fw1t[:, k, m * 128:(m + 1) * 128], rhs,
                       start=(k == 0), stop=(k == 2 * KE - 1))
                f1 = pc.tile([128, C], BF16, tag="a1")
                sc.activation(f1, pf, AF.Relu, bias=fb1_col[:, m:m + 1])
                v.tensor_scalar_min(f1t[:, m, :], f1, 6.0)
            ft = pa.tile([128, KE, C], BF16, tag="a4")
            for m in range(KE):
                pf2 = ps.tile([128, C], F32, tag="mm")
                for k in range(2):
                    mm(pf2, fw2t[:, k, m * 128:(m + 1) * 128], f1t[:, k, :],
                       start=(k == 0), stop=(k == 1))
                sc.activation(ft[:, m, :], pf2, AF.Identity,
                              bias=fb2_col[:, m:m + 1])
            frt = pa.tile([128, KE, C], BF16, tag="a4")
            ln_apply(ft, flng_col, flnb_col, frt)
            # relu after LN
            frf = frt.rearrange("p k c -> p (k c)")
            v.tensor_scalar_max(frf, frf, 0.0)
            return frt

        def final_stage2(ci, frt):
            p1 = ps.tile([128, C], F32, tag="mm")
            for k in range(KE):
                mm(p1, rw1t[:, k, :], frt[:, k, :], start=(k == 0),
                   stop=(k == KE - 1))
            h1f = pc.tile([128, C], BF16, tag="a1")
            sc.activation(h1f, p1, AF.Relu, bias=rb1_col[:, 0:1])
            h1t = pc.tile([128, C], BF16, tag="a1")
            v.tensor_scalar_min(h1t, h1f, 6.0)
            p2 = ps.tile([E8, C], F32, tag="mm")
            mm(p2, rw2t, h1t, start=True, stop=True)
            h2f = pc.tile([E8, C], BF16, tag="a1")
            sc.activation(h2f, p2, AF.Relu, bias=rb2_col[:, 0:1])
            h2t = pc.tile([E8, C], BF16, tag="a1")
            v.tensor_scalar_min(h2t, h2f, 6.0)
            ot = pc.tile([128, NTT, c.OUT], F32, tag="a1")
            for tt in range(NTT):
                p3 = ps.tile([128, 16], F32, tag="mm")
                mm(p3, h2t[:, tt * 128:(tt + 1) * 128], rw3t,
                   start=True, stop=False)
                mm(p3, ONES_ROW, rb3_row, start=False, stop=True)
                sc.activation(ot[:, tt, :], p3[:, 0:c.OUT], AF.Copy)
            nc.sync.dma_start(
                out=out_ap[ci * C:(ci + 1) * C, :].rearrange(
                    "(tt p) o -> p tt o", p=128),
                in_=ot)

        frts = [None] * NC
        for ci in range(NC):
            frts[ci] = final_stage1(ci)
            if ci >= 1:
                final_stage2(ci - 1, frts[ci - 1])
        final_stage2(NC - 1, frts[NC - 1])

    return din, out_dram


# ======================================================================
# kernel() entry point: full inputs in, full outputs out (8-core SPMD).
# ======================================================================
import concourse.bacc as _bacc
from concourse.bass_utils import run_bass_kernel_spmd as _run_spmd

_N_CORES = 8
_CACHE = {}


def _get_nc():
    if "nc" not in _CACHE:
        nc = _bacc.Bacc("TRN2", target_bir_lowering=False, debug=False)
        build(nc, Cfg())
        nc.finalize()
        _CACHE["nc"] = nc
    return _CACHE["nc"]


def _bf16(x):
    import ml_dtypes
    return np.asarray(x, dtype=np.float32).astype(ml_dtypes.bfloat16)


def host_prep(inputs):
    """Host-side weight preprocessing: compose QKV, fold BN, convert bf16."""
    c = Cfg()
    E, X, H, L = c.E, c.X, c.H, c.L
    E4, E2, E8 = E // 4, E // 2, E // 8
    f = {k: np.asarray(v, dtype=np.float32) for k, v in inputs.items()}
    dw, uw, ub = f["dw"], f["uw"], f["ub"]
    # composed q and k|v projection weights
    wq = np.matmul(dw[:, :, 0], uw[:, :, 0])          # (L,4,E,E)
    wk = np.matmul(dw[:, :, 1], uw[:, :, 1])
    wv = np.matmul(dw[:, :, 2], uw[:, :, 2])
    wkv = np.concatenate([wk, wv], axis=-1)           # (L,4,E,2E)
    ubq = ub[:, :, 0]                                 # (L,4,E)
    ubkv = np.concatenate([ub[:, :, 1], ub[:, :, 2]], axis=-1)
    rsq = np.float32(1.0 / np.sqrt(1.0 + BN_EPS))
    A = f["bng"] * rsq                                # (L,2,X)
    B = f["cb"] * A + f["bnb"]
    cwf = f["cw"].transpose(0, 1, 3, 2).copy()        # (L,2,3,X)
    gwd = f["gw2"][:, :, 0] - f["gw2"][:, :, 1]       # (L,E4)
    gb2d = (f["gb2"][:, 0] - f["gb2"][:, 1])[:, None]  # (L,1)
    rw3p = np.zeros((E8, 16), np.float32)
    rw3p[:, :c.OUT] = f["rw3"]
    rb3p = np.zeros((16,), np.float32)
    rb3p[:c.OUT] = f["rb3"]
    dh = E // H
    ident = np.eye(128, dtype=np.float32)
    ones = np.ones((128, 128), dtype=np.float32)
    hmask = np.zeros((E, H), dtype=np.float32)
    for ff in range(E):
        hmask[ff, ff // dh] = 1.0
    cmask = hmask.T.copy()

    b16 = dict(wq=wq, wkv=wkv, ubkv=ubkv, ow=f["ow"],
               w1=f["w1"], w2=f["w2"],
               gw1=f["gw1"], gwd=gwd, fw1=f["fw1"], fw2=f["fw2"],
               rw1=f["rw1"], rw2=f["rw2"], rw3p=rw3p,
               rb3p=rb3p, ident=ident, ones128=ones, hmask=hmask,
               cmask=cmask)
    f32 = dict(ubq=ubq, ob=f["ob"], b1=f["b1"], b2=f["b2"], fb2=f["fb2"],
               cwf=cwf, bnA=A, bnB=B,
               lng=f["lng"], lnb=f["lnb"],
               gb1=f["gb1"], gb2d=gb2d, fb1=f["fb1"], flng=f["flng"],
               flnb=f["flnb"], rb1=f["rb1"], rb2=f["rb2"])
    shared = {k: _bf16(v) for k, v in b16.items()}
    shared.update({k: np.ascontiguousarray(v, dtype=np.float32)
                   for k, v in f32.items()})
    return shared, f["body_feats"], f["limb_feats"]


def kernel(**inputs):
    nc = _get_nc()
    shared, body, limb = host_prep(inputs)
    in_maps = []
    for i in range(_N_CORES):
        m = dict(shared)
        m["body_feats"] = _bf16(body[i])
        m["limb_feats"] = _bf16(limb[i])
        in_maps.append(m)
    res = run_kernel_spmd_cached(nc, in_maps)
    out = np.stack([res[i]["out"] for i in range(_N_CORES)], axis=0)
    return out.astype(np.float32)


def run_kernel_spmd_cached(nc, in_maps, **kw):
    r = _run_spmd(nc, in_maps, list(range(_N_CORES)), **kw)
    _CACHE["last_result"] = r
    return r.results


# revision 9
# speedup vs baseline: 1.2229x; 1.0351x over previous
"""Dual-stream linear-attention transformer — bf16 redesign (per-core).

Layout convention (same as baseline):
  - "layout 1" activation: [E, N] feature-major; SBUF tiles [128, KE, C]
    (feature f = 128*k + p -> partition p, k-th slice; tokens on free dim).
  - alpha k/v are produced token-major per 128-token tile [128, E].
  - Residual streams live in internal DRAM as [E, N] bf16.

Key changes vs baseline:
  - All matmul operands + SBUF activations bf16 (same PE rate as f32r>=256,
    but DVE tensor_tensor 2x / tensor_scalar 4x, half DMA bytes).
  - QKV down+up projections composed into single E x E / E x 2E weights on
    the host (removes the low-rank intermediate copies).
  - Biases folded into matmuls as rank-1 accumulates (ones_row x bias_row).
  - elu+1 via 3 ops: ACT Exp, ACT Relu, DVE scalar_tensor_tensor(min,add).
  - LayerNorm apply via 3 bf16 DVE ops per slice (TT,TT,TS) instead of
    gpsimd tensor_tensor pairs.
  - FFN dwconv via TS/STT chain on DVE; BN folded on host into A,B.
  - relu6 of FFN h on gpsimd (idle engine) straight from PSUM.
  - m-outer matmul groups -> 1 PSUM bank live per group, fewer stalls.
"""

from dataclasses import dataclass
from contextlib import ExitStack

import numpy as np

import concourse.bass as bass
import concourse.mybir as mybir
import concourse.tile as tile

F32 = mybir.dt.float32
BF16 = mybir.dt.bfloat16
AF = mybir.ActivationFunctionType
ALU = mybir.AluOpType

LN_EPS = 1e-5
BN_EPS = 1e-5


@dataclass
class Cfg:
    N: int = 2048
    E: int = 512
    R: int = 256
    X: int = 1024
    H: int = 8
    L: int = 3
    OUT: int = 15
    C: int = 512

    @property
    def KE(self):
        return self.E // 128

    @property
    def KX(self):
        return self.X // 128

    @property
    def NC(self):
        return self.N // self.C

    @property
    def NTT(self):
        return self.C // 128


PHASES = []


def build(nc, cfg):
    c = cfg
    E, X, H, N, C, L = c.E, c.X, c.H, c.N, c.C, c.L
    KE, KX, NC, NTT = c.KE, c.KX, c.NC, c.NTT
    E4, E2, E8 = E // 4, E // 2, E // 8

    din = {}

    def inp(name, shape, dt=BF16):
        din[name] = nc.dram_tensor(name, list(shape), dt, kind="ExternalInput")
        return din[name].ap()

    # activations (host converts to bf16)
    body_feats = inp("body_feats", (N, E))
    limb_feats = inp("limb_feats", (N, E))
    # attention weights (host-composed)
    wq = inp("wq", (L, 4, E, E))
    wkv = inp("wkv", (L, 4, E, 2 * E))
    ubq = inp("ubq", (L, 4, E), F32)
    ubkv = inp("ubkv", (L, 4, 2 * E))
    ow = inp("ow", (L, 4, E, E))
    obf = inp("ob", (L, 4, E), F32)
    # FFN
    w1 = inp("w1", (L, 2, E, X))
    b1f = inp("b1", (L, 2, X), F32)
    cwf = inp("cwf", (L, 2, 3, X), F32)     # conv taps, tap-major
    bnA = inp("bnA", (L, 2, X), F32)        # bng*rsqrt(1+eps)
    bnB = inp("bnB", (L, 2, X), F32)        # cb*A + bnb
    w2 = inp("w2", (L, 2, X, E))
    b2f = inp("b2", (L, 2, E), F32)
    lng = inp("lng", (L, 5, E), F32)
    lnb = inp("lnb", (L, 5, E), F32)
    # gating
    gw1 = inp("gw1", (L, 2 * E, E4))
    gb1 = inp("gb1", (L, E4), F32)
    gwd = inp("gwd", (L, E4))               # gw2[:,0]-gw2[:,1]
    gb2d = inp("gb2d", (L, 1), F32)         # gb2[0]-gb2[1]
    # final head
    fw1 = inp("fw1", (2 * E, E2))
    fb1 = inp("fb1", (E2,), F32)
    fw2 = inp("fw2", (E2, E))
    fb2f = inp("fb2", (E,), F32)
    flng = inp("flng", (E,), F32)
    flnb = inp("flnb", (E,), F32)
    rw1 = inp("rw1", (E, E4))
    rb1 = inp("rb1", (E4,), F32)
    rw2 = inp("rw2", (E4, E8))
    rb2 = inp("rb2", (E8,), F32)
    rw3p = inp("rw3p", (E8, 16))            # zero-padded to 16
    rb3p = inp("rb3p", (16,))               # zero-padded
    ident_in = inp("ident", (128, 128))
    ones_in = inp("ones128", (128, 128))
    hmask_in = inp("hmask", (E, H))
    cmask_in = inp("cmask", (H, E))

    out_dram = nc.dram_tensor("out", [N, c.OUT], F32, kind="ExternalOutput")

    def idram(name):
        return nc.dram_tensor(name, [E, N], BF16).ap().rearrange(
            "(k p) n -> p k n", p=128)

    rs = {}
    for s in ("b", "l"):
        rs[s, 0] = idram(f"r{s}0")
        for l in range(L):
            for st in (1, 2, 3):
                rs[s, (l, st)] = idram(f"r{s}_{l}_{st}")

    lowp = nc.allow_low_precision("bf16 activations within rel-err budget")

    with tile.TileContext(nc) as tc, ExitStack() as ctx, lowp:
        p_ = ctx.enter_context
        cst = p_(tc.tile_pool(name="cst", bufs=1))
        wbig = p_(tc.tile_pool(name="wbig", bufs=3))
        wsm = p_(tc.tile_pool(name="wsm", bufs=2))
        wcol = p_(tc.tile_pool(name="wcol", bufs=10))
        wrow = p_(tc.tile_pool(name="wrow", bufs=6))
        pa = p_(tc.tile_pool(name="pa", bufs=9))      # 4KB bf16 chunk tiles
        pb = p_(tc.tile_pool(name="pb", bufs=5))      # 8KB ht tiles
        pc = p_(tc.tile_pool(name="pc", bufs=12))      # 1KB bf16 / rows
        pat = p_(tc.tile_pool(name="pat", bufs=3))    # per-attn persistents
        phl = p_(tc.tile_pool(name="phl", bufs=6))    # conv halos
        ps = p_(tc.tile_pool(name="ps", bufs=6, space="PSUM"))
        psr = p_(tc.tile_pool(name="psr", bufs=2, space="PSUM"))

        v, sc, gp = nc.vector, nc.scalar, nc.gpsimd

        def mm(out, lhsT, rhs, start, stop):
            nc.tensor.matmul(out, lhsT, rhs, start=start, stop=stop)

        # ---- constants ----
        ident_t = cst.tile([128, 128], BF16, tag="ident")
        nc.sync.dma_start(out=ident_t, in_=ident_in)
        ones_t = cst.tile([128, 128], BF16, tag="ones")
        nc.sync.dma_start(out=ones_t, in_=ones_in)
        hmask_t = cst.tile([128, KE, H], BF16, tag="hmask")
        nc.sync.dma_start(out=hmask_t,
                          in_=hmask_in.rearrange("(k p) h -> p k h", p=128))
        cmask_t = cst.tile([H, KE, 128], BF16, tag="cmask")
        nc.sync.dma_start(out=cmask_t,
                          in_=cmask_in.rearrange("h (k p) -> h k p", p=128))
        ONES_COL = ones_t[:, 0:1]
        ONES_ROW = ones_t[0:1, :]
        onesc_t = cst.tile([1, C], BF16, tag="onesc")
        v.memset(onesc_t, 1.0)
        ONES_C = onesc_t[0:1, :]
        eps_ln = cst.tile([1, 1], F32, tag="epsl")
        v.memset(eps_ln, LN_EPS)

        def col_tile(src_ap, m, tag="col", bufs=None):
            t = wcol.tile([128, m], F32, tag=tag,
                          bufs=(12 if tag == "col" else bufs))
            nc.sync.dma_start(out=t, in_=src_ap.rearrange("(m p) -> p m", p=128))
            return t

        def row_tile(src_ap, n, tag="row", pool=None):
            t = (pool or wrow).tile([1, n], BF16, tag=tag)
            nc.sync.dma_start(out=t, in_=src_ap[None, :])
            return t

        def ln_apply(xs, g_col, b_col, outt, extra_tt=None):
            """LayerNorm over features (layout 1). xs: [128, KE, C] bf16 tile.
            outt: [128, KE, C] bf16 out. extra_tt(m): None."""
            sq = pa.tile([128, KE, C], BF16, tag="a4")
            xf = xs.rearrange("p k c -> p (k c)")
            v.tensor_tensor(out=sq.rearrange("p k c -> p (k c)"),
                            in0=xf, in1=xf, op=ALU.mult)
            ps_s = psr.tile([1, C], F32, tag="row")
            ps_ss = psr.tile([1, C], F32, tag="row")
            for m in range(KE):
                mm(ps_s, ONES_COL, xs[:, m, :], start=(m == 0),
                   stop=(m == KE - 1))
                mm(ps_ss, ONES_COL, sq[:, m, :], start=(m == 0),
                   stop=(m == KE - 1))
            msq = pc.tile([1, C], F32, tag="row", bufs=8)
            sc.activation(msq, ps_ss, AF.Copy, scale=1.0 / E)
            m2 = pc.tile([1, C], F32, tag="row", bufs=8)
            sc.activation(m2, ps_s, AF.Square, scale=1.0 / E)
            var = pc.tile([1, C], F32, tag="row", bufs=8)
            v.tensor_tensor(out=var, in0=msq, in1=m2, op=ALU.subtract)
            srow = pc.tile([1, C], BF16, tag="rowh", bufs=8)
            sc.activation(srow, var, AF.Abs_reciprocal_sqrt,
                          bias=eps_ln[0:1, 0:1])
            trow = pc.tile([1, C], BF16, tag="rowh", bufs=8)
            v.scalar_tensor_tensor(out=trow, in0=ps_s, scalar=1.0 / E,
                                   in1=srow, op0=ALU.mult, op1=ALU.mult)
            sb_s = pc.tile([128, C], BF16, tag="a1")
            gp.partition_broadcast(sb_s, srow)
            sb_t = pc.tile([128, C], BF16, tag="a1")
            gp.partition_broadcast(sb_t, trow)
            for m in range(KE):
                u = pc.tile([128, C], BF16, tag="a1")
                v.tensor_tensor(out=u, in0=xs[:, m, :], in1=sb_s, op=ALU.mult)
                v.tensor_tensor(out=u, in0=u, in1=sb_t, op=ALU.subtract)
                v.tensor_scalar(out=outt[:, m, :], in0=u,
                                scalar1=g_col[:, m:m + 1],
                                scalar2=b_col[:, m:m + 1],
                                op0=ALU.mult, op1=ALU.add)

        def load_x_chunk(dram_l1, ci, tag="a4"):
            xt = pa.tile([128, KE, C], BF16, tag=tag)
            nc.sync.dma_start(out=xt, in_=dram_l1[:, :, ci * C:(ci + 1) * C])
            return xt

        def store_chunk(dram_l1, ci, t):
            gp.dma_start(out=dram_l1[:, :, ci * C:(ci + 1) * C], in_=t)

        # ---- entry transpose ----
        def entry(x_ap, dst):
            for ttk in range(N // 128):
                x2 = pa.tile([128, E], BF16, tag="a4")
                nc.sync.dma_start(out=x2, in_=x_ap[ttk * 128:(ttk + 1) * 128, :])
                xt = pa.tile([128, KE, 128], BF16, tag="a4")
                for f in range(KE):
                    pt = ps.tile([128, 128], BF16, tag="mm")
                    nc.tensor.transpose(pt, x2[:, f * 128:(f + 1) * 128],
                                        ident_t)
                    sc.activation(xt[:, f, :], pt, AF.Copy)
                nc.sync.dma_start(out=dst[:, :, ttk * 128:(ttk + 1) * 128],
                                  in_=xt)

        PHASES.append(("entry", len(nc.inst_map)))
        entry(body_feats, rs["b", 0])
        entry(limb_feats, rs["l", 0])

        # ---- linear attention ----
        def attn_gen(l, a, xq_dram, xkv_dram, tail_m, tail_post):
            """Generator: yields after weight loads, after each alpha chunk
            (bd/kmm ride with the last), and after each beta chunk."""
            wqt = wbig.tile([128, KE, E], BF16, tag="w", bufs=6)
            nc.sync.dma_start(
                out=wqt, in_=wq[l, a].rearrange("(k p) e -> p k e", p=128))
            wkvt = wbig.tile([128, KE, 2 * E], BF16, tag="w2x", bufs=4)
            nc.sync.dma_start(
                out=wkvt, in_=wkv[l, a].rearrange("(k p) e -> p k e", p=128))
            owt = wbig.tile([128, KE, E], BF16, tag="w", bufs=6)
            nc.sync.dma_start(
                out=owt, in_=ow[l, a].rearrange("(k p) e -> p k e", p=128))
            ubq_col = col_tile(ubq[l, a], KE)
            ubkv_row = row_tile(ubkv[l, a], 2 * E)
            ob_col = col_tile(obf[l, a], KE)
            yield

            PHASES.append((f"attn{l}.{a}.alpha", len(nc.inst_map)))
            kv_acc = pat.tile([128, KE, 258], F32, tag="kva", bufs=2)

            xpf = [None] * NC
            xpf[0] = load_x_chunk(xkv_dram, 0)
            for ci in range(NC):
                if ci + 1 < NC:
                    xpf[ci + 1] = load_x_chunk(xkv_dram, ci + 1)
                xt = xpf[ci]
                xpf[ci] = None
                k2f = pa.tile([128, NTT, E], BF16, tag="a4")
                v2x = pa.tile([128, NTT, 2, 258], BF16, tag="a4")
                v.memset(v2x[:, :, :, 256:258], 1.0)
                for tt in range(NTT):
                    xs = xt[:, :, tt * 128:(tt + 1) * 128]
                    pk = ps.tile([128, E], F32, tag="mm")
                    pv = ps.tile([128, E], F32, tag="mm")
                    for k in range(KE):
                        mm(pk, xs[:, k, :], wkvt[:, k, 0:E],
                           start=(k == 0), stop=False)
                        mm(pv, xs[:, k, :], wkvt[:, k, E:2 * E],
                           start=(k == 0), stop=False)
                    mm(pk, ONES_ROW, ubkv_row[:, 0:E], start=False, stop=True)
                    mm(pv, ONES_ROW, ubkv_row[:, E:2 * E], start=False,
                       stop=True)
                    ee = pc.tile([128, E], BF16, tag="a1")
                    rr = pc.tile([128, E], BF16, tag="a1")
                    sc.activation(ee, pk, AF.Exp)
                    sc.activation(rr, pk, AF.Relu)
                    v.scalar_tensor_tensor(out=k2f[:, tt, :], in0=ee,
                                           scalar=1.0, in1=rr,
                                           op0=ALU.min, op1=ALU.add)
                    sc.activation(v2x[:, tt, 0, 0:256], pv[:, 0:256], AF.Copy)
                    sc.activation(v2x[:, tt, 1, 0:256], pv[:, 256:512], AF.Copy)
                yield "a1"
                for p in range(4):
                    pkv = ps.tile([128, 258], F32, tag="mm")
                    for tt in range(NTT):
                        mm(pkv, k2f[:, tt, p * 128:(p + 1) * 128],
                           v2x[:, tt, p // 2, :],
                           start=(tt == 0), stop=(tt == NTT - 1))
                    if ci == 0:
                        sc.activation(kv_acc[:, p, :], pkv, AF.Copy)
                    else:
                        v.tensor_tensor(out=kv_acc[:, p, :],
                                        in0=kv_acc[:, p, :], in1=pkv,
                                        op=ALU.add)
                yield "a2"

            bd = pat.tile([128, KE, 128], BF16, tag="bd", bufs=2)
            v.memset(bd, 0.0)
            for p in range(4):
                h0c = (2 * p % 4) * 64
                h1c = ((2 * p + 1) % 4) * 64
                v.tensor_copy(bd[0:64, p, 0:64], kv_acc[0:64, p, h0c:h0c + 64])
                v.tensor_copy(bd[64:128, p, 64:128],
                              kv_acc[64:128, p, h1c:h1c + 64])
            kmm = pat.tile([128, KE, H], BF16, tag="km")
            for k in range(KE):
                v.tensor_scalar_mul(kmm[:, k, :], hmask_t[:, k, :],
                                    kv_acc[:, k, 256:257])
            yield "bd"

            PHASES.append((f"attn{l}.{a}.beta", len(nc.inst_map)))
            qpf = [None] * NC
            qpf[0] = load_x_chunk(xq_dram, 0)
            for ci in range(NC):
                if ci + 1 < NC:
                    qpf[ci + 1] = load_x_chunk(xq_dram, ci + 1)
                xq = qpf[ci]
                qpf[ci] = None
                qf = pa.tile([128, KE, C], BF16, tag="a4")
                for m in range(KE):
                    pq = ps.tile([128, C], F32, tag="mm")
                    for k in range(KE):
                        mm(pq, wqt[:, k, m * 128:(m + 1) * 128], xq[:, k, :],
                           start=(k == 0), stop=(k == KE - 1))
                    ee = pc.tile([128, C], BF16, tag="a1")
                    rr = pc.tile([128, C], BF16, tag="a1")
                    sc.activation(ee, pq, AF.Exp, bias=ubq_col[:, m:m + 1])
                    sc.activation(rr, pq, AF.Relu, bias=ubq_col[:, m:m + 1])
                    v.scalar_tensor_tensor(out=qf[:, m, :], in0=ee, scalar=1.0,
                                           in1=rr, op0=ALU.min, op1=ALU.add)
                yield "b1"
                pd = psr.tile([8, C], F32, tag="row")
                for k in range(KE):
                    mm(pd, kmm[:, k, :], qf[:, k, :], start=(k == 0),
                       stop=(k == KE - 1))
                rec = pc.tile([8, C], BF16, tag="a1")
                v.reciprocal(out=rec, in_=pd)
                att = pa.tile([128, KE, C], BF16, tag="a4")
                for m in range(KE):
                    pn = ps.tile([128, C], F32, tag="mm")
                    mm(pn, bd[:, m, :], qf[:, m, :], start=True, stop=True)
                    pr = ps.tile([128, C], F32, tag="mm")
                    mm(pr, cmask_t[:, m, :], rec, start=True, stop=True)
                    rb = pc.tile([128, C], BF16, tag="a1")
                    sc.activation(rb, pr, AF.Copy)
                    v.tensor_tensor(out=att[:, m, :], in0=pn, in1=rb,
                                    op=ALU.mult)
                yield "b2"
                for m in range(KE):
                    pos = ps.tile([128, C], F32, tag="mm")
                    for k in range(KE):
                        mm(pos, owt[:, k, m * 128:(m + 1) * 128],
                           att[:, k, :], start=(k == 0), stop=(k == KE - 1))
                    tail_m(ci, m, pos, xq, ob_col)
                tail_post(ci, xq)
                yield "b3"

        # ---- tails ----
        def make_self_tail(l, s, dst):
            g_col = col_tile(lng[l, 0 if s == "b" else 1], KE, tag="lncol", bufs=16)
            b_col = col_tile(lnb[l, 0 if s == "b" else 1], KE, tag="lncol", bufs=16)
            rt_box = [None]

            def tail_m(ci, m, pos, xq, ob_col):
                if m == 0:
                    rt_box[0] = pa.tile([128, KE, C], BF16, tag="a4",
                                        name="rt")
                v.scalar_tensor_tensor(out=rt_box[0][:, m, :], in0=pos,
                                       scalar=ob_col[:, m:m + 1],
                                       in1=xq[:, m, :],
                                       op0=ALU.add, op1=ALU.add)

            def tail_post(ci, xq):
                rt = rt_box[0]
                outt = pa.tile([128, KE, C], BF16, tag="a4")
                ln_apply(rt, g_col, b_col, outt)
                store_chunk(dst, ci, outt)

            return tail_m, tail_post

        def make_cross_tail(l, s, dst):
            gw1t = wsm.tile([128, 2 * KE, E4], BF16, tag="ws")
            nc.sync.dma_start(out=gw1t,
                              in_=gw1[l].rearrange("(k p) g -> p k g", p=128))
            gwd_col = wcol.tile([128, 1], BF16, tag="gwd")
            nc.sync.dma_start(out=gwd_col, in_=gwd[l][:, None])
            gb1_col = col_tile(gb1[l], 1, tag="lncol", bufs=16)
            gb2d_t = pat.tile([1, 1], F32, tag="gb2d")
            nc.sync.dma_start(out=gb2d_t, in_=gb2d[l][None, :])
            g_col = col_tile(lng[l, 2], KE, tag="lncol", bufs=16)
            b_col = col_tile(lnb[l, 2], KE, tag="lncol", bufs=16)
            proj_box = [None]

            def tail_m(ci, m, pos, xq, ob_col):
                if m == 0:
                    proj_box[0] = pa.tile([128, KE, C], BF16, tag="a4",
                                          name="proj")
                sc.activation(proj_box[0][:, m, :], pos, AF.Identity,
                              bias=ob_col[:, m:m + 1])

            def tail_post(ci, xq):
                proj = proj_box[0]
                pg = ps.tile([128, C], F32, tag="mm")
                for k in range(2 * KE):
                    rhs = xq[:, k, :] if k < KE else proj[:, k - KE, :]
                    mm(pg, gw1t[:, k, :], rhs, start=(k == 0),
                       stop=(k == 2 * KE - 1))
                g1 = pc.tile([128, C], BF16, tag="a1")
                sc.activation(g1, pg, AF.Relu, bias=gb1_col[:, 0:1])
                g1t = pc.tile([128, C], BF16, tag="a1")
                v.tensor_scalar_min(g1t, g1, 6.0)
                pg2 = psr.tile([1, C], F32, tag="row")
                mm(pg2, gwd_col, g1t, start=True, stop=True)
                bg = pc.tile([1, C], BF16, tag="rowh", bufs=8)
                sc.activation(bg, pg2, AF.Sigmoid, bias=gb2d_t[0:1, 0:1])
                bgb = pc.tile([128, C], BF16, tag="a1")
                gp.partition_broadcast(bgb, bg)
                mt = pa.tile([128, KE, C], BF16, tag="a4")
                for m in range(KE):
                    dtmp = pc.tile([128, C], BF16, tag="a1")
                    v.tensor_tensor(out=dtmp, in0=xq[:, m, :],
                                    in1=proj[:, m, :], op=ALU.subtract)
                    v.tensor_tensor(out=dtmp, in0=dtmp, in1=bgb, op=ALU.mult)
                    v.tensor_tensor(out=mt[:, m, :], in0=dtmp,
                                    in1=proj[:, m, :], op=ALU.add)
                outt = pa.tile([128, KE, C], BF16, tag="a4")
                ln_apply(mt, g_col, b_col, outt)
                store_chunk(dst, ci, outt)

            return tail_m, tail_post

        # ---- FFN ----
        def ffn_gen(l, s, src, dst):
            si = 0 if s == "b" else 1
            w1t = wbig.tile([128, KE, X], BF16, tag="w2x", bufs=4)
            nc.sync.dma_start(
                out=w1t, in_=w1[l, si].rearrange("(k p) x -> p k x", p=128))
            w2t = wbig.tile([128, KX, E], BF16, tag="w2x", bufs=4)
            nc.sync.dma_start(
                out=w2t, in_=w2[l, si].rearrange("(k p) e -> p k e", p=128))
            b1_col = col_tile(b1f[l, si], KX, tag="ffcol", bufs=16)
            b2_col = col_tile(b2f[l, si], KE, tag="ffcol", bufs=16)
            w0_col = col_tile(cwf[l, si, 0], KX, tag="ffcol", bufs=16)
            w1c_col = col_tile(cwf[l, si, 1], KX, tag="ffcol", bufs=16)
            w2_col = col_tile(cwf[l, si, 2], KX, tag="ffcol", bufs=16)
            A_col = col_tile(bnA[l, si], KX, tag="ffcol", bufs=16)
            B_col = col_tile(bnB[l, si], KX, tag="ffcol", bufs=16)
            g_col = col_tile(lng[l, 3 if s == "b" else 4], KE, tag="lncol", bufs=16)
            bb_col = col_tile(lnb[l, 3 if s == "b" else 4], KE, tag="lncol", bufs=16)
            yield
            PHASES.append((f"ffn{l}.{s}", len(nc.inst_map)))

            hts = [None] * NC
            xts = [None] * NC
            hl0 = [None] * NC
            hf2 = [None] * NC

            fpf = [None] * NC

            def compute_h(ci):
                if fpf[ci] is None:
                    fpf[ci] = load_x_chunk(src, ci)
                if ci + 1 < NC:
                    fpf[ci + 1] = load_x_chunk(src, ci + 1)
                xt = fpf[ci]
                xts[ci] = xt
                ht = pb.tile([128, KX, C], BF16, tag="a8")
                for m in range(KX):
                    ph = ps.tile([128, C], F32, tag="mm")
                    for k in range(KE):
                        mm(ph, w1t[:, k, m * 128:(m + 1) * 128], xt[:, k, :],
                           start=(k == 0), stop=(k == KE - 1))
                    hf = pc.tile([128, C], BF16, tag="a1")
                    sc.activation(hf, ph, AF.Relu, bias=b1_col[:, m:m + 1])
                    v.tensor_scalar_min(ht[:, m, :], hf, 6.0)
                hts[ci] = ht
                l0 = phl.tile([128, KX, 1], BF16, tag="hl")
                f2 = phl.tile([128, KX, 1], BF16, tag="hf")
                for m in range(KX):
                    v.tensor_scalar_mul(l0[:, m, :], ht[:, m, C - 1:C],
                                        w0_col[:, m:m + 1])
                    v.tensor_scalar_mul(f2[:, m, :], ht[:, m, 0:1],
                                        w2_col[:, m:m + 1])
                hl0[ci], hf2[ci] = l0, f2

            def conv_elem(ci):
                ht = hts[ci]
                h2 = pb.tile([128, KX, C], BF16, tag="a8")
                for m in range(KX):
                    acc = pc.tile([128, C], BF16, tag="a1")
                    v.tensor_scalar_mul(acc, ht[:, m, :], w1c_col[:, m:m + 1])
                    v.scalar_tensor_tensor(out=acc[:, 1:C],
                                           in0=ht[:, m, 0:C - 1],
                                           scalar=w0_col[:, m:m + 1],
                                           in1=acc[:, 1:C],
                                           op0=ALU.mult, op1=ALU.add)
                    if ci > 0:
                        v.tensor_tensor(out=acc[:, 0:1], in0=acc[:, 0:1],
                                        in1=hl0[ci - 1][:, m, :], op=ALU.add)
                    v.scalar_tensor_tensor(out=acc[:, 0:C - 1],
                                           in0=ht[:, m, 1:C],
                                           scalar=w2_col[:, m:m + 1],
                                           in1=acc[:, 0:C - 1],
                                           op0=ALU.mult, op1=ALU.add)
                    if ci < NC - 1:
                        v.tensor_tensor(out=acc[:, C - 1:C],
                                        in0=acc[:, C - 1:C],
                                        in1=hf2[ci + 1][:, m, :], op=ALU.add)
                    a2 = pc.tile([128, C], BF16, tag="a1")
                    sc.activation(a2, acc, AF.Relu, scale=A_col[:, m:m + 1],
                                  bias=B_col[:, m:m + 1])
                    v.tensor_scalar_min(h2[:, m, :], a2, 6.0)
                return h2

            def conv_pw(ci, h2):
                rt = pa.tile([128, KE, C], BF16, tag="a4")
                for m in range(KE):
                    pw = ps.tile([128, C], F32, tag="mm")
                    for k in range(KX):
                        mm(pw, w2t[:, k, m * 128:(m + 1) * 128], h2[:, k, :],
                           start=(k == 0), stop=(k == KX - 1))
                    v.scalar_tensor_tensor(out=rt[:, m, :], in0=pw,
                                           scalar=b2_col[:, m:m + 1],
                                           in1=xts[ci][:, m, :],
                                           op0=ALU.add, op1=ALU.add)
                outt = pa.tile([128, KE, C], BF16, tag="a4")
                ln_apply(rt, g_col, bb_col, outt)
                store_chunk(dst, ci, outt)
                hts[ci] = xts[ci] = None

            compute_h(0)
            yield "h"
            compute_h(1)
            yield "h"
            for ci in range(NC - 1):
                h2 = conv_elem(ci)
                yield "cv"
                conv_pw(ci, h2)
                yield "pw"
                if ci + 2 < NC:
                    compute_h(ci + 2)
                    yield "h"
            h2 = conv_elem(NC - 1)
            yield "cv"
            conv_pw(NC - 1, h2)
            yield "pw"

        # ---- layers (chunk-interleaved across independent streams) ----
        def adv(g, n=1):
            for _ in range(n):
                next(g, None)

        for l in range(L):
            bsrc = rs["b", 0] if l == 0 else rs["b", (l - 1, 3)]
            lsrc = rs["l", 0] if l == 0 else rs["l", (l - 1, 3)]
            g0 = attn_gen(l, 0, bsrc, bsrc,
                          *make_self_tail(l, "b", rs["b", (l, 1)]))
            g1 = attn_gen(l, 1, lsrc, lsrc,
                          *make_self_tail(l, "l", rs["l", (l, 1)]))
            g2 = attn_gen(l, 2, rs["b", (l, 1)], rs["l", (l, 1)],
                          *make_cross_tail(l, "b", rs["b", (l, 2)]))
            g3 = attn_gen(l, 3, rs["l", (l, 1)], rs["b", (l, 1)],
                          *make_cross_tail(l, "l", rs["l", (l, 2)]))
            gb = ffn_gen(l, "b", rs["b", (l, 2)], rs["b", (l, 3)])
            gl = ffn_gen(l, "l", rs["l", (l, 2)], rs["l", (l, 3)])
            adv(g0)                     # weights
            adv(g1)
            for _ in range(2 * NC):     # self alphas: a1/a2 staged
                adv(g0)
                adv(g1)
            adv(g0)                     # bd/kmm
            adv(g1)
            adv(g2)                     # prefetch cross weights
            adv(g3)
            for _ in range(3 * NC):     # self betas: b1/b2/b3 staged
                adv(g0)
                adv(g1)
            for _ in range(2 * NC):     # cross alphas staged
                adv(g2)
                adv(g3)
            adv(g2)                     # bd/kmm
            adv(g3)
            adv(gb)                     # prefetch ffn weights
            adv(gl)
            for _ in range(3 * NC):     # cross betas staged
                adv(g2)
                adv(g3)
            for _ in range(12):         # ffn h/cv/pw staged
                adv(gb)
                adv(gl)

        PHASES.append(("final", len(nc.inst_map)))
        # ---- final head ----
        fw1t = wbig.tile([128, 2 * KE, E2], BF16, tag="w", bufs=6)
        nc.sync.dma_start(out=fw1t,
                          in_=fw1.rearrange("(k p) g -> p k g", p=128))
        fw2t = wsm.tile([128, 2, E], BF16, tag="wfin", bufs=4)
        nc.sync.dma_start(out=fw2t,
                          in_=fw2.rearrange("(k p) e -> p k e", p=128))
        rw1t = wsm.tile([128, KE, E4], BF16, tag="wfin", bufs=4)
        nc.sync.dma_start(out=rw1t,
                          in_=rw1.rearrange("(k p) g -> p k g", p=128))
        rw2t = wsm.tile([128, E8], BF16, tag="wfin", bufs=4)
        nc.sync.dma_start(out=rw2t, in_=rw2)
        rw3t = wsm.tile([E8, 16], BF16, tag="wfin", bufs=4)
        nc.sync.dma_start(out=rw3t, in_=rw3p)
        rb3_row = row_tile(rb3p, 16)
        fb2_col = col_tile(fb2f, KE, tag="fcol")
        fb1_col = col_tile(fb1, 2, tag="fcol")
        flng_col = col_tile(flng, KE, tag="fcol")
        flnb_col = col_tile(flnb, KE, tag="fcol")
        rb1_col = col_tile(rb1, 1, tag="fcol")
        rb2_col = wcol.tile([E8, 1], F32, tag="fcol")
        nc.sync.dma_start(out=rb2_col, in_=rb2[:, None])
        out_ap = out_dram.ap()

        bsrc, lsrc = rs["b", (L - 1, 3)], rs["l", (L - 1, 3)]

        def final_stage1(ci):
            xb = load_x_chunk(bsrc, ci)
            xl = load_x_chunk(lsrc, ci)
            f1t = pa.tile([128, 2, C], BF16, tag="a4")
            for m in range(2):
                pf = ps.tile([128, C], F32, tag="mm")
                for k in range(2 * KE):
                    rhs = xb[:, k, :] if k < KE else xl[:, k - KE, :]
                    mm(pf, fw1t[:, k, m * 128:(m + 1) * 128], rhs,
                       start=(k == 0), stop=(k == 2 * KE - 1))
                f1 = pc.tile([128, C], BF16, tag="a1")
                sc.activation(f1, pf, AF.Relu, bias=fb1_col[:, m:m + 1])
                v.tensor_scalar_min(f1t[:, m, :], f1, 6.0)
            ft = pa.tile([128, KE, C], BF16, tag="a4")
            for m in range(KE):
                pf2 = ps.tile([128, C], F32, tag="mm")
                for k in range(2):
                    mm(pf2, fw2t[:, k, m * 128:(m + 1) * 128], f1t[:, k, :],
                       start=(k == 0), stop=(k == 1))
                sc.activation(ft[:, m, :], pf2, AF.Identity,
                              bias=fb2_col[:, m:m + 1])
            frt = pa.tile([128, KE, C], BF16, tag="a4")
            ln_apply(ft, flng_col, flnb_col, frt)
            # relu after LN
            frf = frt.rearrange("p k c -> p (k c)")
            v.tensor_scalar_max(frf, frf, 0.0)
            return frt

        def final_stage2(ci, frt):
            p1 = ps.tile([128, C], F32, tag="mm")
            for k in range(KE):
                mm(p1, rw1t[:, k, :], frt[:, k, :], start=(k == 0),
                   stop=(k == KE - 1))
            h1f = pc.tile([128, C], BF16, tag="a1")
            sc.activation(h1f, p1, AF.Relu, bias=rb1_col[:, 0:1])
            h1t = pc.tile([128, C], BF16, tag="a1")
            v.tensor_scalar_min(h1t, h1f, 6.0)
            p2 = ps.tile([E8, C], F32, tag="mm")
            mm(p2, rw2t, h1t, start=True, stop=True)
            h2f = pc.tile([E8, C], BF16, tag="a1")
            sc.activation(h2f, p2, AF.Relu, bias=rb2_col[:, 0:1])
            h2t = pc.tile([E8, C], BF16, tag="a1")
            v.tensor_scalar_min(h2t, h2f, 6.0)
            ot = pc.tile([128, NTT, c.OUT], F32, tag="a1")
            for tt in range(NTT):
                p3 = ps.tile([128, 16], F32, tag="mm")
                mm(p3, h2t[:, tt * 128:(tt + 1) * 128], rw3t,
                   start=True, stop=False)
                mm(p3, ONES_ROW, rb3_row, start=False, stop=True)
                sc.activation(ot[:, tt, :], p3[:, 0:c.OUT], AF.Copy)
            nc.sync.dma_start(
                out=out_ap[ci * C:(ci + 1) * C, :].rearrange(
                    "(tt p) o -> p tt o", p=128),
                in_=ot)

        frts = [None] * NC
        for ci in range(NC):
            frts[ci] = final_stage1(ci)
            if ci >= 1:
                final_stage2(ci - 1, frts[ci - 1])
        final_stage2(NC - 1, frts[NC - 1])

    return din, out_dram


# ======================================================================
# kernel() entry point: full inputs in, full outputs out (8-core SPMD).
# ======================================================================
import concourse.bacc as _bacc
from concourse.bass_utils import run_bass_kernel_spmd as _run_spmd

_N_CORES = 8
_CACHE = {}


def _get_nc():
    if "nc" not in _CACHE:
        nc = _bacc.Bacc("TRN2", target_bir_lowering=False, debug=False)
        build(nc, Cfg())
        nc.finalize()
        _CACHE["nc"] = nc
    return _CACHE["nc"]


def _bf16(x):
    import ml_dtypes
    return np.asarray(x, dtype=np.float32).astype(ml_dtypes.bfloat16)


def host_prep(inputs):
    """Host-side weight preprocessing: compose QKV, fold BN, convert bf16."""
    c = Cfg()
    E, X, H, L = c.E, c.X, c.H, c.L
    E4, E2, E8 = E // 4, E // 2, E // 8
    f = {k: np.asarray(v, dtype=np.float32) for k, v in inputs.items()}
    dw, uw, ub = f["dw"], f["uw"], f["ub"]
    # composed q and k|v projection weights
    wq = np.matmul(dw[:, :, 0], uw[:, :, 0])          # (L,4,E,E)
    wk = np.matmul(dw[:, :, 1], uw[:, :, 1])
    wv = np.matmul(dw[:, :, 2], uw[:, :, 2])
    wkv = np.concatenate([wk, wv], axis=-1)           # (L,4,E,2E)
    ubq = ub[:, :, 0]                                 # (L,4,E)
    ubkv = np.concatenate([ub[:, :, 1], ub[:, :, 2]], axis=-1)
    rsq = np.float32(1.0 / np.sqrt(1.0 + BN_EPS))
    A = f["bng"] * rsq                                # (L,2,X)
    B = f["cb"] * A + f["bnb"]
    cwf = f["cw"].transpose(0, 1, 3, 2).copy()        # (L,2,3,X)
    gwd = f["gw2"][:, :, 0] - f["gw2"][:, :, 1]       # (L,E4)
    gb2d = (f["gb2"][:, 0] - f["gb2"][:, 1])[:, None]  # (L,1)
    rw3p = np.zeros((E8, 16), np.float32)
    rw3p[:, :c.OUT] = f["rw3"]
    rb3p = np.zeros((16,), np.float32)
    rb3p[:c.OUT] = f["rb3"]
    dh = E // H
    ident = np.eye(128, dtype=np.float32)
    ones = np.ones((128, 128), dtype=np.float32)
    hmask = np.zeros((E, H), dtype=np.float32)
    for ff in range(E):
        hmask[ff, ff // dh] = 1.0
    cmask = hmask.T.copy()

    b16 = dict(wq=wq, wkv=wkv, ubkv=ubkv, ow=f["ow"],
               w1=f["w1"], w2=f["w2"],
               gw1=f["gw1"], gwd=gwd, fw1=f["fw1"], fw2=f["fw2"],
               rw1=f["rw1"], rw2=f["rw2"], rw3p=rw3p,
               rb3p=rb3p, ident=ident, ones128=ones, hmask=hmask,
               cmask=cmask)
    f32 = dict(ubq=ubq, ob=f["ob"], b1=f["b1"], b2=f["b2"], fb2=f["fb2"],
               cwf=cwf, bnA=A, bnB=B,
               lng=f["lng"], lnb=f["lnb"],
               gb1=f["gb1"], gb2d=gb2d, fb1=f["fb1"], flng=f["flng"],
               flnb=f["flnb"], rb1=f["rb1"], rb2=f["rb2"])
    shared = {k: _bf16(v) for k, v in b16.items()}
    shared.update({k: np.ascontiguousarray(v, dtype=np.float32)
                   for k, v in f32.items()})
    return shared, f["body_feats"], f["limb_feats"]


def kernel(**inputs):
    nc = _get_nc()
    shared, body, limb = host_prep(inputs)
    in_maps = []
    for i in range(_N_CORES):
        m = dict(shared)
        m["body_feats"] = _bf16(body[i])
        m["limb_feats"] = _bf16(limb[i])
        in_maps.append(m)
    res = run_kernel_spmd_cached(nc, in_maps)
    out = np.stack([res[i]["out"] for i in range(_N_CORES)], axis=0)
    return out.astype(np.float32)


def run_kernel_spmd_cached(nc, in_maps, **kw):
    r = _run_spmd(nc, in_maps, list(range(_N_CORES)), **kw)
    _CACHE["last_result"] = r
    return r.results


# revision 11
# speedup vs baseline: 1.2541x; 1.0255x over previous
"""Dual-stream linear-attention transformer (per-core, bf16).

Layout: "layout 1" activations [E, N] feature-major; SBUF tiles
[128, KE, C] (feature f = 128*k + p -> partition p, slice k; tokens on
the free dim). Alpha k/v are produced token-major per 128-token tile.
Residual streams live in internal DRAM as [E, N] bf16; the network
inputs are pre-transposed to [E, N] on the host so there is no entry
transpose phase.

Design notes:
  - All matmul operands + SBUF activations bf16 (1 PE cycle/row like
    f32r>=256, but DVE tensor_tensor 2x / tensor_scalar 4x, half DMA).
  - QKV down+up projections composed into E x E / E x 2E weights on the
    host; BN folded into A,B; biases folded into ACT-bias / DVE
    scalar_tensor_tensor ops (no rank-1 bias matmuls).
  - elu+1 = min(exp,1)+relu via ACT Exp, ACT Relu, DVE STT(min,add).
  - LayerNorm: sums via ONES_COL matmuls into PSUM rows, rsqrt via a
    single ACT Abs_reciprocal_sqrt, row broadcast via gpsimd
    partition_broadcast, apply via 3 bf16 DVE ops per slice.
  - attn/ffn emitted by generators yielding at sub-chunk stages; a
    round-robin scheduler interleaves the two independent streams of
    each phase pair (self/self, cross/cross, ffn-b/ffn-l) so each
    engine's in-order queue always holds ready work from the paired
    stream when one stream stalls on a dependency.
  - x-chunk DMA loads prefetched one chunk ahead; stores ride SWDGE.
  - m-outer matmul groups: one PSUM bank live per group (ps bufs=6).
"""

from dataclasses import dataclass
from contextlib import ExitStack

import numpy as np

import concourse.bass as bass
import concourse.mybir as mybir
import concourse.tile as tile

F32 = mybir.dt.float32
BF16 = mybir.dt.bfloat16
AF = mybir.ActivationFunctionType
ALU = mybir.AluOpType

LN_EPS = 1e-5
BN_EPS = 1e-5


@dataclass
class Cfg:
    N: int = 2048
    E: int = 512
    R: int = 256
    X: int = 1024
    H: int = 8
    L: int = 3
    OUT: int = 15
    C: int = 512

    @property
    def KE(self):
        return self.E // 128

    @property
    def KX(self):
        return self.X // 128

    @property
    def NC(self):
        return self.N // self.C

    @property
    def NTT(self):
        return self.C // 128


PHASES = []


def build(nc, cfg):
    c = cfg
    E, X, H, N, C, L = c.E, c.X, c.H, c.N, c.C, c.L
    KE, KX, NC, NTT = c.KE, c.KX, c.NC, c.NTT
    E4, E2, E8 = E // 4, E // 2, E // 8

    din = {}

    def inp(name, shape, dt=BF16):
        din[name] = nc.dram_tensor(name, list(shape), dt, kind="ExternalInput")
        return din[name].ap()

    # activations (host converts to bf16 and pre-transposes to [E, N])
    body_feats = inp("body_feats", (E, N))
    limb_feats = inp("limb_feats", (E, N))
    # attention weights (host-composed)
    wq = inp("wq", (L, 4, E, E))
    wkv = inp("wkv", (L, 4, E, 2 * E))
    ubq = inp("ubq", (L, 4, E), F32)
    ubkv = inp("ubkv", (L, 4, 2 * E))
    ow = inp("ow", (L, 4, E, E))
    obf = inp("ob", (L, 4, E), F32)
    # FFN
    w1 = inp("w1", (L, 2, E, X))
    b1f = inp("b1", (L, 2, X), F32)
    cwf = inp("cwf", (L, 2, 3, X), F32)     # conv taps, tap-major
    bnA = inp("bnA", (L, 2, X), F32)        # bng*rsqrt(1+eps)
    bnB = inp("bnB", (L, 2, X), F32)        # cb*A + bnb
    w2 = inp("w2", (L, 2, X, E))
    b2f = inp("b2", (L, 2, E), F32)
    lng = inp("lng", (L, 5, E), F32)
    lnb = inp("lnb", (L, 5, E), F32)
    # gating
    gw1 = inp("gw1", (L, 2 * E, E4))
    gb1 = inp("gb1", (L, E4), F32)
    gwd = inp("gwd", (L, E4))               # gw2[:,0]-gw2[:,1]
    gb2d = inp("gb2d", (L, 1), F32)         # gb2[0]-gb2[1]
    # final head
    fw1 = inp("fw1", (2 * E, E2))
    fb1 = inp("fb1", (E2,), F32)
    fw2 = inp("fw2", (E2, E))
    fb2f = inp("fb2", (E,), F32)
    flng = inp("flng", (E,), F32)
    flnb = inp("flnb", (E,), F32)
    rw1 = inp("rw1", (E, E4))
    rb1 = inp("rb1", (E4,), F32)
    rw2 = inp("rw2", (E4, E8))
    rb2 = inp("rb2", (E8,), F32)
    rw3p = inp("rw3p", (E8, 16))            # zero-padded to 16
    rb3p = inp("rb3p", (16,))               # zero-padded
    ident_in = inp("ident", (128, 128))
    ones_in = inp("ones128", (128, 128))
    hmask_in = inp("hmask", (E, H))
    cmask_in = inp("cmask", (H, E))

    out_dram = nc.dram_tensor("out", [N, c.OUT], F32, kind="ExternalOutput")

    def idram(name):
        return nc.dram_tensor(name, [E, N], BF16).ap().rearrange(
            "(k p) n -> p k n", p=128)

    rs = {}
    for s in ("b", "l"):
        for l in range(L):
            for st in (1, 2, 3):
                rs[s, (l, st)] = idram(f"r{s}_{l}_{st}")
    rs["b", 0] = body_feats.rearrange("(k p) n -> p k n", p=128)
    rs["l", 0] = limb_feats.rearrange("(k p) n -> p k n", p=128)

    lowp = nc.allow_low_precision("bf16 activations within rel-err budget")

    with tile.TileContext(nc) as tc, ExitStack() as ctx, lowp:
        p_ = ctx.enter_context
        cst = p_(tc.tile_pool(name="cst", bufs=1))
        wbig = p_(tc.tile_pool(name="wbig", bufs=3))
        wsm = p_(tc.tile_pool(name="wsm", bufs=2))
        wcol = p_(tc.tile_pool(name="wcol", bufs=10))
        wrow = p_(tc.tile_pool(name="wrow", bufs=6))
        pa = p_(tc.tile_pool(name="pa", bufs=9))      # 4KB bf16 chunk tiles
        pb = p_(tc.tile_pool(name="pb", bufs=5))      # 8KB ht tiles
        pc = p_(tc.tile_pool(name="pc", bufs=12))      # 1KB bf16 / rows
        pat = p_(tc.tile_pool(name="pat", bufs=3))    # per-attn persistents
        phl = p_(tc.tile_pool(name="phl", bufs=6))    # conv halos
        ps = p_(tc.tile_pool(name="ps", bufs=6, space="PSUM"))
        psr = p_(tc.tile_pool(name="psr", bufs=2, space="PSUM"))

        v, sc, gp = nc.vector, nc.scalar, nc.gpsimd

        def mm(out, lhsT, rhs, start, stop):
            nc.tensor.matmul(out, lhsT, rhs, start=start, stop=stop)

        # ---- constants ----
        ident_t = cst.tile([128, 128], BF16, tag="ident")
        nc.sync.dma_start(out=ident_t, in_=ident_in)
        ones_t = cst.tile([128, 128], BF16, tag="ones")
        nc.sync.dma_start(out=ones_t, in_=ones_in)
        hmask_t = cst.tile([128, KE, H], BF16, tag="hmask")
        nc.sync.dma_start(out=hmask_t,
                          in_=hmask_in.rearrange("(k p) h -> p k h", p=128))
        cmask_t = cst.tile([H, KE, 128], BF16, tag="cmask")
        nc.sync.dma_start(out=cmask_t,
                          in_=cmask_in.rearrange("h (k p) -> h k p", p=128))
        ONES_COL = ones_t[:, 0:1]
        ONES_ROW = ones_t[0:1, :]
        onesc_t = cst.tile([1, C], BF16, tag="onesc")
        v.memset(onesc_t, 1.0)
        ONES_C = onesc_t[0:1, :]
        eps_ln = cst.tile([1, 1], F32, tag="epsl")
        v.memset(eps_ln, LN_EPS)

        def col_tile(src_ap, m, tag="col", bufs=None):
            t = wcol.tile([128, m], F32, tag=tag,
                          bufs=(12 if tag == "col" else bufs))
            nc.sync.dma_start(out=t, in_=src_ap.rearrange("(m p) -> p m", p=128))
            return t

        def row_tile(src_ap, n, tag="row", pool=None):
            t = (pool or wrow).tile([1, n], BF16, tag=tag)
            nc.sync.dma_start(out=t, in_=src_ap[None, :])
            return t

        def ln_apply(xs, g_col, b_col, outt, extra_tt=None):
            """LayerNorm over features (layout 1). xs: [128, KE, C] bf16 tile.
            outt: [128, KE, C] bf16 out. extra_tt(m): None."""
            sq = pa.tile([128, KE, C], BF16, tag="a4")
            xf = xs.rearrange("p k c -> p (k c)")
            v.tensor_tensor(out=sq.rearrange("p k c -> p (k c)"),
                            in0=xf, in1=xf, op=ALU.mult)
            ps_s = psr.tile([1, C], F32, tag="row")
            ps_ss = psr.tile([1, C], F32, tag="row")
            for m in range(KE):
                mm(ps_s, ONES_COL, xs[:, m, :], start=(m == 0),
                   stop=(m == KE - 1))
                mm(ps_ss, ONES_COL, sq[:, m, :], start=(m == 0),
                   stop=(m == KE - 1))
            msq = pc.tile([1, C], F32, tag="row", bufs=8)
            sc.activation(msq, ps_ss, AF.Copy, scale=1.0 / E)
            m2 = pc.tile([1, C], F32, tag="row", bufs=8)
            sc.activation(m2, ps_s, AF.Square, scale=1.0 / E)
            var = pc.tile([1, C], F32, tag="row", bufs=8)
            v.tensor_tensor(out=var, in0=msq, in1=m2, op=ALU.subtract)
            srow = pc.tile([1, C], BF16, tag="rowh", bufs=8)
            sc.activation(srow, var, AF.Abs_reciprocal_sqrt,
                          bias=eps_ln[0:1, 0:1])
            trow = pc.tile([1, C], BF16, tag="rowh", bufs=8)
            v.scalar_tensor_tensor(out=trow, in0=ps_s, scalar=1.0 / E,
                                   in1=srow, op0=ALU.mult, op1=ALU.mult)
            sb_s = pc.tile([128, C], BF16, tag="a1")
            gp.partition_broadcast(sb_s, srow)
            sb_t = pc.tile([128, C], BF16, tag="a1")
            gp.partition_broadcast(sb_t, trow)
            for m in range(KE):
                u = pc.tile([128, C], BF16, tag="a1")
                v.tensor_tensor(out=u, in0=xs[:, m, :], in1=sb_s, op=ALU.mult)
                v.tensor_tensor(out=u, in0=u, in1=sb_t, op=ALU.subtract)
                v.tensor_scalar(out=outt[:, m, :], in0=u,
                                scalar1=g_col[:, m:m + 1],
                                scalar2=b_col[:, m:m + 1],
                                op0=ALU.mult, op1=ALU.add)

        def load_x_chunk(dram_l1, ci, tag="a4"):
            xt = pa.tile([128, KE, C], BF16, tag=tag)
            nc.sync.dma_start(out=xt, in_=dram_l1[:, :, ci * C:(ci + 1) * C])
            return xt

        def store_chunk(dram_l1, ci, t):
            gp.dma_start(out=dram_l1[:, :, ci * C:(ci + 1) * C], in_=t)


        # ---- linear attention ----
        def attn_gen(l, a, xq_dram, xkv_dram, tail_m, tail_post):
            """Generator: yields after weight loads, after each alpha chunk
            (bd/kmm ride with the last), and after each beta chunk."""
            wqt = wbig.tile([128, KE, E], BF16, tag="w", bufs=6)
            nc.sync.dma_start(
                out=wqt, in_=wq[l, a].rearrange("(k p) e -> p k e", p=128))
            wkvt = wbig.tile([128, KE, 2 * E], BF16, tag="w2x", bufs=4)
            nc.sync.dma_start(
                out=wkvt, in_=wkv[l, a].rearrange("(k p) e -> p k e", p=128))
            owt = wbig.tile([128, KE, E], BF16, tag="w", bufs=6)
            nc.sync.dma_start(
                out=owt, in_=ow[l, a].rearrange("(k p) e -> p k e", p=128))
            ubq_col = col_tile(ubq[l, a], KE)
            ubkv_row = row_tile(ubkv[l, a], 2 * E)
            ob_col = col_tile(obf[l, a], KE)
            yield

            PHASES.append((f"attn{l}.{a}.alpha", len(nc.inst_map)))
            kv_acc = pat.tile([128, KE, 258], F32, tag="kva", bufs=2)

            xpf = [None] * NC
            xpf[0] = load_x_chunk(xkv_dram, 0)
            for ci in range(NC):
                if ci + 1 < NC:
                    xpf[ci + 1] = load_x_chunk(xkv_dram, ci + 1)
                xt = xpf[ci]
                xpf[ci] = None
                k2f = pa.tile([128, NTT, E], BF16, tag="a4")
                v2x = pa.tile([128, NTT, 2, 258], BF16, tag="a4")
                v.memset(v2x[:, :, :, 256:258], 1.0)
                for tt in range(NTT):
                    xs = xt[:, :, tt * 128:(tt + 1) * 128]
                    pk = ps.tile([128, E], F32, tag="mm")
                    pv = ps.tile([128, E], F32, tag="mm")
                    for k in range(KE):
                        mm(pk, xs[:, k, :], wkvt[:, k, 0:E],
                           start=(k == 0), stop=False)
                        mm(pv, xs[:, k, :], wkvt[:, k, E:2 * E],
                           start=(k == 0), stop=False)
                    mm(pk, ONES_ROW, ubkv_row[:, 0:E], start=False, stop=True)
                    mm(pv, ONES_ROW, ubkv_row[:, E:2 * E], start=False,
                       stop=True)
                    ee = pc.tile([128, E], BF16, tag="a1")
                    rr = pc.tile([128, E], BF16, tag="a1")
                    sc.activation(ee, pk, AF.Exp)
                    sc.activation(rr, pk, AF.Relu)
                    v.scalar_tensor_tensor(out=k2f[:, tt, :], in0=ee,
                                           scalar=1.0, in1=rr,
                                           op0=ALU.min, op1=ALU.add)
                    sc.activation(v2x[:, tt, 0, 0:256], pv[:, 0:256], AF.Copy)
                    sc.activation(v2x[:, tt, 1, 0:256], pv[:, 256:512], AF.Copy)
                yield "a1"
                for p in range(4):
                    pkv = ps.tile([128, 258], F32, tag="mm")
                    for tt in range(NTT):
                        mm(pkv, k2f[:, tt, p * 128:(p + 1) * 128],
                           v2x[:, tt, p // 2, :],
                           start=(tt == 0), stop=(tt == NTT - 1))
                    if ci == 0:
                        sc.activation(kv_acc[:, p, :], pkv, AF.Copy)
                    else:
                        v.tensor_tensor(out=kv_acc[:, p, :],
                                        in0=kv_acc[:, p, :], in1=pkv,
                                        op=ALU.add)
                yield "a2"

            bd = pat.tile([128, KE, 128], BF16, tag="bd", bufs=2)
            v.memset(bd, 0.0)
            for p in range(4):
                h0c = (2 * p % 4) * 64
                h1c = ((2 * p + 1) % 4) * 64
                v.tensor_copy(bd[0:64, p, 0:64], kv_acc[0:64, p, h0c:h0c + 64])
                v.tensor_copy(bd[64:128, p, 64:128],
                              kv_acc[64:128, p, h1c:h1c + 64])
            kmm = pat.tile([128, KE, H], BF16, tag="km")
            for k in range(KE):
                v.tensor_scalar_mul(kmm[:, k, :], hmask_t[:, k, :],
                                    kv_acc[:, k, 256:257])
            yield "bd"

            PHASES.append((f"attn{l}.{a}.beta", len(nc.inst_map)))
            qpf = [None] * NC
            qpf[0] = load_x_chunk(xq_dram, 0)
            for ci in range(NC):
                if ci + 1 < NC:
                    qpf[ci + 1] = load_x_chunk(xq_dram, ci + 1)
                xq = qpf[ci]
                qpf[ci] = None
                qf = pa.tile([128, KE, C], BF16, tag="a4")
                for m in range(KE):
                    pq = ps.tile([128, C], F32, tag="mm")
                    for k in range(KE):
                        mm(pq, wqt[:, k, m * 128:(m + 1) * 128], xq[:, k, :],
                           start=(k == 0), stop=(k == KE - 1))
                    ee = pc.tile([128, C], BF16, tag="a1")
                    rr = pc.tile([128, C], BF16, tag="a1")
                    sc.activation(ee, pq, AF.Exp, bias=ubq_col[:, m:m + 1])
                    sc.activation(rr, pq, AF.Relu, bias=ubq_col[:, m:m + 1])
                    v.scalar_tensor_tensor(out=qf[:, m, :], in0=ee, scalar=1.0,
                                           in1=rr, op0=ALU.min, op1=ALU.add)
                yield "b1"
                pd = psr.tile([8, C], F32, tag="row")
                for k in range(KE):
                    mm(pd, kmm[:, k, :], qf[:, k, :], start=(k == 0),
                       stop=(k == KE - 1))
                rec = pc.tile([8, C], BF16, tag="a1")
                v.reciprocal(out=rec, in_=pd)
                att = pa.tile([128, KE, C], BF16, tag="a4")
                for m in range(KE):
                    pn = ps.tile([128, C], F32, tag="mm")
                    mm(pn, bd[:, m, :], qf[:, m, :], start=True, stop=True)
                    pr = ps.tile([128, C], F32, tag="mm")
                    mm(pr, cmask_t[:, m, :], rec, start=True, stop=True)
                    rb = pc.tile([128, C], BF16, tag="a1")
                    sc.activation(rb, pr, AF.Copy)
                    v.tensor_tensor(out=att[:, m, :], in0=pn, in1=rb,
                                    op=ALU.mult)
                yield "b2"
                for m in range(KE):
                    pos = ps.tile([128, C], F32, tag="mm")
                    for k in range(KE):
                        mm(pos, owt[:, k, m * 128:(m + 1) * 128],
                           att[:, k, :], start=(k == 0), stop=(k == KE - 1))
                    tail_m(ci, m, pos, xq, ob_col)
                tail_post(ci, xq)
                yield "b3"

        # ---- tails ----
        def make_self_tail(l, s, dst):
            g_col = col_tile(lng[l, 0 if s == "b" else 1], KE, tag="lncol", bufs=16)
            b_col = col_tile(lnb[l, 0 if s == "b" else 1], KE, tag="lncol", bufs=16)
            rt_box = [None]

            def tail_m(ci, m, pos, xq, ob_col):
                if m == 0:
                    rt_box[0] = pa.tile([128, KE, C], BF16, tag="a4",
                                        name="rt")
                v.scalar_tensor_tensor(out=rt_box[0][:, m, :], in0=pos,
                                       scalar=ob_col[:, m:m + 1],
                                       in1=xq[:, m, :],
                                       op0=ALU.add, op1=ALU.add)

            def tail_post(ci, xq):
                rt = rt_box[0]
                outt = pa.tile([128, KE, C], BF16, tag="a4")
                ln_apply(rt, g_col, b_col, outt)
                store_chunk(dst, ci, outt)

            return tail_m, tail_post

        def make_cross_tail(l, s, dst):
            gw1t = wsm.tile([128, 2 * KE, E4], BF16, tag="ws")
            nc.sync.dma_start(out=gw1t,
                              in_=gw1[l].rearrange("(k p) g -> p k g", p=128))
            gwd_col = wcol.tile([128, 1], BF16, tag="gwd")
            nc.sync.dma_start(out=gwd_col, in_=gwd[l][:, None])
            gb1_col = col_tile(gb1[l], 1, tag="lncol", bufs=16)
            gb2d_t = pat.tile([1, 1], F32, tag="gb2d")
            nc.sync.dma_start(out=gb2d_t, in_=gb2d[l][None, :])
            g_col = col_tile(lng[l, 2], KE, tag="lncol", bufs=16)
            b_col = col_tile(lnb[l, 2], KE, tag="lncol", bufs=16)
            proj_box = [None]

            def tail_m(ci, m, pos, xq, ob_col):
                if m == 0:
                    proj_box[0] = pa.tile([128, KE, C], BF16, tag="a4",
                                          name="proj")
                sc.activation(proj_box[0][:, m, :], pos, AF.Identity,
                              bias=ob_col[:, m:m + 1])

            def tail_post(ci, xq):
                proj = proj_box[0]
                pg = ps.tile([128, C], F32, tag="mm")
                for k in range(2 * KE):
                    rhs = xq[:, k, :] if k < KE else proj[:, k - KE, :]
                    mm(pg, gw1t[:, k, :], rhs, start=(k == 0),
                       stop=(k == 2 * KE - 1))
                g1 = pc.tile([128, C], BF16, tag="a1")
                sc.activation(g1, pg, AF.Relu, bias=gb1_col[:, 0:1])
                g1t = pc.tile([128, C], BF16, tag="a1")
                v.tensor_scalar_min(g1t, g1, 6.0)
                pg2 = psr.tile([1, C], F32, tag="row")
                mm(pg2, gwd_col, g1t, start=True, stop=True)
                bg = pc.tile([1, C], BF16, tag="rowh", bufs=8)
                sc.activation(bg, pg2, AF.Sigmoid, bias=gb2d_t[0:1, 0:1])
                bgb = pc.tile([128, C], BF16, tag="a1")
                gp.partition_broadcast(bgb, bg)
                mt = pa.tile([128, KE, C], BF16, tag="a4")
                for m in range(KE):
                    dtmp = pc.tile([128, C], BF16, tag="a1")
                    v.tensor_tensor(out=dtmp, in0=xq[:, m, :],
                                    in1=proj[:, m, :], op=ALU.subtract)
                    v.tensor_tensor(out=dtmp, in0=dtmp, in1=bgb, op=ALU.mult)
                    v.tensor_tensor(out=mt[:, m, :], in0=dtmp,
                                    in1=proj[:, m, :], op=ALU.add)
                outt = pa.tile([128, KE, C], BF16, tag="a4")
                ln_apply(mt, g_col, b_col, outt)
                store_chunk(dst, ci, outt)

            return tail_m, tail_post

        # ---- FFN ----
        def ffn_gen(l, s, src, dst):
            si = 0 if s == "b" else 1
            w1t = wbig.tile([128, KE, X], BF16, tag="w2x", bufs=4)
            nc.sync.dma_start(
                out=w1t, in_=w1[l, si].rearrange("(k p) x -> p k x", p=128))
            w2t = wbig.tile([128, KX, E], BF16, tag="w2x", bufs=4)
            nc.sync.dma_start(
                out=w2t, in_=w2[l, si].rearrange("(k p) e -> p k e", p=128))
            b1_col = col_tile(b1f[l, si], KX, tag="ffcol", bufs=16)
            b2_col = col_tile(b2f[l, si], KE, tag="ffcol", bufs=16)
            w0_col = col_tile(cwf[l, si, 0], KX, tag="ffcol", bufs=16)
            w1c_col = col_tile(cwf[l, si, 1], KX, tag="ffcol", bufs=16)
            w2_col = col_tile(cwf[l, si, 2], KX, tag="ffcol", bufs=16)
            A_col = col_tile(bnA[l, si], KX, tag="ffcol", bufs=16)
            B_col = col_tile(bnB[l, si], KX, tag="ffcol", bufs=16)
            g_col = col_tile(lng[l, 3 if s == "b" else 4], KE, tag="lncol", bufs=16)
            bb_col = col_tile(lnb[l, 3 if s == "b" else 4], KE, tag="lncol", bufs=16)
            yield
            PHASES.append((f"ffn{l}.{s}", len(nc.inst_map)))

            hts = [None] * NC
            xts = [None] * NC
            hl0 = [None] * NC
            hf2 = [None] * NC

            fpf = [None] * NC

            def compute_h(ci):
                if fpf[ci] is None:
                    fpf[ci] = load_x_chunk(src, ci)
                if ci + 1 < NC:
                    fpf[ci + 1] = load_x_chunk(src, ci + 1)
                xt = fpf[ci]
                xts[ci] = xt
                ht = pb.tile([128, KX, C], BF16, tag="a8")
                for m in range(KX):
                    ph = ps.tile([128, C], F32, tag="mm")
                    for k in range(KE):
                        mm(ph, w1t[:, k, m * 128:(m + 1) * 128], xt[:, k, :],
                           start=(k == 0), stop=(k == KE - 1))
                    hf = pc.tile([128, C], BF16, tag="a1")
                    sc.activation(hf, ph, AF.Relu, bias=b1_col[:, m:m + 1])
                    v.tensor_scalar_min(ht[:, m, :], hf, 6.0)
                hts[ci] = ht
                l0 = phl.tile([128, KX, 1], BF16, tag="hl")
                f2 = phl.tile([128, KX, 1], BF16, tag="hf")
                for m in range(KX):
                    v.tensor_scalar_mul(l0[:, m, :], ht[:, m, C - 1:C],
                                        w0_col[:, m:m + 1])
                    v.tensor_scalar_mul(f2[:, m, :], ht[:, m, 0:1],
                                        w2_col[:, m:m + 1])
                hl0[ci], hf2[ci] = l0, f2

            def conv_elem(ci):
                ht = hts[ci]
                h2 = pb.tile([128, KX, C], BF16, tag="a8")
                for m in range(KX):
                    acc = pc.tile([128, C], BF16, tag="a1")
                    v.tensor_scalar_mul(acc, ht[:, m, :], w1c_col[:, m:m + 1])
                    v.scalar_tensor_tensor(out=acc[:, 1:C],
                                           in0=ht[:, m, 0:C - 1],
                                           scalar=w0_col[:, m:m + 1],
                                           in1=acc[:, 1:C],
                                           op0=ALU.mult, op1=ALU.add)
                    if ci > 0:
                        v.tensor_tensor(out=acc[:, 0:1], in0=acc[:, 0:1],
                                        in1=hl0[ci - 1][:, m, :], op=ALU.add)
                    v.scalar_tensor_tensor(out=acc[:, 0:C - 1],
                                           in0=ht[:, m, 1:C],
                                           scalar=w2_col[:, m:m + 1],
                                           in1=acc[:, 0:C - 1],
                                           op0=ALU.mult, op1=ALU.add)
                    if ci < NC - 1:
                        v.tensor_tensor(out=acc[:, C - 1:C],
                                        in0=acc[:, C - 1:C],
                                        in1=hf2[ci + 1][:, m, :], op=ALU.add)
                    a2 = pc.tile([128, C], BF16, tag="a1")
                    sc.activation(a2, acc, AF.Relu, scale=A_col[:, m:m + 1],
                                  bias=B_col[:, m:m + 1])
                    v.tensor_scalar_min(h2[:, m, :], a2, 6.0)
                return h2

            def conv_pw(ci, h2):
                rt = pa.tile([128, KE, C], BF16, tag="a4")
                for m in range(KE):
                    pw = ps.tile([128, C], F32, tag="mm")
                    for k in range(KX):
                        mm(pw, w2t[:, k, m * 128:(m + 1) * 128], h2[:, k, :],
                           start=(k == 0), stop=(k == KX - 1))
                    v.scalar_tensor_tensor(out=rt[:, m, :], in0=pw,
                                           scalar=b2_col[:, m:m + 1],
                                           in1=xts[ci][:, m, :],
                                           op0=ALU.add, op1=ALU.add)
                outt = pa.tile([128, KE, C], BF16, tag="a4")
                ln_apply(rt, g_col, bb_col, outt)
                store_chunk(dst, ci, outt)
                hts[ci] = xts[ci] = None

            compute_h(0)
            yield "h"
            compute_h(1)
            yield "h"
            for ci in range(NC - 1):
                h2 = conv_elem(ci)
                yield "cv"
                conv_pw(ci, h2)
                yield "pw"
                if ci + 2 < NC:
                    compute_h(ci + 2)
                    yield "h"
            h2 = conv_elem(NC - 1)
            yield "cv"
            conv_pw(NC - 1, h2)
            yield "pw"

        # ---- layers (chunk-interleaved across independent streams) ----
        def adv(g, n=1):
            for _ in range(n):
                next(g, None)

        for l in range(L):
            bsrc = rs["b", 0] if l == 0 else rs["b", (l - 1, 3)]
            lsrc = rs["l", 0] if l == 0 else rs["l", (l - 1, 3)]
            g0 = attn_gen(l, 0, bsrc, bsrc,
                          *make_self_tail(l, "b", rs["b", (l, 1)]))
            g1 = attn_gen(l, 1, lsrc, lsrc,
                          *make_self_tail(l, "l", rs["l", (l, 1)]))
            g2 = attn_gen(l, 2, rs["b", (l, 1)], rs["l", (l, 1)],
                          *make_cross_tail(l, "b", rs["b", (l, 2)]))
            g3 = attn_gen(l, 3, rs["l", (l, 1)], rs["b", (l, 1)],
                          *make_cross_tail(l, "l", rs["l", (l, 2)]))
            gb = ffn_gen(l, "b", rs["b", (l, 2)], rs["b", (l, 3)])
            gl = ffn_gen(l, "l", rs["l", (l, 2)], rs["l", (l, 3)])
            adv(g0)                     # weights
            adv(g1)
            for _ in range(2 * NC):     # self alphas: a1/a2 staged
                adv(g0)
                adv(g1)
            adv(g0)                     # bd/kmm
            adv(g1)
            adv(g2)                     # prefetch cross weights
            adv(g3)
            for _ in range(3 * NC):     # self betas: b1/b2/b3 staged
                adv(g0)
                adv(g1)
            for _ in range(2 * NC):     # cross alphas staged
                adv(g2)
                adv(g3)
            adv(g2)                     # bd/kmm
            adv(g3)
            adv(gb)                     # prefetch ffn weights
            adv(gl)
            for _ in range(3 * NC):     # cross betas staged
                adv(g2)
                adv(g3)
            for _ in range(12):         # ffn h/cv/pw staged
                adv(gb)
                adv(gl)

        PHASES.append(("final", len(nc.inst_map)))
        # ---- final head ----
        fw1t = wbig.tile([128, 2 * KE, E2], BF16, tag="w", bufs=6)
        nc.sync.dma_start(out=fw1t,
                          in_=fw1.rearrange("(k p) g -> p k g", p=128))
        fw2t = wsm.tile([128, 2, E], BF16, tag="wfin", bufs=4)
        nc.sync.dma_start(out=fw2t,
                          in_=fw2.rearrange("(k p) e -> p k e", p=128))
        rw1t = wsm.tile([128, KE, E4], BF16, tag="wfin", bufs=4)
        nc.sync.dma_start(out=rw1t,
                          in_=rw1.rearrange("(k p) g -> p k g", p=128))
        rw2t = wsm.tile([128, E8], BF16, tag="wfin", bufs=4)
        nc.sync.dma_start(out=rw2t, in_=rw2)
        rw3t = wsm.tile([E8, 16], BF16, tag="wfin", bufs=4)
        nc.sync.dma_start(out=rw3t, in_=rw3p)
        rb3_row = row_tile(rb3p, 16)
        fb2_col = col_tile(fb2f, KE, tag="fcol")
        fb1_col = col_tile(fb1, 2, tag="fcol")
        flng_col = col_tile(flng, KE, tag="fcol")
        flnb_col = col_tile(flnb, KE, tag="fcol")
        rb1_col = col_tile(rb1, 1, tag="fcol")
        rb2_col = wcol.tile([E8, 1], F32, tag="fcol")
        nc.sync.dma_start(out=rb2_col, in_=rb2[:, None])
        out_ap = out_dram.ap()

        bsrc, lsrc = rs["b", (L - 1, 3)], rs["l", (L - 1, 3)]

        def final_stage1(ci):
            xb = load_x_chunk(bsrc, ci)
            xl = load_x_chunk(lsrc, ci)
            f1t = pa.tile([128, 2, C], BF16, tag="a4")
            for m in range(2):
                pf = ps.tile([128, C], F32, tag="mm")
                for k in range(2 * KE):
                    rhs = xb[:, k, :] if k < KE else xl[:, k - KE, :]
                    mm(pf, fw1t[:, k, m * 128:(m + 1) * 128], rhs,
                       start=(k == 0), stop=(k == 2 * KE - 1))
                f1 = pc.tile([128, C], BF16, tag="a1")
                sc.activation(f1, pf, AF.Relu, bias=fb1_col[:, m:m + 1])
                v.tensor_scalar_min(f1t[:, m, :], f1, 6.0)
            ft = pa.tile([128, KE, C], BF16, tag="a4")
            for m in range(KE):
                pf2 = ps.tile([128, C], F32, tag="mm")
                for k in range(2):
                    mm(pf2, fw2t[:, k, m * 128:(m + 1) * 128], f1t[:, k, :],
                       start=(k == 0), stop=(k == 1))
                sc.activation(ft[:, m, :], pf2, AF.Identity,
                              bias=fb2_col[:, m:m + 1])
            frt = pa.tile([128, KE, C], BF16, tag="a4")
            ln_apply(ft, flng_col, flnb_col, frt)
            # relu after LN
            frf = frt.rearrange("p k c -> p (k c)")
            v.tensor_scalar_max(frf, frf, 0.0)
            return frt

        def final_stage2(ci, frt):
            p1 = ps.tile([128, C], F32, tag="mm")
            for k in range(KE):
                mm(p1, rw1t[:, k, :], frt[:, k, :], start=(k == 0),
                   stop=(k == KE - 1))
            h1f = pc.tile([128, C], BF16, tag="a1")
            sc.activation(h1f, p1, AF.Relu, bias=rb1_col[:, 0:1])
            h1t = pc.tile([128, C], BF16, tag="a1")
            v.tensor_scalar_min(h1t, h1f, 6.0)
            p2 = ps.tile([E8, C], F32, tag="mm")
            mm(p2, rw2t, h1t, start=True, stop=True)
            h2f = pc.tile([E8, C], BF16, tag="a1")
            sc.activation(h2f, p2, AF.Relu, bias=rb2_col[:, 0:1])
            h2t = pc.tile([E8, C], BF16, tag="a1")
            v.tensor_scalar_min(h2t, h2f, 6.0)
            ot = pc.tile([128, NTT, c.OUT], F32, tag="a1")
            for tt in range(NTT):
                p3 = ps.tile([128, 16], F32, tag="mm")
                mm(p3, h2t[:, tt * 128:(tt + 1) * 128], rw3t,
                   start=True, stop=False)
                mm(p3, ONES_ROW, rb3_row, start=False, stop=True)
                sc.activation(ot[:, tt, :], p3[:, 0:c.OUT], AF.Copy)
            nc.sync.dma_start(
                out=out_ap[ci * C:(ci + 1) * C, :].rearrange(
                    "(tt p) o -> p tt o", p=128),
                in_=ot)

        frts = [None] * NC
        for ci in range(NC):
            frts[ci] = final_stage1(ci)
            if ci >= 1:
                final_stage2(ci - 1, frts[ci - 1])
        final_stage2(NC - 1, frts[NC - 1])

    return din, out_dram


# ======================================================================
# kernel() entry point: full inputs in, full outputs out (8-core SPMD).
# ======================================================================
import concourse.bacc as _bacc
from concourse.bass_utils import run_bass_kernel_spmd as _run_spmd

_N_CORES = 8
_CACHE = {}


def _get_nc():
    if "nc" not in _CACHE:
        nc = _bacc.Bacc("TRN2", target_bir_lowering=False, debug=False)
        build(nc, Cfg())
        nc.finalize()
        _CACHE["nc"] = nc
    return _CACHE["nc"]


def _bf16(x):
    import ml_dtypes
    return np.asarray(x, dtype=np.float32).astype(ml_dtypes.bfloat16)


def host_prep(inputs):
    """Host-side weight preprocessing: compose QKV, fold BN, convert bf16."""
    c = Cfg()
    E, X, H, L = c.E, c.X, c.H, c.L
    E4, E2, E8 = E // 4, E // 2, E // 8
    f = {k: np.asarray(v, dtype=np.float32) for k, v in inputs.items()}
    dw, uw, ub = f["dw"], f["uw"], f["ub"]
    # composed q and k|v projection weights
    wq = np.matmul(dw[:, :, 0], uw[:, :, 0])          # (L,4,E,E)
    wk = np.matmul(dw[:, :, 1], uw[:, :, 1])
    wv = np.matmul(dw[:, :, 2], uw[:, :, 2])
    wkv = np.concatenate([wk, wv], axis=-1)           # (L,4,E,2E)
    ubq = ub[:, :, 0]                                 # (L,4,E)
    ubkv = np.concatenate([ub[:, :, 1], ub[:, :, 2]], axis=-1)
    rsq = np.float32(1.0 / np.sqrt(1.0 + BN_EPS))
    A = f["bng"] * rsq                                # (L,2,X)
    B = f["cb"] * A + f["bnb"]
    cwf = f["cw"].transpose(0, 1, 3, 2).copy()        # (L,2,3,X)
    gwd = f["gw2"][:, :, 0] - f["gw2"][:, :, 1]       # (L,E4)
    gb2d = (f["gb2"][:, 0] - f["gb2"][:, 1])[:, None]  # (L,1)
    rw3p = np.zeros((E8, 16), np.float32)
    rw3p[:, :c.OUT] = f["rw3"]
    rb3p = np.zeros((16,), np.float32)
    rb3p[:c.OUT] = f["rb3"]
    dh = E // H
    ident = np.eye(128, dtype=np.float32)
    ones = np.ones((128, 128), dtype=np.float32)
    hmask = np.zeros((E, H), dtype=np.float32)
    for ff in range(E):
        hmask[ff, ff // dh] = 1.0
    cmask = hmask.T.copy()

    b16 = dict(wq=wq, wkv=wkv, ubkv=ubkv, ow=f["ow"],
               w1=f["w1"], w2=f["w2"],
               gw1=f["gw1"], gwd=gwd, fw1=f["fw1"], fw2=f["fw2"],
               rw1=f["rw1"], rw2=f["rw2"], rw3p=rw3p,
               rb3p=rb3p, ident=ident, ones128=ones, hmask=hmask,
               cmask=cmask)
    f32 = dict(ubq=ubq, ob=f["ob"], b1=f["b1"], b2=f["b2"], fb2=f["fb2"],
               cwf=cwf, bnA=A, bnB=B,
               lng=f["lng"], lnb=f["lnb"],
               gb1=f["gb1"], gb2d=gb2d, fb1=f["fb1"], flng=f["flng"],
               flnb=f["flnb"], rb1=f["rb1"], rb2=f["rb2"])
    shared = {k: _bf16(v) for k, v in b16.items()}
    shared.update({k: np.ascontiguousarray(v, dtype=np.float32)
                   for k, v in f32.items()})
    return shared, f["body_feats"], f["limb_feats"]


def kernel(**inputs):
    nc = _get_nc()
    shared, body, limb = host_prep(inputs)
    in_maps = []
    for i in range(_N_CORES):
        m = dict(shared)
        m["body_feats"] = np.ascontiguousarray(_bf16(body[i]).T)
        m["limb_feats"] = np.ascontiguousarray(_bf16(limb[i]).T)
        in_maps.append(m)
    res = run_kernel_spmd_cached(nc, in_maps)
    out = np.stack([res[i]["out"] for i in range(_N_CORES)], axis=0)
    return out.astype(np.float32)


def run_kernel_spmd_cached(nc, in_maps, **kw):
    r = _run_spmd(nc, in_maps, list(range(_N_CORES)), **kw)
    _CACHE["last_result"] = r
    return r.results
